# revision 1
# baseline (speedup 1.0000x reference)
"""Trainium2 Bass kernel for nn_DMLoss (contour matching loss), 8-core data parallel.

Self-contained: only needs the system bass runtime at /opt/trn_rl_repo.

Algorithm (per instance, P=128 points, TIME=10):
  item1: for each pred point, nearest of 1280 interpolated gt points.
    Segment p spans A_p = gt[p-1] .. gt[p]; the 10 candidates are A_p + (t/10)*D_p.
    Within a segment the best discrete t is floor(clamp(10u + 0.5, 0.5, 9.5)) where
    u = <pred - A, D>/|D|^2 (projection).  The per-(pred,segment) best distance is
    evaluated in expanded form on the TensorEngine + a short vector chain, scaled by
    SC and shifted so it is a positive float < 2^24.  The segment index is packed
    into the low 7 bits (after quantizing the distance to multiples of 128), and a
    single free-dim reduce-max over the negated packed value gives argmin+min.
    Selected values are gathered with a one-hot matmul; the exact target is
    recomputed from the gathered segment data.
  item2: same machinery without interpolation (nearest pred point per gt key point).

Output: scalar f32 loss (matches reference()'s masked means).
"""
import sys

for _p in ("/opt/trn_rl_repo",):
    if _p not in sys.path:
        sys.path.insert(0, _p)

import numpy as np

import concourse.bass as bass
import concourse.tile as tile
from concourse import bacc, mybir
from concourse.bass_utils import run_bass_kernel_spmd

dt = mybir.dt
Alu = mybir.AluOpType
Ax = mybir.AxisListType
f32 = np.float32

N_CORES = 8
N, P, TIME = 256, 128, 10
G = N // N_CORES          # instances per core = 32
BG = 4                    # instances per chain batch
NB = G // BG
SC = 131072.0             # distance scale (quantum = 128/SC = 2^-10)
SHIFT = 48.0              # keeps scaled distance positive
BETA = 0.25               # smooth-l1 beta = 1/STRIDE
CQ = float(2 ** 30 + 2 ** 25)   # (x+CQ)-CQ rounds x to multiples of 128 for x in [-3.3e7, -3e6]
CR = 8388607.5                  # (x+CR)-8388608 = round(x-0.5) = floor for our positive ranges


def _build(nc, pc_d, po_d, gc_d, gk_d, mk_d, out_d):
    FP = dt.float32

    with tile.TileContext(nc) as tc:
        with (
            tc.tile_pool(name="const", bufs=1) as cpool,
            tc.tile_pool(name="prep", bufs=1) as prep,
            tc.tile_pool(name="main", bufs=2) as main,
            tc.tile_pool(name="keep", bufs=1) as keep,
            tc.tile_pool(name="ps_mm", bufs=1, space="PSUM") as ps_mm,
            tc.tile_pool(name="ps_one", bufs=2, space="PSUM") as ps_one,
        ):
            # ---------------- constants ----------------
            iota_i = cpool.tile([128, 128], dt.int32)
            nc.gpsimd.iota(iota_i[:], pattern=[[1, 128]], channel_multiplier=0)
            iotaF = cpool.tile([128, 128], FP, tag="iotaF")
            nc.vector.tensor_copy(iotaF[:], iota_i[:])
            iotaC_i = cpool.tile([128, 1], dt.int32)
            nc.gpsimd.iota(iotaC_i[:], pattern=[[0, 1]], channel_multiplier=1)
            iotaC = cpool.tile([128, 1], FP, tag="iotaC")
            nc.vector.tensor_copy(iotaC[:], iotaC_i[:])
            ident = cpool.tile([128, 128], FP, tag="ident")
            nc.vector.tensor_scalar(ident[:], iotaF[:], iotaC[:], None, Alu.is_equal)
            ones1 = cpool.tile([1, 128], FP, tag="ones1")
            nc.gpsimd.memset(ones1[:], 1.0)
            onesc = cpool.tile([128, 1], FP, tag="onesc")
            nc.gpsimd.memset(onesc[:], 1.0)

            # ---------------- seg-layout loads (128 x G) ----------------
            # dst[p, g] = gc[g, p, c]
            def seg_load(dst, dram, c, roll=False):
                if not roll:
                    nc.sync.dma_start(
                        dst[:], dram[:, :, c].rearrange("g p -> p g")
                    )
                else:
                    # dst[p, g] = gc[g, (p-1) % P, c]
                    nc.sync.dma_start(
                        dst[1:P, :], dram[:, 0:P - 1, c].rearrange("g p -> p g")
                    )
                    nc.sync.dma_start(
                        dst[0:1, :], dram[:, P - 1:P, c].rearrange("g p -> p g")
                    )

            gx = prep.tile([128, G], FP, tag="gx")
            gy = prep.tile([128, G], FP, tag="gy")
            ax = prep.tile([128, G], FP, tag="ax")
            ay = prep.tile([128, G], FP, tag="ay")
            pxs = prep.tile([128, G], FP, tag="pxs")
            pys = prep.tile([128, G], FP, tag="pys")
            oxs = prep.tile([128, G], FP, tag="oxs")
            oys = prep.tile([128, G], FP, tag="oys")
            seg_load(gx, gc_d, 0)
            seg_load(gy, gc_d, 1)
            seg_load(ax, gc_d, 0, roll=True)
            seg_load(ay, gc_d, 1, roll=True)
            seg_load(pxs, pc_d, 0)
            seg_load(pys, pc_d, 1)
            seg_load(oxs, po_d, 0)
            seg_load(oys, po_d, 1)

            # ---------------- segment geometry (128 x G ops) ----------------
            dx = prep.tile([128, G], FP, tag="dx")
            dy = prep.tile([128, G], FP, tag="dy")
            nc.vector.tensor_tensor(dx[:], gx[:], ax[:], Alu.subtract)
            nc.vector.tensor_tensor(dy[:], gy[:], ay[:], Alu.subtract)
            t0 = prep.tile([128, G], FP, tag="t0")
            t1 = prep.tile([128, G], FP, tag="t1")
            e = prep.tile([128, G], FP, tag="e")
            nc.vector.tensor_tensor(t0[:], dx[:], dx[:], Alu.mult)
            nc.vector.tensor_tensor(t1[:], dy[:], dy[:], Alu.mult)
            nc.vector.tensor_tensor(e[:], t0[:], t1[:], Alu.add)
            einv = prep.tile([128, G], FP, tag="einv")
            nc.vector.reciprocal(einv[:], e[:])
            a2 = prep.tile([128, G], FP, tag="a2")
            nc.vector.tensor_tensor(t0[:], ax[:], dx[:], Alu.mult)
            nc.vector.tensor_tensor(t1[:], ay[:], dy[:], Alu.mult)
            nc.vector.tensor_tensor(a2[:], t0[:], t1[:], Alu.add)
            zA = prep.tile([128, G], FP, tag="zA")
            nc.vector.tensor_tensor(t0[:], ax[:], ax[:], Alu.mult)
            nc.vector.tensor_tensor(t1[:], ay[:], ay[:], Alu.mult)
            nc.vector.tensor_tensor(zA[:], t0[:], t1[:], Alu.add)

            # rows for the g-matmul: g = 10*u + 0.5
            r0 = prep.tile([128, G], FP, tag="r0")
            r1 = prep.tile([128, G], FP, tag="r1")
            r2 = prep.tile([128, G], FP, tag="r2")
            nc.vector.scalar_tensor_tensor(r0[:], dx[:], 10.0, einv[:], Alu.mult, Alu.mult)
            nc.vector.scalar_tensor_tensor(r1[:], dy[:], 10.0, einv[:], Alu.mult, Alu.mult)
            nc.vector.scalar_tensor_tensor(r2[:], a2[:], -10.0, einv[:], Alu.mult, Alu.mult)
            nc.vector.tensor_scalar(r2[:], r2[:], 0.5, None, Alu.add)
            # rows for the out0-matmul: (-2<p,A> + |A|^2 + SHIFT) * SC
            o0 = prep.tile([128, G], FP, tag="o0")
            o1 = prep.tile([128, G], FP, tag="o1")
            o2 = prep.tile([128, G], FP, tag="o2")
            nc.vector.tensor_scalar(o0[:], ax[:], 2.0 * SC, None, Alu.mult)
            nc.vector.tensor_scalar(o1[:], ay[:], 2.0 * SC, None, Alu.mult)
            nc.vector.tensor_scalar(o2[:], zA[:], SHIFT, -SC, Alu.add, Alu.mult)
            erow = prep.tile([128, G], FP, tag="erow")
            nc.vector.tensor_scalar(erow[:], e[:], -SC / 100.0, None, Alu.mult)
            # rows for the item2 matmul: (-2<k,p> + |p|^2 + SHIFT) * SC
            q0 = prep.tile([128, G], FP, tag="q0")
            q1 = prep.tile([128, G], FP, tag="q1")
            q2 = prep.tile([128, G], FP, tag="q2")
            nc.vector.tensor_scalar(q0[:], pxs[:], 2.0 * SC, None, Alu.mult)
            nc.vector.tensor_scalar(q1[:], pys[:], 2.0 * SC, None, Alu.mult)
            nc.vector.tensor_tensor(t0[:], pxs[:], pxs[:], Alu.mult)
            nc.vector.tensor_tensor(t1[:], pys[:], pys[:], Alu.mult)
            nc.vector.tensor_tensor(q2[:], t0[:], t1[:], Alu.add)
            nc.vector.tensor_scalar(q2[:], q2[:], SHIFT, -SC, Alu.add, Alu.mult)

            # ---------------- gather tables (seg layout) ----------------
            # T1 = [Ax, Ay, Dx, Dy], T2 = [r0, r1, r2, 1], T3 = [px, py, ox, oy]
            T1 = keep.tile([128, G, 4], FP, tag="T1")
            T2 = keep.tile([128, G, 4], FP, tag="T2")
            T3 = keep.tile([128, G, 4], FP, tag="T3")
            for j, src in enumerate((ax, ay, dx, dy)):
                nc.vector.tensor_copy(T1[:, :, j], src[:])
            for j, src in enumerate((r0, r1, r2)):
                nc.vector.tensor_copy(T2[:, :, j], src[:])
            nc.gpsimd.memset(T2[:, :, 3], 1.0)
            for j, src in enumerate((pxs, pys, oxs, oys)):
                nc.vector.tensor_copy(T3[:, :, j], src[:])

            # ---------------- matmul operands ----------------
            # lhsT6[k, g, m]: rows 0-2 = [px, py, 1], rows 3-5 = [kx, ky, 1]
            lhsT6 = keep.tile([6, G, 128], FP, tag="lhsT6")
            nc.gpsimd.memset(lhsT6[:], 1.0)
            nc.sync.dma_start(lhsT6[0:1, :, :], pc_d[:, :, 0].rearrange("(o g) p -> o g p", o=1))
            nc.sync.dma_start(lhsT6[1:2, :, :], pc_d[:, :, 1].rearrange("(o g) p -> o g p", o=1))
            nc.sync.dma_start(lhsT6[3:4, :, :], gk_d[:, :, 0].rearrange("(o g) p -> o g p", o=1))
            nc.sync.dma_start(lhsT6[4:5, :, :], gk_d[:, :, 1].rearrange("(o g) p -> o g p", o=1))

            # rhs tiles (6, G, 128); unused halves zero
            rhsG = keep.tile([6, G, 128], FP, tag="rhsG")
            rhsO = keep.tile([6, G, 128], FP, tag="rhsO")
            rhsK = keep.tile([6, G, 128], FP, tag="rhsK")
            nc.gpsimd.memset(rhsG[:], 0.0)
            nc.gpsimd.memset(rhsO[:], 0.0)
            nc.gpsimd.memset(rhsK[:], 0.0)
            erowR = keep.tile([1, G, 128], FP, tag="erowR")

            # collapse (128, G) seg tiles into single-partition rows via a DRAM
            # bounce (SBUF APs cannot cross partitions; DRAM APs are linear).
            scratch = nc.dram_tensor("rowscratch", [10, 128, G], FP)
            row_moves = [
                (r0, rhsG[0:1, :, :]), (r1, rhsG[1:2, :, :]), (r2, rhsG[2:3, :, :]),
                (o0, rhsO[0:1, :, :]), (o1, rhsO[1:2, :, :]), (o2, rhsO[2:3, :, :]),
                (q0, rhsK[3:4, :, :]), (q1, rhsK[4:5, :, :]), (q2, rhsK[5:6, :, :]),
                (erow, erowR[0:1, :, :]),
            ]
            for idx, (src, dst_row) in enumerate(row_moves):
                nc.sync.dma_start(scratch[idx], src[:])
                nc.sync.dma_start(
                    dst_row, scratch[idx].rearrange("(o p) g -> o g p", o=1)
                )

            # ---------------- pred-layout tail inputs (128 x G) ----------------
            pxP = keep.tile([128, G], FP, tag="pxP")
            pyP = keep.tile([128, G], FP, tag="pyP")
            oxP = keep.tile([128, G], FP, tag="oxP")
            oyP = keep.tile([128, G], FP, tag="oyP")
            kxP = keep.tile([128, G], FP, tag="kxP")
            kyP = keep.tile([128, G], FP, tag="kyP")
            mkP = keep.tile([128, G], FP, tag="mkP")
            nc.sync.dma_start(pxP[:], pc_d[:, :, 0].rearrange("g p -> p g"))
            nc.sync.dma_start(pyP[:], pc_d[:, :, 1].rearrange("g p -> p g"))
            nc.sync.dma_start(oxP[:], po_d[:, :, 0].rearrange("g p -> p g"))
            nc.sync.dma_start(oyP[:], po_d[:, :, 1].rearrange("g p -> p g"))
            nc.sync.dma_start(kxP[:], gk_d[:, :, 0].rearrange("g p -> p g"))
            nc.sync.dma_start(kyP[:], gk_d[:, :, 1].rearrange("g p -> p g"))
            nc.sync.dma_start(mkP[:], mk_d[:, :].rearrange("g p -> p g"))

            # gathered-values accumulation area: [sAx sAy sDx sDy][sr0 sr1 sr2 cnt1][spx spy sox soy][cnt2 ...]
            ex1a = keep.tile([128, G, 4], FP, tag="ex1a")
            ex1b = keep.tile([128, G, 4], FP, tag="ex1b")
            ex2a = keep.tile([128, G, 4], FP, tag="ex2a")
            ex2b = keep.tile([128, G, 1], FP, tag="ex2b")

            # ---------------- main loop ----------------
            for b in range(NB):
                g0 = b * BG
                gps = ps_mm.tile([128, BG, 128], FP, tag="gps")
                ops = ps_mm.tile([128, BG, 128], FP, tag="ops")
                d2ps = ps_mm.tile([128, BG, 128], FP, tag="d2ps")
                erep = ps_mm.tile([128, BG, 128], FP, tag="erep")
                nc.tensor.matmul(
                    erep[:].rearrange("q b p -> q (b p)"),
                    ones1[:],
                    erowR[:, g0:g0 + BG, :].rearrange("o b p -> o (b p)"),
                    start=True, stop=True,
                )
                for i in range(BG):
                    g = g0 + i
                    nc.tensor.matmul(gps[:, i, :], lhsT6[:, g, :], rhsG[:, g, :], start=True, stop=True)
                    nc.tensor.matmul(ops[:, i, :], lhsT6[:, g, :], rhsO[:, g, :], start=True, stop=True)
                    nc.tensor.matmul(d2ps[:, i, :], lhsT6[:, g, :], rhsK[:, g, :], start=True, stop=True)

                # -------- item1 chain --------
                iotaB = iotaF[:].rearrange("p (o q) -> p o q", o=1).broadcast_to([128, BG, 128])
                w = main.tile([128, BG, 128], FP, tag="w")
                nc.vector.tensor_scalar(w[:], gps[:], 0.5, 9.5, Alu.max, Alu.min)
                t = main.tile([128, BG, 128], FP, tag="t")
                nc.vector.tensor_scalar(t[:], w[:], CR, 8388608.0, Alu.add, Alu.subtract)
                h = main.tile([128, BG, 128], FP, tag="h")
                nc.vector.scalar_tensor_tensor(h[:], gps[:], -2.0, t[:], Alu.mult, Alu.add)
                q = main.tile([128, BG, 128], FP, tag="q")
                nc.vector.scalar_tensor_tensor(q[:], h[:], 1.0, t[:], Alu.add, Alu.mult)
                vE = main.tile([128, BG, 128], FP, tag="vE")
                nc.vector.tensor_tensor(vE[:], q[:], erep[:], Alu.mult)
                D2 = main.tile([128, BG, 128], FP, tag="D2")
                nc.vector.tensor_tensor(D2[:], vE[:], ops[:], Alu.add)
                dq = main.tile([128, BG, 128], FP, tag="dq")
                nc.vector.tensor_scalar(dq[:], D2[:], CQ, CQ, Alu.add, Alu.subtract)
                pkN = main.tile([128, BG, 128], FP, tag="pkN")
                nc.gpsimd.tensor_tensor(pkN[:], dq[:], iotaB, Alu.subtract)
                mx = main.tile([128, BG], FP, tag="mx")
                nc.vector.tensor_reduce(mx[:], pkN[:], Ax.X, Alu.max)
                oh = main.tile([128, BG, 128], FP, tag="oh")
                nc.vector.tensor_tensor(oh[:], pkN[:], mx[:].broadcast_to([128, BG, 128]), Alu.is_equal)

                # -------- item2 chain --------
                dq2 = main.tile([128, BG, 128], FP, tag="dq2")
                nc.vector.tensor_scalar(dq2[:], d2ps[:], CQ, CQ, Alu.add, Alu.subtract)
                pk2 = main.tile([128, BG, 128], FP, tag="pk2")
                nc.gpsimd.tensor_tensor(pk2[:], dq2[:], iotaB, Alu.subtract)
                mx2 = main.tile([128, BG], FP, tag="mx2")
                nc.vector.tensor_reduce(mx2[:], pk2[:], Ax.X, Alu.max)
                oh2 = main.tile([128, BG, 128], FP, tag="oh2")
                nc.vector.tensor_tensor(oh2[:], pk2[:], mx2[:].broadcast_to([128, BG, 128]), Alu.is_equal)

                # -------- transpose one-hots, gather via matmul --------
                ohT_ps = ps_one.tile([128, BG, 128], FP, tag="ohT_ps")
                for i in range(BG):
                    nc.tensor.transpose(ohT_ps[:, i, :], oh[:, i, :], ident[:])
                ohT = main.tile([128, BG, 128], FP, tag="ohT")
                nc.vector.tensor_copy(ohT[:], ohT_ps[:])
                oh2T_ps = ps_one.tile([128, BG, 128], FP, tag="ohT_ps")
                for i in range(BG):
                    nc.tensor.transpose(oh2T_ps[:, i, :], oh2[:, i, :], ident[:])
                oh2T = main.tile([128, BG, 128], FP, tag="oh2T")
                nc.vector.tensor_copy(oh2T[:], oh2T_ps[:])

                exPS = ps_one.tile([128, BG, 16], FP, tag="exPS")
                for i in range(BG):
                    g = g0 + i
                    nc.tensor.matmul(exPS[:, i, 0:4], ohT[:, i, :], T1[:, g, :], start=True, stop=True)
                    nc.tensor.matmul(exPS[:, i, 4:8], ohT[:, i, :], T2[:, g, :], start=True, stop=True)
                    nc.tensor.matmul(exPS[:, i, 8:12], oh2T[:, i, :], T3[:, g, :], start=True, stop=True)
                    nc.tensor.matmul(exPS[:, i, 12:13], oh2T[:, i, :], onesc[:], start=True, stop=True)
                nc.vector.tensor_copy(ex1a[:, g0:g0 + BG, :], exPS[:, :, 0:4])
                nc.vector.tensor_copy(ex1b[:, g0:g0 + BG, :], exPS[:, :, 4:8])
                nc.vector.tensor_copy(ex2a[:, g0:g0 + BG, :], exPS[:, :, 8:12])
                nc.vector.tensor_copy(ex2b[:, g0:g0 + BG, :], exPS[:, :, 12:13])

            # ---------------- tail (128 x G ops) ----------------
            tl = prep  # reuse pool
            def TT(opname, a, bb, op):
                r = tl.tile([128, G], FP, tag=opname)
                nc.vector.tensor_tensor(r[:], a, bb, op)
                return r

            ic1 = tl.tile([128, G], FP, tag="ic1")
            nc.vector.reciprocal(ic1[:], ex1b[:, :, 3])
            nAx = TT("nAx", ex1a[:, :, 0], ic1[:], Alu.mult)
            nAy = TT("nAy", ex1a[:, :, 1], ic1[:], Alu.mult)
            nDx = TT("nDx", ex1a[:, :, 2], ic1[:], Alu.mult)
            nDy = TT("nDy", ex1a[:, :, 3], ic1[:], Alu.mult)
            nr0 = TT("nr0", ex1b[:, :, 0], ic1[:], Alu.mult)
            nr1 = TT("nr1", ex1b[:, :, 1], ic1[:], Alu.mult)
            nr2 = TT("nr2", ex1b[:, :, 2], ic1[:], Alu.mult)
            u1 = TT("u1", pxP[:], nr0[:], Alu.mult)
            u2 = TT("u2", pyP[:], nr1[:], Alu.mult)
            gst = TT("gst", u1[:], u2[:], Alu.add)
            gst = TT("gst2", gst[:], nr2[:], Alu.add)
            wst = tl.tile([128, G], FP, tag="wst")
            nc.vector.tensor_scalar(wst[:], gst[:], 0.5, 9.5, Alu.max, Alu.min)
            tst = tl.tile([128, G], FP, tag="tst")
            nc.vector.tensor_scalar(tst[:], wst[:], CR, 8388608.0, Alu.add, Alu.subtract)
            sst = tl.tile([128, G], FP, tag="sst")
            nc.vector.tensor_scalar(sst[:], tst[:], 0.1, None, Alu.mult)
            tgx = tl.tile([128, G], FP, tag="tgx")
            nc.vector.scalar_tensor_tensor(tgx[:], sst[:], 1.0, nDx[:], Alu.mult, Alu.mult)
            tgx = TT("tgx2", tgx[:], nAx[:], Alu.add)
            tgy = tl.tile([128, G], FP, tag="tgy")
            nc.vector.scalar_tensor_tensor(tgy[:], sst[:], 1.0, nDy[:], Alu.mult, Alu.mult)
            tgy = TT("tgy2", tgy[:], nAy[:], Alu.add)

            def smooth_l1_sum(pred_x, pred_y, tx, ty, px_, py_, name):
                # sum over coords of smooth_l1(pred, (t - p)/4); returns (128, G)
                acc = None
                for ci, (pr, tt_, pp) in enumerate(((pred_x, tx, px_), (pred_y, ty, py_))):
                    sfx = name + str(ci)
                    dfe = tl.tile([128, G], FP, tag=sfx + "dfe")
                    # off = (t - p) * 0.25 ; d = pr - off
                    nc.vector.scalar_tensor_tensor(dfe[:], pp, -1.0, tt_, Alu.mult, Alu.add)
                    nc.vector.tensor_scalar(dfe[:], dfe[:], -0.25, None, Alu.mult)
                    nc.vector.tensor_tensor(dfe[:], dfe[:], pr, Alu.add)
                    ad = tl.tile([128, G], FP, tag=sfx + "ad")
                    nc.scalar.activation(ad[:], dfe[:], mybir.ActivationFunctionType.Abs)
                    m = tl.tile([128, G], FP, tag=sfx + "m")
                    nc.vector.tensor_scalar(m[:], ad[:], BETA, None, Alu.min)
                    uu = tl.tile([128, G], FP, tag=sfx + "u")
                    nc.vector.scalar_tensor_tensor(uu[:], m[:], -0.5, ad[:], Alu.mult, Alu.add)
                    sl = tl.tile([128, G], FP, tag=sfx + "sl")
                    nc.vector.scalar_tensor_tensor(sl[:], m[:], 4.0, uu[:], Alu.mult, Alu.mult)
                    if acc is None:
                        acc = sl
                    else:
                        acc = TT(name + "acc", acc[:], sl[:], Alu.add)
                return acc

            s1 = smooth_l1_sum(oxP[:], oyP[:], tgx[:], tgy[:], pxP[:], pyP[:], "i1")

            ic2 = tl.tile([128, G], FP, tag="ic2")
            nc.vector.reciprocal(ic2[:], ex2b[:, :, 0])
            npx = TT("npx", ex2a[:, :, 0], ic2[:], Alu.mult)
            npy = TT("npy", ex2a[:, :, 1], ic2[:], Alu.mult)
            nox = TT("nox", ex2a[:, :, 2], ic2[:], Alu.mult)
            noy = TT("noy", ex2a[:, :, 3], ic2[:], Alu.mult)
            s2 = smooth_l1_sum(nox[:], noy[:], kxP[:], kyP[:], npx[:], npy[:], "i2")
            s2 = TT("s2m", s2[:], mkP[:], Alu.mult)

            # ---------------- reduce to 2 scalars ----------------
            s1r = tl.tile([128, 1], FP, tag="s1r")
            nc.vector.tensor_reduce(s1r[:], s1[:], Ax.X, Alu.add)
            s2r = tl.tile([128, 1], FP, tag="s2r")
            nc.vector.tensor_reduce(s2r[:], s2[:], Ax.X, Alu.add)
            sboth = tl.tile([128, 2], FP, tag="sboth")
            nc.vector.tensor_copy(sboth[:, 0:1], s1r[:])
            nc.vector.tensor_copy(sboth[:, 1:2], s2r[:])
            sc_ps = ps_one.tile([2, 1], FP, tag="exPS")
            nc.tensor.matmul(sc_ps[:], sboth[:], onesc[:], start=True, stop=True)
            outsb = tl.tile([2, 1], FP, tag="outsb")
            nc.vector.tensor_copy(outsb[:], sc_ps[:])
            nc.sync.dma_start(out_d[:].rearrange("(a b) -> a b", b=1), outsb[:])

    return nc


_CACHE = {}


def _get_program():
    if "nc" not in _CACHE:
        nc = bacc.Bacc("TRN2", target_bir_lowering=False, num_devices=N_CORES)
        pc_d = nc.declare_dram_parameter("pc", [G, P, 2], dt.float32, isOutput=False)
        po_d = nc.declare_dram_parameter("po", [G, P, 2], dt.float32, isOutput=False)
        gc_d = nc.declare_dram_parameter("gc", [G, P, 2], dt.float32, isOutput=False)
        gk_d = nc.declare_dram_parameter("gk", [G, P, 2], dt.float32, isOutput=False)
        mk_d = nc.declare_dram_parameter("mk", [G, P], dt.float32, isOutput=False)
        out_d = nc.declare_dram_parameter("out", [2], dt.float32, isOutput=True)
        _build(nc, pc_d[:], po_d[:], gc_d[:], gk_d[:], mk_d[:], out_d[:])
        nc.compile()
        _CACHE["nc"] = nc
    return _CACHE["nc"]


def _in_maps(inputs):
    pc = np.ascontiguousarray(inputs["pred_contours"], dtype=np.float32)
    po = np.ascontiguousarray(inputs["pred_offsets"], dtype=np.float32)
    gc = np.ascontiguousarray(inputs["gt_contours"], dtype=np.float32)
    gk = np.ascontiguousarray(inputs["gt_key_points"], dtype=np.float32)
    mk = np.ascontiguousarray(inputs["gt_key_points_mask"]).astype(np.float32)
    maps = []
    for c in range(N_CORES):
        s = slice(c * G, (c + 1) * G)
        maps.append({
            "pc": pc[s], "po": po[s], "gc": gc[s], "gk": gk[s], "mk": mk[s],
        })
    return maps


def kernel(pred_contours, pred_offsets, gt_contours, gt_key_points, gt_key_points_mask,
           _results_hook=None):
    inputs = {
        "pred_contours": pred_contours,
        "pred_offsets": pred_offsets,
        "gt_contours": gt_contours,
        "gt_key_points": gt_key_points,
        "gt_key_points_mask": gt_key_points_mask,
    }
    nc = _get_program()
    res = run_bass_kernel_spmd(nc, _in_maps(inputs), list(range(N_CORES)))
    if _results_hook is not None:
        _results_hook(res)
    s1 = f32(0.0)
    s2 = f32(0.0)
    for r in res.results:
        s1 = f32(s1 + f32(r["out"][0]))
        s2 = f32(s2 + f32(r["out"][1]))
    cnt1 = f32(N * P * 2)
    cnt2 = f32(max(float(np.sum(gt_key_points_mask != 0)) * 2.0, 1.0))
    loss = f32(f32(s1 / cnt1) * f32(0.5) + f32(s2 / cnt2) * f32(0.5))
    return np.asarray(loss, dtype=np.float32)



# revision 12
# speedup vs baseline: 2.1924x; 2.1924x over previous
"""Trainium2 Bass kernel for nn_DMLoss (contour matching loss), 8-core data parallel.

v2: bf16 split-precision matmuls + descriptor-light DMA.

Algorithm (per instance, P=128 points, TIME=10):
  item1: for each pred point, nearest of 1280 interpolated gt points.
    Segment n spans A_n = gt[n-1] .. gt[n]; candidates are A_n + (t/10)*D_n.
    g = 10*u + 0.5 with u = <p - A, D>/|D|^2; best discrete t = round(clamp(g,.5,9.5)-.5).
    dist^2 = |p-A|^2 + (e/100)*t*(t-2g+1).  We evaluate, per (pred m, seg n),
    D2 = -SC*(dist^2 - |p|^2) via TensorE (the |p|^2 row-constant cancels in the
    argmin) and pack quantized distance + segment index into one float; a free-dim
    reduce-max gives argmin.  The one-hot (exact, unique) gathers segment data via
    a transposed bf16 matmul; the target is recomputed exactly in the tail.
  item2: same machinery without interpolation (nearest pred point per gt key point).

All heavy matmuls run in bf16 at 1 cycle/row; full fp32-level precision is
recovered by splitting each factor x into bf16 hi/lo (x ~ hi + lo, rel err 2^-17)
and pairing hi*hi + lo*hi + hi*lo as separate contraction rows.

Output per core: [sum_loss1, sum_loss2]; host divides by counts and combines.
"""
import sys

for _p in ("/opt/trn_rl_repo",):
    if _p not in sys.path:
        sys.path.insert(0, _p)

import numpy as np

import concourse.bass as bass
import concourse.tile as tile
from concourse import bacc, mybir
from concourse.bass_utils import run_bass_kernel_spmd

dt = mybir.dt
Alu = mybir.AluOpType
Ax = mybir.AxisListType
Act = mybir.ActivationFunctionType
f32 = np.float32

N_CORES = 8
N, P = 256, 128
G = N // N_CORES          # instances per core = 32
BG = 4                    # instances per chain batch
NB = G // BG              # 8 blocks
SC = 131072.0             # distance scale (quantum = 128/SC = 2^-10)
SHIFT = 48.0
BETA = 0.25               # smooth-l1 beta = 1/STRIDE
CQ = float(2 ** 30 + 2 ** 25)       # 1107296256
C1 = CQ - SC * SHIFT                # 1101004800 (exactly representable)
M23 = 8388608.0

# SPL slab indices (each slab is a (32, 128) bf16 plane).
# lhsT_pred rows: [px_hi, py_hi, px_lo, py_lo, px_hi, py_hi, 1, 1]
P0 = 0          # px_hi py_hi px_lo py_lo px_hi2 py_hi2 ones ones   (8)
K0 = 8          # kx_hi ky_hi kx_lo ky_lo kx_hi2 ky_hi2 ones ones  (8)
G0 = 16         # r0_hi r1_hi r0_hi2 r1_hi2 r0_lo r1_lo r2_hi r2_lo (8)
O0 = 24         # o0_hi o1_hi o0_hi2 o1_hi2 o0_lo o1_lo o2_hi o2_lo (8)
Q0 = 32         # q0_hi q1_hi q0_hi2 q1_hi2 q0_lo q1_lo q2_hi q2_lo (8)
E0 = 40         # zeros x6, er_hi, er_lo                             (8)
T0 = 48         # ax_hi ay_hi dx_hi dy_hi ox_hi oy_hi                (6)
NSLAB = 54


def _build(nc, pc_d, po_d, gc_d, gk_d, mk_d, out_d):
    FP = dt.float32
    BF = dt.bfloat16

    with tile.TileContext(nc) as tc:
        with (
            tc.tile_pool(name="const", bufs=1) as cpool,
            tc.tile_pool(name="prep", bufs=1) as prep,
            tc.tile_pool(name="oper", bufs=1) as oper,
            tc.tile_pool(name="main", bufs=2) as main,
            tc.tile_pool(name="keep", bufs=1) as keep,
        ):
            # ---------------- constants ----------------
            iota_i = cpool.tile([128, 128], dt.int32)
            nc.gpsimd.iota(iota_i[:], pattern=[[1, 128]], channel_multiplier=0)
            iotaF = cpool.tile([128, 128], FP, tag="iotaF")
            nc.vector.tensor_copy(iotaF[:], iota_i[:])
            iotaC_i = cpool.tile([128, 1], dt.int32)
            nc.gpsimd.iota(iotaC_i[:], pattern=[[0, 1]], channel_multiplier=1)
            iotaC = cpool.tile([128, 1], FP, tag="iotaC")
            nc.vector.tensor_copy(iotaC[:], iotaC_i[:])
            identB = cpool.tile([128, 128], BF, tag="identB")
            nc.vector.tensor_scalar(identB[:], iotaF[:], iotaC[:], None, Alu.is_equal)
            ident32B = cpool.tile([32, 32], BF, tag="ident32B")
            nc.vector.tensor_scalar(
                ident32B[:], iotaF[0:32, 0:32], iotaC[0:32], None, Alu.is_equal
            )
            ident32F = cpool.tile([32, 32], FP, tag="ident32F")
            nc.vector.tensor_scalar(
                ident32F[:], iotaF[0:32, 0:32], iotaC[0:32], None, Alu.is_equal
            )
            onesc = cpool.tile([128, 1], FP, tag="onesc")
            nc.gpsimd.memset(onesc[:], 1.0)
            bneg05 = cpool.tile([128, 1], FP, tag="bneg05")
            nc.gpsimd.memset(bneg05[:], -0.5)

            # ---------------- contiguous input loads ----------------
            pc_i = prep.tile([32, 128, 2], FP, tag="pc_i")
            po_i = prep.tile([32, 128, 2], FP, tag="po_i")
            gc_i = prep.tile([32, 128, 2], FP, tag="gc_i")
            gk_i = prep.tile([32, 128, 2], FP, tag="gk_i")
            mk_i = prep.tile([32, 128], FP, tag="mk_i")
            a_i = prep.tile([32, 128, 2], FP, tag="a_i")
            nc.sync.dma_start(pc_i[:], pc_d[:, :, :])
            nc.sync.dma_start(po_i[:], po_d[:, :, :])
            nc.sync.dma_start(gc_i[:], gc_d[:, :, :])
            nc.sync.dma_start(gk_i[:], gk_d[:, :, :])
            nc.sync.dma_start(mk_i[:], mk_d[:, :])
            # roll-by-one load: a[p] = gc[p-1 mod 128]
            nc.sync.dma_start(a_i[:, 1:128, :], gc_d[:, 0:127, :])
            nc.sync.dma_start(a_i[:, 0:1, :], gc_d[:, 127:128, :])

            # ---------------- segment geometry (g-major layout) ----------------
            d_i = prep.tile([32, 128, 2], FP, tag="d_i")
            nc.vector.tensor_tensor(d_i[:], gc_i[:], a_i[:], Alu.subtract)
            dsq = prep.tile([32, 128, 2], FP, tag="dsq")
            nc.gpsimd.tensor_tensor(dsq[:], d_i[:], d_i[:], Alu.mult)
            e = prep.tile([32, 128], FP, tag="e")
            nc.vector.tensor_tensor(e[:], dsq[:, :, 0], dsq[:, :, 1], Alu.add)
            einv = prep.tile([32, 128], FP, tag="einv")
            nc.vector.reciprocal(einv[:], e[:])
            t_ad = prep.tile([32, 128, 2], FP, tag="t_ad")
            nc.gpsimd.tensor_tensor(t_ad[:], a_i[:], d_i[:], Alu.mult)
            a2 = prep.tile([32, 128], FP, tag="a2")
            nc.vector.tensor_tensor(a2[:], t_ad[:, :, 0], t_ad[:, :, 1], Alu.add)
            asq = prep.tile([32, 128, 2], FP, tag="asq")
            nc.gpsimd.tensor_tensor(asq[:], a_i[:], a_i[:], Alu.mult)
            zA = prep.tile([32, 128], FP, tag="zA")
            nc.vector.tensor_tensor(zA[:], asq[:, :, 0], asq[:, :, 1], Alu.add)
            psq = prep.tile([32, 128, 2], FP, tag="psq")
            nc.gpsimd.tensor_tensor(psq[:], pc_i[:], pc_i[:], Alu.mult)
            zP = prep.tile([32, 128], FP, tag="zP")
            nc.vector.tensor_tensor(zP[:], psq[:, :, 0], psq[:, :, 1], Alu.add)

            einv_b = einv[:].rearrange("g q -> g q ()").broadcast_to([32, 128, 2])
            r_01 = prep.tile([32, 128, 2], FP, tag="r_01")
            nc.vector.scalar_tensor_tensor(r_01[:], d_i[:], 10.0, einv_b, Alu.mult, Alu.mult)
            r2 = prep.tile([32, 128], FP, tag="r2")
            nc.vector.scalar_tensor_tensor(r2[:], a2[:], -10.0, einv[:], Alu.mult, Alu.mult)
            nc.vector.tensor_scalar(r2[:], r2[:], 0.5, None, Alu.add)
            o_01 = prep.tile([32, 128, 2], FP, tag="o_01")
            nc.gpsimd.tensor_scalar(o_01[:], a_i[:], 2.0 * SC, None, Alu.mult)
            o2 = prep.tile([32, 128], FP, tag="o2")
            nc.gpsimd.tensor_scalar(o2[:], zA[:], -SC, None, Alu.mult)
            er = prep.tile([32, 128], FP, tag="er")
            nc.gpsimd.tensor_scalar(er[:], e[:], -SC / 100.0, None, Alu.mult)
            q_01 = prep.tile([32, 128, 2], FP, tag="q_01")
            nc.gpsimd.tensor_scalar(q_01[:], pc_i[:], 2.0 * SC, None, Alu.mult)
            q2 = prep.tile([32, 128], FP, tag="q2")
            nc.gpsimd.tensor_scalar(q2[:], zP[:], -SC, None, Alu.mult)

            # ---------------- bf16 hi/lo splits into SPL slabs ----------------
            SPL = prep.tile([32, NSLAB, 128], BF, tag="SPL")

            def pair_view(s):
                # (32, 128, 2) view over slabs s, s+1 (c maps to slab index)
                return SPL[:, s:s + 2, :].rearrange("g s q -> g q s")

            tmpP = prep.tile([32, 128, 2], FP, tag="tmpP")

            def split_pair(src, s_hi, s_lo, eng_hi, eng_up, eng_lo):
                # src (32, 128, 2) f32 -> hi slabs (s_hi, s_hi+1), lo slabs
                eng_hi.tensor_copy(pair_view(s_hi), src)
                if s_lo is not None:
                    eng_up.tensor_copy(tmpP[:], pair_view(s_hi))
                    eng_lo.tensor_tensor(pair_view(s_lo), src, tmpP[:], Alu.subtract)

            def split_one(src, s_hi, s_lo, eng_hi, eng_up, eng_lo):
                eng_hi.tensor_copy(SPL[:, s_hi, :], src)
                if s_lo is not None:
                    eng_up.tensor_copy(tmpP[:, :, 0], SPL[:, s_hi, :])
                    eng_lo.tensor_tensor(SPL[:, s_lo, :], src, tmpP[:, :, 0], Alu.subtract)

            V, Gp = nc.vector, nc.gpsimd
            split_pair(pc_i[:], P0 + 0, P0 + 2, V, Gp, V)
            split_pair(gk_i[:], K0 + 0, K0 + 2, V, Gp, V)
            split_pair(r_01[:], G0 + 0, G0 + 4, V, Gp, V)
            split_pair(o_01[:], O0 + 0, O0 + 4, V, Gp, V)
            split_pair(q_01[:], Q0 + 0, Q0 + 4, V, Gp, V)
            split_one(r2[:], G0 + 6, G0 + 7, V, Gp, V)
            split_one(o2[:], O0 + 6, O0 + 7, V, Gp, V)
            split_one(q2[:], Q0 + 6, Q0 + 7, V, Gp, V)
            split_one(er[:], E0 + 6, E0 + 7, V, Gp, V)
            # hi-only table slabs
            nc.gpsimd.tensor_copy(pair_view(T0 + 0), a_i[:])
            nc.gpsimd.tensor_copy(pair_view(T0 + 2), d_i[:])
            nc.gpsimd.tensor_copy(pair_view(T0 + 4), po_i[:])
            # duplicated hi rows
            nc.vector.tensor_copy(SPL[:, P0 + 4:P0 + 6, :], SPL[:, P0 + 0:P0 + 2, :])
            nc.vector.tensor_copy(SPL[:, K0 + 4:K0 + 6, :], SPL[:, K0 + 0:K0 + 2, :])
            nc.vector.tensor_copy(SPL[:, G0 + 2:G0 + 4, :], SPL[:, G0 + 0:G0 + 2, :])
            nc.vector.tensor_copy(SPL[:, O0 + 2:O0 + 4, :], SPL[:, O0 + 0:O0 + 2, :])
            nc.vector.tensor_copy(SPL[:, Q0 + 2:Q0 + 4, :], SPL[:, Q0 + 0:Q0 + 2, :])
            # ones / zeros
            nc.gpsimd.memset(SPL[:, P0 + 6:P0 + 8, :], 1.0)
            nc.gpsimd.memset(SPL[:, K0 + 6:K0 + 8, :], 1.0)
            nc.gpsimd.memset(SPL[:, E0 + 0:E0 + 6, :], 0.0)

            # ---------------- DRAM bounce: slabs -> matmul operand tiles ----------------
            slab_d = nc.dram_tensor("slabs", [32, NSLAB, 128], BF)
            nc.sync.dma_start(slab_d[:, :, :], SPL[:])

            lhsP = oper.tile([8, 32, 128], BF, tag="lhsP")
            lhsK = oper.tile([8, 32, 128], BF, tag="lhsK")
            rhsG = oper.tile([8, 32, 128], BF, tag="rhsG")
            rhsO = oper.tile([8, 32, 128], BF, tag="rhsO")
            rhsQ = oper.tile([8, 32, 128], BF, tag="rhsQ")
            rhsE = oper.tile([8, 32, 128], BF, tag="rhsE")
            for dst, s in ((lhsP, P0), (lhsK, K0), (rhsG, G0), (rhsO, O0),
                           (rhsQ, Q0), (rhsE, E0)):
                # dst[t, g, p] = slab_d[g, s+t, p]
                nc.sync.dma_start(
                    dst[:], slab_d[:, s:s + 8, :].rearrange("g t p -> t g p")
                )

            # ---------------- gather tables via TensorE transposes ----------------
            # T12[n, g, j]: [Ax, Ay, Dx, Dy, r0h, r0l, r1h, r1l, r2h, r2l]
            T12 = keep.tile([128, G, 10], BF, tag="T12")
            T3c = keep.tile([128, G, 4], BF, tag="T3c")
            t12_src = [T0 + 0, T0 + 1, T0 + 2, T0 + 3, G0 + 0, G0 + 4,
                       G0 + 1, G0 + 5, G0 + 6, G0 + 7]
            t3_src = [P0 + 0, P0 + 1, T0 + 4, T0 + 5]
            pxP = keep.tile([128, G], FP, tag="pxP")
            pyP = keep.tile([128, G], FP, tag="pyP")
            oxP = keep.tile([128, G], FP, tag="oxP")
            oyP = keep.tile([128, G], FP, tag="oyP")
            kxP = keep.tile([128, G], FP, tag="kxP")
            kyP = keep.tile([128, G], FP, tag="kyP")
            mkP = keep.tile([128, G], FP, tag="mkP")
            with tc.tile_pool(name="ps_prep", bufs=3, space="PSUM") as ps_prep:
                for j, s in enumerate(t12_src):
                    tps = ps_prep.tile([128, 32], BF, tag="tpsB")
                    nc.tensor.transpose(tps[:], SPL[:, s, :], ident32B[:])
                    nc.scalar.activation(T12[:, :, j], tps[:], Act.Copy)
                for j, s in enumerate(t3_src):
                    tps = ps_prep.tile([128, 32], BF, tag="tpsB")
                    nc.tensor.transpose(tps[:], SPL[:, s, :], ident32B[:])
                    nc.scalar.activation(T3c[:, :, j], tps[:], Act.Copy)

                # ---------------- f32 transposes for the tail ----------------
                for dst, src in ((pxP, pc_i[:, :, 0]), (pyP, pc_i[:, :, 1]),
                                 (oxP, po_i[:, :, 0]), (oyP, po_i[:, :, 1]),
                                 (kxP, gk_i[:, :, 0]), (kyP, gk_i[:, :, 1]),
                                 (mkP, mk_i[:])):
                    fps = ps_prep.tile([128, 32], FP, tag="tpsF")
                    nc.tensor.transpose(fps[:], src, ident32F[:])
                    nc.scalar.activation(dst[:], fps[:], Act.Copy)

            # gathered values: [0:10]=T12 slots, [10:14]=T3c slots
            ex = keep.tile([128, G, 14], FP, tag="ex")

            ps_grid_cm = tc.tile_pool(name="ps_grid", bufs=1, space="PSUM")
            ps_oh_cm = tc.tile_pool(name="ps_oh", bufs=1, space="PSUM")
            ps_ex_cm = tc.tile_pool(name="ps_ex", bufs=1, space="PSUM")
            ps_grid = ps_grid_cm.__enter__()
            ps_oh = ps_oh_cm.__enter__()
            ps_ex = ps_ex_cm.__enter__()

            iotaB = iotaF[:].rearrange("p (o q) -> p o q", o=1).broadcast_to([128, BG, 128])

            # ---------------- main loop ----------------
            for b in range(NB):
                g0 = b * BG
                gps = ps_grid.tile([128, BG, 128], FP, tag="gps")
                ops = ps_grid.tile([128, BG, 128], FP, tag="ops")
                erep = ps_grid.tile([128, BG, 128], FP, tag="erep")
                d2ps = ps_grid.tile([128, BG, 128], FP, tag="d2ps")
                for i in range(BG):
                    g = g0 + i
                    nc.tensor.matmul(gps[:, i, :], lhsP[:, g, :], rhsG[:, g, :], start=True, stop=True)
                    nc.tensor.matmul(ops[:, i, :], lhsP[:, g, :], rhsO[:, g, :], start=True, stop=True)
                    nc.tensor.matmul(erep[:, i, :], lhsP[:, g, :], rhsE[:, g, :], start=True, stop=True)
                    nc.tensor.matmul(d2ps[:, i, :], lhsK[:, g, :], rhsQ[:, g, :], start=True, stop=True)

                # -------- item1 chain --------
                # t = min(round(relu(gps - 0.5)), 9)  [== round(clamp(gps,.5,9.5)-.5)]
                s1t = main.tile([128, BG, 128], FP, tag="s1t")
                nc.scalar.activation(s1t[:], gps[:], Act.Relu, bias=bneg05[:], scale=1.0)
                s2t = main.tile([128, BG, 128], FP, tag="s2t")
                nc.scalar.activation(s2t[:], s1t[:], Act.Copy, bias=M23, scale=1.0)
                s3t = main.tile([128, BG, 128], FP, tag="s3t")
                nc.scalar.activation(s3t[:], s2t[:], Act.Copy, bias=-M23, scale=1.0)
                t = main.tile([128, BG, 128], FP, tag="t")
                nc.gpsimd.tensor_scalar(t[:], s3t[:], 9.0, None, Alu.min)
                h = main.tile([128, BG, 128], FP, tag="h")
                nc.vector.scalar_tensor_tensor(h[:], gps[:], -2.0, t[:], Alu.mult, Alu.add)
                q = main.tile([128, BG, 128], FP, tag="q")
                nc.vector.scalar_tensor_tensor(q[:], h[:], 1.0, t[:], Alu.add, Alu.mult)
                vE = main.tile([128, BG, 128], FP, tag="vE")
                nc.vector.tensor_tensor(vE[:], q[:], erep[:], Alu.mult)
                # dqA = (vE + C1) + ops; both adds round on the 128-grid near 2^30,
                # keeping dq an exact multiple of 128 (extra quantization <= 1 ulp)
                dqA = main.tile([128, BG, 128], FP, tag="dqA")
                nc.vector.scalar_tensor_tensor(dqA[:], vE[:], C1, ops[:], Alu.add, Alu.add)
                dq = main.tile([128, BG, 128], FP, tag="dq")
                nc.gpsimd.tensor_scalar(dq[:], dqA[:], CQ, None, Alu.subtract)
                pkN = main.tile([128, BG, 128], FP, tag="pkN")
                nc.gpsimd.tensor_tensor(pkN[:], dq[:], iotaB, Alu.subtract)
                mx = main.tile([128, BG], FP, tag="mx")
                nc.vector.tensor_reduce(mx[:], pkN[:], Ax.X, Alu.max)
                oh = main.tile([128, BG, 128], BF, tag="oh")
                nc.vector.tensor_tensor(oh[:], pkN[:], mx[:].broadcast_to([128, BG, 128]), Alu.is_equal)

                # -------- item2 chain --------
                dq2 = main.tile([128, BG, 128], FP, tag="dq2")
                nc.vector.tensor_scalar(dq2[:], d2ps[:], C1, CQ, Alu.add, Alu.subtract)
                pk2 = main.tile([128, BG, 128], FP, tag="pk2")
                nc.gpsimd.tensor_tensor(pk2[:], dq2[:], iotaB, Alu.subtract)
                mx2 = main.tile([128, BG], FP, tag="mx2")
                nc.vector.tensor_reduce(mx2[:], pk2[:], Ax.X, Alu.max)
                oh2 = main.tile([128, BG, 128], BF, tag="oh2")
                nc.vector.tensor_tensor(oh2[:], pk2[:], mx2[:].broadcast_to([128, BG, 128]), Alu.is_equal)

                # -------- transpose one-hots (bf16), gather via matmul --------
                ohT_ps = ps_oh.tile([128, BG, 128], BF, tag="ohT_ps")
                oh2T_ps = ps_oh.tile([128, BG, 128], BF, tag="oh2T_ps")
                for i in range(BG):
                    nc.tensor.transpose(ohT_ps[:, i, :], oh[:, i, :], identB[:])
                    nc.tensor.transpose(oh2T_ps[:, i, :], oh2[:, i, :], identB[:])
                ohT = main.tile([128, BG, 128], BF, tag="ohT")
                nc.scalar.activation(ohT[:], ohT_ps[:], Act.Copy)
                oh2T = main.tile([128, BG, 128], BF, tag="oh2T")
                nc.scalar.activation(oh2T[:], oh2T_ps[:], Act.Copy)

                exPS = ps_ex.tile([128, BG, 14], FP, tag="exPS")
                for i in range(BG):
                    g = g0 + i
                    nc.tensor.matmul(exPS[:, i, 0:10], ohT[:, i, :], T12[:, g, :], start=True, stop=True)
                    nc.tensor.matmul(exPS[:, i, 10:14], oh2T[:, i, :], T3c[:, g, :], start=True, stop=True)
                nc.scalar.activation(ex[:, g0:g0 + BG, :], exPS[:], Act.Copy)

            # ---------------- tail (128 x G f32 ops) ----------------
            tl = prep

            def TT(name, a, bb, op, eng=nc.vector):
                r = tl.tile([128, G], FP, tag=name)
                eng.tensor_tensor(r[:], a, bb, op)
                return r

            r0 = TT("r0", ex[:, :, 4], ex[:, :, 5], Alu.add)
            r1 = TT("r1", ex[:, :, 6], ex[:, :, 7], Alu.add, nc.gpsimd)
            r2t = TT("r2t", ex[:, :, 8], ex[:, :, 9], Alu.add)
            v1 = TT("v1", pxP[:], r0[:], Alu.mult, nc.gpsimd)
            v2 = TT("v2", pyP[:], r1[:], Alu.mult)
            gst = TT("gst", v1[:], v2[:], Alu.add, nc.gpsimd)
            gst = TT("gst2", gst[:], r2t[:], Alu.add)
            # t* = min(round(relu(gst - .5)), 9)
            c1t = tl.tile([128, G], FP, tag="c1t")
            nc.scalar.activation(c1t[:], gst[:], Act.Relu, bias=bneg05[:], scale=1.0)
            c2t = tl.tile([128, G], FP, tag="c2t")
            nc.scalar.activation(c2t[:], c1t[:], Act.Copy, bias=M23, scale=1.0)
            c3t = tl.tile([128, G], FP, tag="c3t")
            nc.scalar.activation(c3t[:], c2t[:], Act.Copy, bias=-M23, scale=1.0)
            tst = tl.tile([128, G], FP, tag="tst")
            nc.vector.tensor_scalar(tst[:], c3t[:], 9.0, None, Alu.min)
            # target = A + 0.1 * t* * D
            m1 = TT("m1", tst[:], ex[:, :, 2], Alu.mult, nc.gpsimd)
            tgx = tl.tile([128, G], FP, tag="tgx")
            nc.vector.scalar_tensor_tensor(tgx[:], m1[:], 0.1, ex[:, :, 0], Alu.mult, Alu.add)
            m2 = TT("m2", tst[:], ex[:, :, 3], Alu.mult, nc.gpsimd)
            tgy = tl.tile([128, G], FP, tag="tgy")
            nc.vector.scalar_tensor_tensor(tgy[:], m2[:], 0.1, ex[:, :, 1], Alu.mult, Alu.add)

            def smooth_l1_sum(pred_x, pred_y, tx, ty, px_, py_, name):
                # sum over coords of smooth_l1(pred - (t - p)*0.25)
                acc = None
                for ci, (pr, tt_, pp) in enumerate(((pred_x, tx, px_), (pred_y, ty, py_))):
                    sfx = name + str(ci)
                    e1 = TT(sfx + "e1", tt_, pp, Alu.subtract, nc.gpsimd)
                    dfe = tl.tile([128, G], FP, tag=sfx + "dfe")
                    nc.vector.scalar_tensor_tensor(dfe[:], e1[:], -0.25, pr, Alu.mult, Alu.add)
                    ad = tl.tile([128, G], FP, tag=sfx + "ad")
                    nc.scalar.activation(ad[:], dfe[:], Act.Abs)
                    m = tl.tile([128, G], FP, tag=sfx + "m")
                    nc.vector.tensor_scalar(m[:], ad[:], BETA, None, Alu.min)
                    uu = tl.tile([128, G], FP, tag=sfx + "u")
                    nc.vector.scalar_tensor_tensor(uu[:], m[:], -0.5, ad[:], Alu.mult, Alu.add)
                    sl = tl.tile([128, G], FP, tag=sfx + "sl")
                    nc.vector.scalar_tensor_tensor(sl[:], m[:], 4.0, uu[:], Alu.mult, Alu.mult)
                    if acc is None:
                        acc = sl
                    else:
                        acc = TT(name + "acc", acc[:], sl[:], Alu.add)
                return acc

            s1 = smooth_l1_sum(oxP[:], oyP[:], tgx[:], tgy[:], pxP[:], pyP[:], "i1")

            s2 = smooth_l1_sum(ex[:, :, 12], ex[:, :, 13], kxP[:], kyP[:],
                               ex[:, :, 10], ex[:, :, 11], "i2")
            s2 = TT("s2m", s2[:], mkP[:], Alu.mult)

            # ---------------- reduce to 2 scalars ----------------
            s1r = tl.tile([128, 1], FP, tag="s1r")
            nc.vector.tensor_reduce(s1r[:], s1[:], Ax.X, Alu.add)
            s2r = tl.tile([128, 1], FP, tag="s2r")
            nc.vector.tensor_reduce(s2r[:], s2[:], Ax.X, Alu.add)
            sboth = tl.tile([128, 2], FP, tag="sboth")
            nc.vector.tensor_copy(sboth[:, 0:1], s1r[:])
            nc.vector.tensor_copy(sboth[:, 1:2], s2r[:])
            sc_ps = ps_oh.tile([2, 1], FP, tag="sc_ps")
            nc.tensor.matmul(sc_ps[:], sboth[:], onesc[:], start=True, stop=True)
            outsb = tl.tile([2, 1], FP, tag="outsb")
            nc.vector.tensor_copy(outsb[:], sc_ps[:])
            nc.sync.dma_start(out_d[:].rearrange("(a b) -> a b", b=1), outsb[:])
            ps_ex_cm.__exit__(None, None, None)
            ps_oh_cm.__exit__(None, None, None)
            ps_grid_cm.__exit__(None, None, None)

    return nc


_CACHE = {}


def _get_program():
    if "nc" not in _CACHE:
        nc = bacc.Bacc("TRN2", target_bir_lowering=False, num_devices=N_CORES)
        pc_d = nc.declare_dram_parameter("pc", [G, P, 2], dt.float32, isOutput=False)
        po_d = nc.declare_dram_parameter("po", [G, P, 2], dt.float32, isOutput=False)
        gc_d = nc.declare_dram_parameter("gc", [G, P, 2], dt.float32, isOutput=False)
        gk_d = nc.declare_dram_parameter("gk", [G, P, 2], dt.float32, isOutput=False)
        mk_d = nc.declare_dram_parameter("mk", [G, P], dt.float32, isOutput=False)
        out_d = nc.declare_dram_parameter("out", [2], dt.float32, isOutput=True)
        _build(nc, pc_d[:], po_d[:], gc_d[:], gk_d[:], mk_d[:], out_d[:])
        nc.compile()
        _CACHE["nc"] = nc
    return _CACHE["nc"]


def _in_maps(inputs):
    pc = np.ascontiguousarray(inputs["pred_contours"], dtype=np.float32)
    po = np.ascontiguousarray(inputs["pred_offsets"], dtype=np.float32)
    gc = np.ascontiguousarray(inputs["gt_contours"], dtype=np.float32)
    gk = np.ascontiguousarray(inputs["gt_key_points"], dtype=np.float32)
    mk = np.ascontiguousarray(inputs["gt_key_points_mask"]).astype(np.float32)
    maps = []
    for c in range(N_CORES):
        s = slice(c * G, (c + 1) * G)
        maps.append({
            "pc": pc[s], "po": po[s], "gc": gc[s], "gk": gk[s], "mk": mk[s],
        })
    return maps


def kernel(pred_contours, pred_offsets, gt_contours, gt_key_points, gt_key_points_mask,
           _results_hook=None):
    inputs = {
        "pred_contours": pred_contours,
        "pred_offsets": pred_offsets,
        "gt_contours": gt_contours,
        "gt_key_points": gt_key_points,
        "gt_key_points_mask": gt_key_points_mask,
    }
    nc = _get_program()
    res = run_bass_kernel_spmd(nc, _in_maps(inputs), list(range(N_CORES)))
    if _results_hook is not None:
        _results_hook(res)
    s1 = f32(0.0)
    s2 = f32(0.0)
    for r in res.results:
        s1 = f32(s1 + f32(r["out"][0]))
        s2 = f32(s2 + f32(r["out"][1]))
    cnt1 = f32(N * P * 2)
    cnt2 = f32(max(float(np.sum(gt_key_points_mask != 0)) * 2.0, 1.0))
    loss = f32(f32(s1 / cnt1) * f32(0.5) + f32(s2 / cnt2) * f32(0.5))
    return np.asarray(loss, dtype=np.float32)


# revision 13
# speedup vs baseline: 3.6199x; 1.6511x over previous
"""Trainium2 Bass kernel for nn_DMLoss (contour matching loss), 8-core data parallel.

v3: block-diagonal bf16 split-precision matmuls, XBAR DMA-transposed one-hots,
engine-balanced elementwise chain.

Per instance (P=128 points, TIME=10):
  item1: nearest of 1280 interpolated gt points per pred point.  Segment n spans
    A_n = gt[n-1]..gt[n]; g' = 10*u - 0.5 with u = <p-A,D>/|D|^2; best discrete
    t = clamp(round(g'), 0, 9); dist^2 = |p-A|^2 + (e/100)*t*(t-2g').
    D2 = -SC*(dist^2 - |p|^2) is evaluated per (pred m, seg n) via TensorE
    grids (the |p|^2 row-constant cancels in the argmin); quantized distance
    and segment index are packed into one float; a free-dim reduce-max gives
    the argmin; Relu(pk - mx + 1) builds an exact 0/1 one-hot which gathers
    segment data via an XBAR DMA transpose + tiny bf16 matmul.
  item2: same machinery without interpolation (nearest pred per gt key point).

All grid matmuls are bf16 at 1 cycle/row, block-diagonal over 4 instances
(free=512); fp32-level precision comes from splitting each factor into bf16
hi/lo contraction rows (rel err ~2^-17, far below the 2^-10 pack quantum).

Output per core: [sum_loss1, sum_loss2]; host divides by counts and combines.
"""
import sys

for _p in ("/opt/trn_rl_repo",):
    if _p not in sys.path:
        sys.path.insert(0, _p)

import numpy as np

import concourse.bass as bass
import concourse.tile as tile
from concourse import bacc, mybir
from concourse.bass_utils import run_bass_kernel_spmd

dt = mybir.dt
Alu = mybir.AluOpType
Ax = mybir.AxisListType
Act = mybir.ActivationFunctionType
f32 = np.float32

N_CORES = 8
N, P = 256, 128
G = N // N_CORES          # instances per core = 32
BG = 4                    # instances per block
NB = G // BG              # 8 blocks
SC = 131072.0             # distance scale (quantum = 128/SC = 2^-10)
SHIFT = 48.0
BETA = 0.25               # smooth-l1 beta = 1/STRIDE
CQ = float(2 ** 30 + 2 ** 25)       # 1107296256
C1 = CQ - SC * SHIFT                # 1101004800 (exactly representable)
M23 = 8388608.0

# SPL slab indices (each slab is a (32, 128) bf16 plane).  Within a block,
# contraction row k = 8*i + t for instance i; row-type t order per operand:
#   lhsT (pred): [px_hi, py_hi, px_lo, py_lo, px_hi2, py_hi2, 1, 1]
#   rhs   (r):   [r0_hi, r1_hi, r0_hi2, r1_hi2, r0_lo, r1_lo, r2_hi, r2_lo]
# pairing: px_hi*r0_hi + py_hi*r1_hi + px_lo*r0_hi + py_lo*r1_hi
#        + px_hi*r0_lo + py_hi*r1_lo + r2_hi + r2_lo   (lo*lo dropped)
P0 = 0
K0 = 8
RG = 16
RO = 24
RQ = 32
RE = 40         # zeros x6, er_hi, er_lo
TB = 48         # ax_hi ay_hi dx_hi dy_hi ox_hi oy_hi
NSLAB = 54


def _build(nc, pc_d, po_d, gc_d, gk_d, mk_d, out_d):
    FP = dt.float32
    BF = dt.bfloat16

    with tile.TileContext(nc) as tc:
        with (
            tc.tile_pool(name="const", bufs=1) as cpool,
            tc.tile_pool(name="prep", bufs=1) as prep,
            tc.tile_pool(name="oper", bufs=1) as oper,
            tc.tile_pool(name="main", bufs=2) as main,
            tc.tile_pool(name="keep", bufs=1) as keep,
        ):
            V, Gp, S = nc.vector, nc.gpsimd, nc.scalar

            # ---------------- constants ----------------
            iota_i = cpool.tile([128, 128], dt.int32)
            Gp.iota(iota_i[:], pattern=[[1, 128]], channel_multiplier=0)
            iotaF = cpool.tile([128, 128], FP, tag="iotaF")
            V.tensor_copy(iotaF[:], iota_i[:])
            iotaC_i = cpool.tile([128, 1], dt.int32)
            Gp.iota(iotaC_i[:], pattern=[[0, 1]], channel_multiplier=1)
            iotaC = cpool.tile([128, 1], FP, tag="iotaC")
            V.tensor_copy(iotaC[:], iotaC_i[:])
            ident32F = cpool.tile([32, 32], FP, tag="ident32F")
            V.tensor_scalar(ident32F[:], iotaF[0:32, 0:32], iotaC[0:32], None, Alu.is_equal)
            onesc = cpool.tile([128, 1], FP, tag="onesc")
            Gp.memset(onesc[:], 1.0)

            # ---------------- contiguous input loads ----------------
            pc_i = prep.tile([32, 128, 2], FP, tag="pc_i")
            po_i = prep.tile([32, 128, 2], FP, tag="po_i")
            gc_i = prep.tile([32, 128, 2], FP, tag="gc_i")
            gk_i = prep.tile([32, 128, 2], FP, tag="gk_i")
            mk_i = prep.tile([32, 128], FP, tag="mk_i")
            a_i = prep.tile([32, 128, 2], FP, tag="a_i")
            nc.sync.dma_start(pc_i[:], pc_d[:, :, :])
            nc.sync.dma_start(po_i[:], po_d[:, :, :])
            nc.sync.dma_start(gc_i[:], gc_d[:, :, :])
            nc.scalar.dma_start(gk_i[:], gk_d[:, :, :])
            nc.scalar.dma_start(mk_i[:], mk_d[:, :])
            # roll-by-one load: a[p] = gc[p-1 mod 128]
            nc.sync.dma_start(a_i[:, 1:128, :], gc_d[:, 0:127, :])
            nc.scalar.dma_start(a_i[:, 0:1, :], gc_d[:, 127:128, :])

            # ---------------- segment geometry (g-major layout) ----------------
            d_i = prep.tile([32, 128, 2], FP, tag="d_i")
            V.tensor_tensor(d_i[:], gc_i[:], a_i[:], Alu.subtract)
            dsq = prep.tile([32, 128, 2], FP, tag="dsq")
            Gp.tensor_tensor(dsq[:], d_i[:], d_i[:], Alu.mult)
            e = prep.tile([32, 128], FP, tag="e")
            V.tensor_tensor(e[:], dsq[:, :, 0], dsq[:, :, 1], Alu.add)
            einv = prep.tile([32, 128], FP, tag="einv")
            V.reciprocal(einv[:], e[:])
            t_ad = prep.tile([32, 128, 2], FP, tag="t_ad")
            Gp.tensor_tensor(t_ad[:], a_i[:], d_i[:], Alu.mult)
            a2 = prep.tile([32, 128], FP, tag="a2")
            V.tensor_tensor(a2[:], t_ad[:, :, 0], t_ad[:, :, 1], Alu.add)
            asq = prep.tile([32, 128, 2], FP, tag="asq")
            Gp.tensor_tensor(asq[:], a_i[:], a_i[:], Alu.mult)
            zA = prep.tile([32, 128], FP, tag="zA")
            V.tensor_tensor(zA[:], asq[:, :, 0], asq[:, :, 1], Alu.add)
            psq = prep.tile([32, 128, 2], FP, tag="psq")
            Gp.tensor_tensor(psq[:], pc_i[:], pc_i[:], Alu.mult)
            zP = prep.tile([32, 128], FP, tag="zP")
            V.tensor_tensor(zP[:], psq[:, :, 0], psq[:, :, 1], Alu.add)

            # r rows: g' = g - 0.5 = px*r0 + py*r1 + r2'   (-0.5 folded into r2')
            r_01 = prep.tile([32, 128, 2], FP, tag="r_01")
            V.scalar_tensor_tensor(r_01[:, :, 0], d_i[:, :, 0], 10.0, einv[:], Alu.mult, Alu.mult)
            V.scalar_tensor_tensor(r_01[:, :, 1], d_i[:, :, 1], 10.0, einv[:], Alu.mult, Alu.mult)
            r2 = prep.tile([32, 128], FP, tag="r2")
            V.scalar_tensor_tensor(r2[:], a2[:], -10.0, einv[:], Alu.mult, Alu.mult)
            o_01 = prep.tile([32, 128, 2], FP, tag="o_01")
            S.activation(o_01[:], a_i[:], Act.Copy, scale=2.0 * SC)
            o2 = prep.tile([32, 128], FP, tag="o2")
            S.activation(o2[:], zA[:], Act.Copy, scale=-SC)
            er = prep.tile([32, 128], FP, tag="er")
            S.activation(er[:], e[:], Act.Copy, scale=-SC / 100.0)
            q_01 = prep.tile([32, 128, 2], FP, tag="q_01")
            S.activation(q_01[:], pc_i[:], Act.Copy, scale=2.0 * SC)
            q2 = prep.tile([32, 128], FP, tag="q2")
            S.activation(q2[:], zP[:], Act.Copy, scale=-SC)

            # ---------------- bf16 hi/lo splits into SPL slabs ----------------
            SPL = prep.tile([32, NSLAB, 128], BF, tag="SPL")

            def pair_view(s):
                return SPL[:, s:s + 2, :].rearrange("g s q -> g q s")

            tmpP = prep.tile([32, 128, 2], FP, tag="tmpP")

            def split_pair(src, s_hi, s_lo):
                V.tensor_copy(pair_view(s_hi), src)
                if s_lo is not None:
                    S.activation(tmpP[:], pair_view(s_hi), Act.Copy)
                    V.tensor_tensor(pair_view(s_lo), src, tmpP[:], Alu.subtract)

            def split_one(src, s_hi, s_lo):
                V.tensor_copy(SPL[:, s_hi, :], src)
                if s_lo is not None:
                    S.activation(tmpP[:, :, 0], SPL[:, s_hi, :], Act.Copy)
                    V.tensor_tensor(SPL[:, s_lo, :], src, tmpP[:, :, 0], Alu.subtract)

            split_pair(pc_i[:], P0 + 0, P0 + 2)
            split_pair(gk_i[:], K0 + 0, K0 + 2)
            split_pair(r_01[:], RG + 0, RG + 4)
            split_pair(o_01[:], RO + 0, RO + 4)
            split_pair(q_01[:], RQ + 0, RQ + 4)
            split_one(r2[:], RG + 6, RG + 7)
            split_one(o2[:], RO + 6, RO + 7)
            split_one(q2[:], RQ + 6, RQ + 7)
            split_one(er[:], RE + 6, RE + 7)
            # hi-only table slabs
            split_pair(a_i[:], TB + 0, None)
            split_pair(d_i[:], TB + 2, None)
            split_pair(po_i[:], TB + 4, None)
            # duplicated hi rows (re-reads in row pairing)
            V.tensor_copy(SPL[:, P0 + 4:P0 + 6, :], SPL[:, P0 + 0:P0 + 2, :])
            V.tensor_copy(SPL[:, K0 + 4:K0 + 6, :], SPL[:, K0 + 0:K0 + 2, :])
            V.tensor_copy(SPL[:, RG + 2:RG + 4, :], SPL[:, RG + 0:RG + 2, :])
            V.tensor_copy(SPL[:, RO + 2:RO + 4, :], SPL[:, RO + 0:RO + 2, :])
            V.tensor_copy(SPL[:, RQ + 2:RQ + 4, :], SPL[:, RQ + 0:RQ + 2, :])
            # ones / zeros
            Gp.memset(SPL[:, P0 + 6:P0 + 8, :], 1.0)
            Gp.memset(SPL[:, K0 + 6:K0 + 8, :], 1.0)
            Gp.memset(SPL[:, RE + 0:RE + 6, :], 0.0)

            # ---------------- DRAM bounce: slabs -> block-diag operands ------
            slab_d = nc.dram_tensor("slabs", [32, NSLAB, 128], BF)
            nc.sync.dma_start(slab_d[:, :, :], SPL[:])

            # zeros region for rhs backfill
            zero_d = nc.dram_tensor("zeros", [32, 8, 512], BF)
            ztile = prep.tile([32, 512], BF, tag="ztile")
            V.memset(ztile[:], 0.0)
            nc.scalar.dma_start(zero_d[:, 0, :], ztile[:])
            nc.scalar.dma_start(zero_d[:, 1, :], zero_d[:, 0, :])
            nc.scalar.dma_start(zero_d[:, 2:4, :], zero_d[:, 0:2, :])
            nc.scalar.dma_start(zero_d[:, 4:8, :], zero_d[:, 0:4, :])

            lhsP = oper.tile([32, 8, 128], BF, tag="lhsP")
            lhsK = oper.tile([32, 8, 128], BF, tag="lhsK")
            rhsG = oper.tile([32, 8, 512], BF, tag="rhsG")
            rhsO = oper.tile([32, 8, 512], BF, tag="rhsO")
            rhsQ = oper.tile([32, 8, 512], BF, tag="rhsQ")
            rhsE = oper.tile([32, 8, 512], BF, tag="rhsE")
            for rt in (rhsG, rhsO, rhsQ, rhsE):
                nc.sync.dma_start(rt[:], zero_d[:, :, :])
            for i in range(BG):
                src_p = slab_d[i:32:4, P0:P0 + 8, :].rearrange("b t p -> t b p")
                nc.sync.dma_start(lhsP[8 * i:8 * i + 8, :, :], src_p)
                src_k = slab_d[i:32:4, K0:K0 + 8, :].rearrange("b t p -> t b p")
                nc.scalar.dma_start(lhsK[8 * i:8 * i + 8, :, :], src_k)
                for rt, s0 in ((rhsG, RG), (rhsO, RO), (rhsQ, RQ), (rhsE, RE)):
                    eng = nc.sync if (i % 2 == 0) else nc.scalar
                    eng.dma_start(
                        rt[8 * i:8 * i + 8, :, 128 * i:128 * (i + 1)],
                        slab_d[i:32:4, s0:s0 + 8, :].rearrange("b t p -> t b p"),
                    )

            # ---------------- gather tables (XBAR transposes + copies) -------
            # T12[n, g, j]: [Ax, Ay, Dx, Dy, r0h, r0l, r1h, r1l, r2h, r2l]
            T12 = keep.tile([128, G, 10], BF, tag="T12")
            T3c = keep.tile([128, G, 4], BF, tag="T3c")
            t12_src = [TB + 0, TB + 1, TB + 2, TB + 3, RG + 0, RG + 4,
                       RG + 1, RG + 5, RG + 6, RG + 7]
            t3_src = [P0 + 0, P0 + 1, TB + 4, TB + 5]
            for j, s in enumerate(t12_src):
                stg = main.tile([128, 32], BF, tag="stgT")
                eng = nc.sync if (j % 2 == 0) else nc.scalar
                eng.dma_start_transpose(stg[:], SPL[:, s, :])
                V.tensor_copy(T12[:, :, j], stg[:])
            for j, s in enumerate(t3_src):
                stg = main.tile([128, 32], BF, tag="stgT")
                eng = nc.sync if (j % 2 == 0) else nc.scalar
                eng.dma_start_transpose(stg[:], SPL[:, s, :])
                V.tensor_copy(T3c[:, :, j], stg[:])

            # ---------------- f32 transposes for the tail --------------------
            pxP = keep.tile([128, G], FP, tag="pxP")
            pyP = keep.tile([128, G], FP, tag="pyP")
            oxP = keep.tile([128, G], FP, tag="oxP")
            oyP = keep.tile([128, G], FP, tag="oyP")
            kxP = keep.tile([128, G], FP, tag="kxP")
            kyP = keep.tile([128, G], FP, tag="kyP")
            mkP = keep.tile([128, G], FP, tag="mkP")
            with tc.tile_pool(name="ps_prep", bufs=3, space="PSUM") as ps_prep:
                for dst, src in ((pxP, pc_i[:, :, 0]), (pyP, pc_i[:, :, 1]),
                                 (oxP, po_i[:, :, 0]), (oyP, po_i[:, :, 1]),
                                 (kxP, gk_i[:, :, 0]), (kyP, gk_i[:, :, 1]),
                                 (mkP, mk_i[:])):
                    fps = ps_prep.tile([128, 32], FP, tag="tpsF")
                    nc.tensor.transpose(fps[:], src, ident32F[:])
                    S.activation(dst[:], fps[:], Act.Copy)

            # gathered values: [0:10]=T12 slots, [10:14]=T3c slots
            ex = keep.tile([128, G, 14], FP, tag="ex")

            iotaB = iotaF[:].rearrange("p (o q) -> p o q", o=1).broadcast_to([128, BG, 128])

            ps_grid_cm = tc.tile_pool(name="ps_grid", bufs=1, space="PSUM")
            ps_ex_cm = tc.tile_pool(name="ps_ex", bufs=2, space="PSUM")
            ps_out_cm = tc.tile_pool(name="ps_out", bufs=1, space="PSUM")
            ps_grid = ps_grid_cm.__enter__()
            ps_ex = ps_ex_cm.__enter__()
            ps_out = ps_out_cm.__enter__()

            # ---------------- main loop ----------------
            for b in range(NB):
                g0 = b * BG
                gps = ps_grid.tile([128, BG, 128], FP, tag="gps")
                ops = ps_grid.tile([128, BG, 128], FP, tag="ops")
                erep = ps_grid.tile([128, BG, 128], FP, tag="erep")
                d2ps = ps_grid.tile([128, BG, 128], FP, tag="d2ps")
                gv = gps[:].rearrange("p i n -> p (i n)")
                ov = ops[:].rearrange("p i n -> p (i n)")
                ev = erep[:].rearrange("p i n -> p (i n)")
                dv = d2ps[:].rearrange("p i n -> p (i n)")
                nc.tensor.matmul(gv, lhsP[:, b, :], rhsG[:, b, :], start=True, stop=True)
                nc.tensor.matmul(ov, lhsP[:, b, :], rhsO[:, b, :], start=True, stop=True)
                nc.tensor.matmul(ev, lhsP[:, b, :], rhsE[:, b, :], start=True, stop=True)
                nc.tensor.matmul(dv, lhsK[:, b, :], rhsQ[:, b, :], start=True, stop=True)

                # -------- item1 chain:  t = clamp(round(g'), 0, 9) --------
                s2t = main.tile([128, BG, 128], FP, tag="s2t")
                S.activation(s2t[:], gps[:], Act.Copy, bias=M23)
                s3t = main.tile([128, BG, 128], FP, tag="s3t")
                S.activation(s3t[:], s2t[:], Act.Copy, bias=-M23)
                t = main.tile([128, BG, 128], FP, tag="t")
                V.tensor_scalar(t[:], s3t[:], 0.0, 9.0, Alu.max, Alu.min)
                hq = main.tile([128, BG, 128], FP, tag="hq")
                V.scalar_tensor_tensor(hq[:], gps[:], -2.0, t[:], Alu.mult, Alu.add)
                q = main.tile([128, BG, 128], FP, tag="q")
                Gp.tensor_tensor(q[:], hq[:], t[:], Alu.mult)
                vE = main.tile([128, BG, 128], FP, tag="vE")
                V.tensor_tensor(vE[:], q[:], erep[:], Alu.mult)
                dqA = main.tile([128, BG, 128], FP, tag="dqA")
                V.scalar_tensor_tensor(dqA[:], vE[:], C1, ops[:], Alu.add, Alu.add)
                dq = main.tile([128, BG, 128], FP, tag="dq")
                S.activation(dq[:], dqA[:], Act.Copy, bias=-CQ)
                pkN = main.tile([128, BG, 128], FP, tag="pkN")
                Gp.tensor_tensor(pkN[:], dq[:], iotaB, Alu.subtract)
                mx = main.tile([128, BG], FP, tag="mx")
                V.tensor_reduce(mx[:], pkN[:], Ax.X, Alu.max)
                mxb1 = main.tile([128, BG], FP, tag="mxb1")
                V.tensor_scalar(mxb1[:], mx[:], -1.0, 1.0, Alu.mult, Alu.add)
                oh = main.tile([128, BG, 128], BF, tag="oh")
                for i in range(BG):
                    S.activation(oh[:, i, :], pkN[:, i, :], Act.Relu, bias=mxb1[:, i:i + 1])

                # -------- item2 chain --------
                dq2 = main.tile([128, BG, 128], FP, tag="dq2")
                V.tensor_scalar(dq2[:], d2ps[:], C1, CQ, Alu.add, Alu.subtract)
                pk2 = main.tile([128, BG, 128], FP, tag="pk2")
                Gp.tensor_tensor(pk2[:], dq2[:], iotaB, Alu.subtract)
                mx2 = main.tile([128, BG], FP, tag="mx2")
                V.tensor_reduce(mx2[:], pk2[:], Ax.X, Alu.max)
                mxb2 = main.tile([128, BG], FP, tag="mxb2")
                V.tensor_scalar(mxb2[:], mx2[:], -1.0, 1.0, Alu.mult, Alu.add)
                oh2 = main.tile([128, BG, 128], BF, tag="oh2")
                for i in range(BG):
                    S.activation(oh2[:, i, :], pk2[:, i, :], Act.Relu, bias=mxb2[:, i:i + 1])

                # -------- XBAR-transpose one-hots, gather via matmul --------
                ohT = main.tile([128, BG, 128], BF, tag="ohT")
                oh2T = main.tile([128, BG, 128], BF, tag="oh2T")
                nc.sync.dma_start_transpose(
                    ohT[:], oh[:].rearrange("m i n -> m (i n)"))
                nc.sync.dma_start_transpose(
                    oh2T[:], oh2[:].rearrange("m i n -> m (i n)"))

                exPS = ps_ex.tile([128, BG, 14], FP, tag="exPS")
                for i in range(BG):
                    g = g0 + i
                    nc.tensor.matmul(exPS[:, i, 0:10], ohT[:, i, :], T12[:, g, :], start=True, stop=True)
                    nc.tensor.matmul(exPS[:, i, 10:14], oh2T[:, i, :], T3c[:, g, :], start=True, stop=True)
                V.tensor_copy(ex[:, g0:g0 + BG, :], exPS[:])

            # ---------------- tail (128 x G f32 ops) ----------------
            tl = prep

            def TT(name, a, bb, op, eng=V):
                r = tl.tile([128, G], FP, tag=name)
                eng.tensor_tensor(r[:], a, bb, op)
                return r

            r0 = TT("r0", ex[:, :, 4], ex[:, :, 5], Alu.add)
            r1 = TT("r1", ex[:, :, 6], ex[:, :, 7], Alu.add, Gp)
            r2t = TT("r2t", ex[:, :, 8], ex[:, :, 9], Alu.add)
            v1 = TT("v1", pxP[:], r0[:], Alu.mult, Gp)
            v2 = TT("v2", pyP[:], r1[:], Alu.mult)
            gst = TT("gst", v1[:], v2[:], Alu.add, Gp)
            gst = TT("gst2", gst[:], r2t[:], Alu.add)
            # t* = clamp(round(gst), 0, 9)
            c2t = tl.tile([128, G], FP, tag="c2t")
            S.activation(c2t[:], gst[:], Act.Copy, bias=M23)
            c3t = tl.tile([128, G], FP, tag="c3t")
            S.activation(c3t[:], c2t[:], Act.Copy, bias=-M23)
            tst = tl.tile([128, G], FP, tag="tst")
            V.tensor_scalar(tst[:], c3t[:], 0.0, 9.0, Alu.max, Alu.min)
            # target = A + 0.1 * t* * D
            m1 = TT("m1", tst[:], ex[:, :, 2], Alu.mult, Gp)
            tgx = tl.tile([128, G], FP, tag="tgx")
            V.scalar_tensor_tensor(tgx[:], m1[:], 0.1, ex[:, :, 0], Alu.mult, Alu.add)
            m2 = TT("m2", tst[:], ex[:, :, 3], Alu.mult, Gp)
            tgy = tl.tile([128, G], FP, tag="tgy")
            V.scalar_tensor_tensor(tgy[:], m2[:], 0.1, ex[:, :, 1], Alu.mult, Alu.add)

            def smooth_l1_sum(pred_x, pred_y, tx, ty, px_, py_, name):
                # sum over coords of smooth_l1(pred - (t - p)*0.25)
                acc = None
                for ci, (pr, tt_, pp) in enumerate(((pred_x, tx, px_), (pred_y, ty, py_))):
                    sfx = name + str(ci)
                    e1 = TT(sfx + "e1", tt_, pp, Alu.subtract, Gp)
                    dfe = tl.tile([128, G], FP, tag=sfx + "dfe")
                    V.scalar_tensor_tensor(dfe[:], e1[:], -0.25, pr, Alu.mult, Alu.add)
                    ad = tl.tile([128, G], FP, tag=sfx + "ad")
                    S.activation(ad[:], dfe[:], Act.Abs)
                    m = tl.tile([128, G], FP, tag=sfx + "m")
                    V.tensor_scalar(m[:], ad[:], BETA, None, Alu.min)
                    uu = tl.tile([128, G], FP, tag=sfx + "u")
                    V.scalar_tensor_tensor(uu[:], m[:], -0.5, ad[:], Alu.mult, Alu.add)
                    sl = tl.tile([128, G], FP, tag=sfx + "sl")
                    V.scalar_tensor_tensor(sl[:], m[:], 4.0, uu[:], Alu.mult, Alu.mult)
                    if acc is None:
                        acc = sl
                    else:
                        acc = TT(name + "acc", acc[:], sl[:], Alu.add, Gp)
                return acc

            s1 = smooth_l1_sum(oxP[:], oyP[:], tgx[:], tgy[:], pxP[:], pyP[:], "i1")
            s2 = smooth_l1_sum(ex[:, :, 12], ex[:, :, 13], kxP[:], kyP[:],
                               ex[:, :, 10], ex[:, :, 11], "i2")
            s2 = TT("s2m", s2[:], mkP[:], Alu.mult)

            # ---------------- reduce to 2 scalars ----------------
            s1r = tl.tile([128, 1], FP, tag="s1r")
            V.tensor_reduce(s1r[:], s1[:], Ax.X, Alu.add)
            s2r = tl.tile([128, 1], FP, tag="s2r")
            V.tensor_reduce(s2r[:], s2[:], Ax.X, Alu.add)
            sboth = tl.tile([128, 2], FP, tag="sboth")
            V.tensor_copy(sboth[:, 0:1], s1r[:])
            V.tensor_copy(sboth[:, 1:2], s2r[:])
            sc_ps = ps_out.tile([2, 1], FP, tag="sc_ps")
            nc.tensor.matmul(sc_ps[:], sboth[:], onesc[:], start=True, stop=True)
            outsb = tl.tile([2, 1], FP, tag="outsb")
            V.tensor_copy(outsb[:], sc_ps[:])
            nc.sync.dma_start(out_d[:].rearrange("(a b) -> a b", b=1), outsb[:])
            ps_out_cm.__exit__(None, None, None)
            ps_ex_cm.__exit__(None, None, None)
            ps_grid_cm.__exit__(None, None, None)

    return nc


_CACHE = {}


def _get_program():
    if "nc" not in _CACHE:
        nc = bacc.Bacc("TRN2", target_bir_lowering=False, num_devices=N_CORES)
        pc_d = nc.declare_dram_parameter("pc", [G, P, 2], dt.float32, isOutput=False)
        po_d = nc.declare_dram_parameter("po", [G, P, 2], dt.float32, isOutput=False)
        gc_d = nc.declare_dram_parameter("gc", [G, P, 2], dt.float32, isOutput=False)
        gk_d = nc.declare_dram_parameter("gk", [G, P, 2], dt.float32, isOutput=False)
        mk_d = nc.declare_dram_parameter("mk", [G, P], dt.float32, isOutput=False)
        out_d = nc.declare_dram_parameter("out", [2], dt.float32, isOutput=True)
        _build(nc, pc_d[:], po_d[:], gc_d[:], gk_d[:], mk_d[:], out_d[:])
        nc.compile()
        _CACHE["nc"] = nc
    return _CACHE["nc"]


def _in_maps(inputs):
    pc = np.ascontiguousarray(inputs["pred_contours"], dtype=np.float32)
    po = np.ascontiguousarray(inputs["pred_offsets"], dtype=np.float32)
    gc = np.ascontiguousarray(inputs["gt_contours"], dtype=np.float32)
    gk = np.ascontiguousarray(inputs["gt_key_points"], dtype=np.float32)
    mk = np.ascontiguousarray(inputs["gt_key_points_mask"]).astype(np.float32)
    maps = []
    for c in range(N_CORES):
        s = slice(c * G, (c + 1) * G)
        maps.append({
            "pc": pc[s], "po": po[s], "gc": gc[s], "gk": gk[s], "mk": mk[s],
        })
    return maps


def kernel(pred_contours, pred_offsets, gt_contours, gt_key_points, gt_key_points_mask,
           _results_hook=None):
    inputs = {
        "pred_contours": pred_contours,
        "pred_offsets": pred_offsets,
        "gt_contours": gt_contours,
        "gt_key_points": gt_key_points,
        "gt_key_points_mask": gt_key_points_mask,
    }
    nc = _get_program()
    res = run_bass_kernel_spmd(nc, _in_maps(inputs), list(range(N_CORES)))
    if _results_hook is not None:
        _results_hook(res)
    s1 = f32(0.0)
    s2 = f32(0.0)
    for r in res.results:
        s1 = f32(s1 + f32(r["out"][0]))
        s2 = f32(s2 + f32(r["out"][1]))
    cnt1 = f32(N * P * 2)
    cnt2 = f32(max(float(np.sum(gt_key_points_mask != 0)) * 2.0, 1.0))
    loss = f32(f32(s1 / cnt1) * f32(0.5) + f32(s2 / cnt2) * f32(0.5))
    return np.asarray(loss, dtype=np.float32)


# revision 15
# speedup vs baseline: 3.7006x; 1.0223x over previous
"""Trainium2 Bass kernel for nn_DMLoss (contour matching loss), 8-core data parallel.

v4: block-diagonal bf16 split-precision matmuls with the C1 rounding offset
folded in as extra contraction rows, XBAR DMA-transposed one-hots, combined
packed argmin reduce, engine-balanced chain, piecewise prep DMA overlap.

Per instance (P=128 points, TIME=10):
  item1: nearest of 1280 interpolated gt points per pred point.  Segment n
    spans A_n = gt[n-1]..gt[n]; g' = 10*u - 0.5 with u = <p-A,D>/|D|^2; best
    discrete t = clamp(round(g'), 0, 9); dist^2 = |p-A|^2 + (e/100)*t*(t-2g').
    TensorE produces grids g', opsC1 = SC*(2<p,A> - |A|^2) + C1 and
    erep = -SC*e/100 (bf16 hi/lo split rows, C1 = C1H + C1L exactly).
    dq = (q*erep + opsC1) - CQ lands exactly on the 128-grid; pk = dq - n
    packs quantized distance + segment index; reduce-max = argmin;
    exact 0/1 one-hots gather segment data via XBAR transpose + bf16 matmul.
  item2: same machinery without interpolation (nearest pred per key point).

Output per core: [sum_loss1, sum_loss2]; host divides by counts and combines.
"""
import sys

for _p in ("/opt/trn_rl_repo",):
    if _p not in sys.path:
        sys.path.insert(0, _p)

import numpy as np

import concourse.bass as bass
import concourse.tile as tile
from concourse import bacc, mybir
from concourse.bass_utils import run_bass_kernel_spmd

dt = mybir.dt
Alu = mybir.AluOpType
Ax = mybir.AxisListType
Act = mybir.ActivationFunctionType
f32 = np.float32

N_CORES = 8
N, P = 256, 128
G = N // N_CORES          # instances per core = 32
BG = 4                    # instances per block
NB = G // BG              # 8 blocks
SC = 131072.0             # distance scale (quantum = 128/SC = 2^-10)
SHIFT = 48.0
BETA = 0.25               # smooth-l1 beta = 1/STRIDE
CQ = float(2 ** 30 + 2 ** 25)       # 1107296256
C1 = CQ - SC * SHIFT                # 1101004800
C1H = 1098907648.0                  # bf16-exact hi part of C1
C1L = 2097152.0                     # bf16-exact lo part (C1H + C1L == C1)
M23 = 8388608.0

# SPL slab regions, 10 row-types per operand (k = 10*i + t within a block):
#   lhsT: [x_hi, y_hi, x_lo, y_lo, x_hi2, y_hi2, 1, 1, 1, 1]
#   rhs:  [u_hi, v_hi, u_hi2, v_hi2, u_lo, v_lo, c_hi, c_lo, w8, w9]
# pairing t: xh*uh + yh*vh + xl*uh + yl*vh + xh*ul + yh*vl + ch + cl + w8 + w9
P0 = 0
K0 = 10
RG = 20         # w8 = w9 = 0
RO = 30         # w8, w9 = C1H, C1L
RQ = 40         # w8, w9 = C1H, C1L
RE = 50         # [z z z z z z er_hi er_lo z z]
TB = 60         # ax_hi ay_hi dx_hi dy_hi ox_hi oy_hi
NSLAB = 66
KB = 10 * BG    # block contraction rows = 40


def _build(nc, pc_d, po_d, gc_d, gk_d, mk_d, out_d):
    FP = dt.float32
    BF = dt.bfloat16

    with tile.TileContext(nc) as tc:
        with (
            tc.tile_pool(name="const", bufs=1) as cpool,
            tc.tile_pool(name="prep", bufs=1) as prep,
            tc.tile_pool(name="oper", bufs=1) as oper,
            tc.tile_pool(name="main", bufs=2) as main,
            tc.tile_pool(name="keep", bufs=1) as keep,
        ):
            V, Gp, S = nc.vector, nc.gpsimd, nc.scalar

            # ---------------- constants ----------------
            iota_i = cpool.tile([128, 128], dt.int32)
            Gp.iota(iota_i[:], pattern=[[1, 128]], channel_multiplier=0)
            iotaF = cpool.tile([128, 128], FP, tag="iotaF")
            V.tensor_copy(iotaF[:], iota_i[:])
            iotaC_i = cpool.tile([128, 1], dt.int32)
            Gp.iota(iotaC_i[:], pattern=[[0, 1]], channel_multiplier=1)
            iotaC = cpool.tile([128, 1], FP, tag="iotaC")
            V.tensor_copy(iotaC[:], iotaC_i[:])
            ident32F = cpool.tile([32, 32], FP, tag="ident32F")
            V.tensor_scalar(ident32F[:], iotaF[0:32, 0:32], iotaC[0:32], None, Alu.is_equal)
            onesc = cpool.tile([128, 1], FP, tag="onesc")
            Gp.memset(onesc[:], 1.0)

            # ---------------- contiguous input loads ----------------
            pc_i = prep.tile([32, 128, 2], FP, tag="pc_i")
            po_i = prep.tile([32, 128, 2], FP, tag="po_i")
            gc_i = prep.tile([32, 128, 2], FP, tag="gc_i")
            gk_i = prep.tile([32, 128, 2], FP, tag="gk_i")
            mk_i = prep.tile([32, 128], FP, tag="mk_i")
            a_i = prep.tile([32, 128, 2], FP, tag="a_i")
            nc.sync.dma_start(pc_i[:], pc_d[:, :, :])
            nc.sync.dma_start(po_i[:], po_d[:, :, :])
            nc.sync.dma_start(gc_i[:], gc_d[:, :, :])
            nc.scalar.dma_start(gk_i[:], gk_d[:, :, :])
            nc.scalar.dma_start(mk_i[:], mk_d[:, :])
            nc.sync.dma_start(a_i[:, 1:128, :], gc_d[:, 0:127, :])
            nc.scalar.dma_start(a_i[:, 0:1, :], gc_d[:, 127:128, :])

            # ---------------- segment geometry (g-major layout) ----------------
            d_i = prep.tile([32, 128, 2], FP, tag="d_i")
            V.tensor_tensor(d_i[:], gc_i[:], a_i[:], Alu.subtract)
            dsq = prep.tile([32, 128, 2], FP, tag="dsq")
            Gp.tensor_tensor(dsq[:], d_i[:], d_i[:], Alu.mult)
            e = prep.tile([32, 128], FP, tag="e")
            V.tensor_tensor(e[:], dsq[:, :, 0], dsq[:, :, 1], Alu.add)
            einv = prep.tile([32, 128], FP, tag="einv")
            V.reciprocal(einv[:], e[:])
            t_ad = prep.tile([32, 128, 2], FP, tag="t_ad")
            Gp.tensor_tensor(t_ad[:], a_i[:], d_i[:], Alu.mult)
            a2 = prep.tile([32, 128], FP, tag="a2")
            V.tensor_tensor(a2[:], t_ad[:, :, 0], t_ad[:, :, 1], Alu.add)
            asq = prep.tile([32, 128, 2], FP, tag="asq")
            Gp.tensor_tensor(asq[:], a_i[:], a_i[:], Alu.mult)
            zA = prep.tile([32, 128], FP, tag="zA")
            V.tensor_tensor(zA[:], asq[:, :, 0], asq[:, :, 1], Alu.add)
            psq = prep.tile([32, 128, 2], FP, tag="psq")
            Gp.tensor_tensor(psq[:], pc_i[:], pc_i[:], Alu.mult)
            zP = prep.tile([32, 128], FP, tag="zP")
            V.tensor_tensor(zP[:], psq[:, :, 0], psq[:, :, 1], Alu.add)

            # r rows: g' = g - 0.5 = px*r0 + py*r1 + r2'   (-0.5 folded into r2')
            r_01 = prep.tile([32, 128, 2], FP, tag="r_01")
            V.scalar_tensor_tensor(r_01[:, :, 0], d_i[:, :, 0], 10.0, einv[:], Alu.mult, Alu.mult)
            V.scalar_tensor_tensor(r_01[:, :, 1], d_i[:, :, 1], 10.0, einv[:], Alu.mult, Alu.mult)
            r2 = prep.tile([32, 128], FP, tag="r2")
            V.scalar_tensor_tensor(r2[:], a2[:], -10.0, einv[:], Alu.mult, Alu.mult)
            o_01 = prep.tile([32, 128, 2], FP, tag="o_01")
            S.activation(o_01[:], a_i[:], Act.Copy, scale=2.0 * SC)
            o2 = prep.tile([32, 128], FP, tag="o2")
            S.activation(o2[:], zA[:], Act.Copy, scale=-SC)
            er = prep.tile([32, 128], FP, tag="er")
            S.activation(er[:], e[:], Act.Copy, scale=-SC / 100.0)
            q_01 = prep.tile([32, 128, 2], FP, tag="q_01")
            S.activation(q_01[:], pc_i[:], Act.Copy, scale=2.0 * SC)
            q2 = prep.tile([32, 128], FP, tag="q2")
            S.activation(q2[:], zP[:], Act.Copy, scale=-SC)

            # ---------------- bf16 hi/lo splits into SPL slabs ----------------
            SPL = prep.tile([32, NSLAB, 128], BF, tag="SPL")

            def pair_view(s):
                return SPL[:, s:s + 2, :].rearrange("g s q -> g q s")

            def split_pair(src, s_hi, s_lo):
                V.tensor_copy(pair_view(s_hi), src)
                if s_lo is not None:
                    # mixed-dtype subtract: f32 - bf16 -> bf16
                    V.tensor_tensor(pair_view(s_lo), src, pair_view(s_hi), Alu.subtract)

            def split_one(src, s_hi, s_lo):
                V.tensor_copy(SPL[:, s_hi, :], src)
                if s_lo is not None:
                    V.tensor_tensor(SPL[:, s_lo, :], src, SPL[:, s_hi, :], Alu.subtract)

            split_pair(pc_i[:], P0 + 0, P0 + 2)
            split_pair(gk_i[:], K0 + 0, K0 + 2)
            split_pair(r_01[:], RG + 0, RG + 4)
            split_pair(o_01[:], RO + 0, RO + 4)
            split_pair(q_01[:], RQ + 0, RQ + 4)
            split_one(r2[:], RG + 6, RG + 7)
            split_one(o2[:], RO + 6, RO + 7)
            split_one(q2[:], RQ + 6, RQ + 7)
            split_one(er[:], RE + 6, RE + 7)
            split_pair(a_i[:], TB + 0, None)
            split_pair(d_i[:], TB + 2, None)
            split_pair(po_i[:], TB + 4, None)
            # duplicated hi rows
            V.tensor_copy(SPL[:, P0 + 4:P0 + 6, :], SPL[:, P0 + 0:P0 + 2, :])
            V.tensor_copy(SPL[:, K0 + 4:K0 + 6, :], SPL[:, K0 + 0:K0 + 2, :])
            V.tensor_copy(SPL[:, RG + 2:RG + 4, :], SPL[:, RG + 0:RG + 2, :])
            V.tensor_copy(SPL[:, RO + 2:RO + 4, :], SPL[:, RO + 0:RO + 2, :])
            V.tensor_copy(SPL[:, RQ + 2:RQ + 4, :], SPL[:, RQ + 0:RQ + 2, :])
            # ones / zeros / C1 rows
            Gp.memset(SPL[:, P0 + 6:P0 + 10, :], 1.0)
            Gp.memset(SPL[:, K0 + 6:K0 + 10, :], 1.0)
            Gp.memset(SPL[:, RG + 8:RG + 10, :], 0.0)
            Gp.memset(SPL[:, RO + 8, :], C1H)
            Gp.memset(SPL[:, RO + 9, :], C1L)
            Gp.memset(SPL[:, RQ + 8, :], C1H)
            Gp.memset(SPL[:, RQ + 9, :], C1L)
            Gp.memset(SPL[:, RE + 0:RE + 6, :], 0.0)
            Gp.memset(SPL[:, RE + 8:RE + 10, :], 0.0)

            # ---------------- DRAM bounce: piecewise stores ------------------
            slab_d = nc.dram_tensor("slabs", [32, NSLAB, 128], BF)
            for j, (s0, n) in enumerate(((P0, 10), (K0, 10), (RG, 10), (RO, 10),
                                         (RQ, 10), (RE, 10))):
                eng = nc.sync if (j % 2 == 0) else nc.scalar
                eng.dma_start(slab_d[:, s0:s0 + n, :], SPL[:, s0:s0 + n, :])

            # zeros region for rhs backfill
            zero_d = nc.dram_tensor("zeros", [KB, 8, 512], BF)
            ztile = prep.tile([KB, 512], BF, tag="ztile")
            V.memset(ztile[:], 0.0)
            nc.scalar.dma_start(zero_d[:, 0, :], ztile[:])
            nc.scalar.dma_start(zero_d[:, 1, :], zero_d[:, 0, :])
            nc.scalar.dma_start(zero_d[:, 2:4, :], zero_d[:, 0:2, :])
            nc.scalar.dma_start(zero_d[:, 4:8, :], zero_d[:, 0:4, :])

            lhsP = oper.tile([KB, 8, 128], BF, tag="lhsP")
            lhsK = oper.tile([KB, 8, 128], BF, tag="lhsK")
            rhsG = oper.tile([KB, 8, 512], BF, tag="rhsG")
            rhsO = oper.tile([KB, 8, 512], BF, tag="rhsO")
            rhsQ = oper.tile([KB, 8, 512], BF, tag="rhsQ")
            rhsE = oper.tile([KB, 8, 512], BF, tag="rhsE")
            for rt in (rhsG, rhsO, rhsQ, rhsE):
                nc.sync.dma_start(rt[:], zero_d[:, :, :])
            for i in range(BG):
                src_p = slab_d[i:32:4, P0:P0 + 10, :].rearrange("b t p -> t b p")
                nc.sync.dma_start(lhsP[10 * i:10 * i + 10, :, :], src_p)
                src_k = slab_d[i:32:4, K0:K0 + 10, :].rearrange("b t p -> t b p")
                nc.scalar.dma_start(lhsK[10 * i:10 * i + 10, :, :], src_k)
                for rt, s0 in ((rhsG, RG), (rhsO, RO), (rhsQ, RQ), (rhsE, RE)):
                    eng = nc.sync if (i % 2 == 0) else nc.scalar
                    eng.dma_start(
                        rt[10 * i:10 * i + 10, :, 128 * i:128 * (i + 1)],
                        slab_d[i:32:4, s0:s0 + 10, :].rearrange("b t p -> t b p"),
                    )

            # ---------------- gather tables (XBAR transposes + copies) -------
            T12 = keep.tile([128, G, 10], BF, tag="T12")
            T3c = keep.tile([128, G, 4], BF, tag="T3c")
            t12_src = [TB + 0, TB + 1, TB + 2, TB + 3, RG + 0, RG + 4,
                       RG + 1, RG + 5, RG + 6, RG + 7]
            t3_src = [P0 + 0, P0 + 1, TB + 4, TB + 5]
            for j, s in enumerate(t12_src):
                stg = main.tile([128, 32], BF, tag="stgT")
                eng = nc.sync if (j % 2 == 0) else nc.scalar
                eng.dma_start_transpose(stg[:], SPL[:, s, :])
                V.tensor_copy(T12[:, :, j], stg[:])
            for j, s in enumerate(t3_src):
                stg = main.tile([128, 32], BF, tag="stgT")
                eng = nc.sync if (j % 2 == 0) else nc.scalar
                eng.dma_start_transpose(stg[:], SPL[:, s, :])
                V.tensor_copy(T3c[:, :, j], stg[:])

            # ---------------- f32 transposes for the tail --------------------
            pxP = keep.tile([128, G], FP, tag="pxP")
            pyP = keep.tile([128, G], FP, tag="pyP")
            oxP = keep.tile([128, G], FP, tag="oxP")
            oyP = keep.tile([128, G], FP, tag="oyP")
            kxP = keep.tile([128, G], FP, tag="kxP")
            kyP = keep.tile([128, G], FP, tag="kyP")
            mkP = keep.tile([128, G], FP, tag="mkP")
            with tc.tile_pool(name="ps_prep", bufs=3, space="PSUM") as ps_prep:
                for dst, src in ((pxP, pc_i[:, :, 0]), (pyP, pc_i[:, :, 1]),
                                 (oxP, po_i[:, :, 0]), (oyP, po_i[:, :, 1]),
                                 (kxP, gk_i[:, :, 0]), (kyP, gk_i[:, :, 1]),
                                 (mkP, mk_i[:])):
                    fps = ps_prep.tile([128, 32], FP, tag="tpsF")
                    nc.tensor.transpose(fps[:], src, ident32F[:])
                    S.activation(dst[:], fps[:], Act.Copy)

            ex = keep.tile([128, G, 14], FP, tag="ex")
            iotaB = iotaF[:].rearrange("p (o q) -> p o q", o=1).broadcast_to([128, BG, 128])

            ps_grid_cm = tc.tile_pool(name="ps_grid", bufs=1, space="PSUM")
            ps_d2_cm = tc.tile_pool(name="ps_d2", bufs=2, space="PSUM")
            ps_ex_cm = tc.tile_pool(name="ps_ex", bufs=2, space="PSUM")
            ps_out_cm = tc.tile_pool(name="ps_out", bufs=1, space="PSUM")
            ps_grid = ps_grid_cm.__enter__()
            ps_d2 = ps_d2_cm.__enter__()
            ps_ex = ps_ex_cm.__enter__()
            ps_out = ps_out_cm.__enter__()

            # ---------------- main loop ----------------
            for b in range(NB):
                g0 = b * BG
                gps = ps_grid.tile([128, BG, 128], FP, tag="gps")
                ops = ps_grid.tile([128, BG, 128], FP, tag="ops")
                erep = ps_grid.tile([128, BG, 128], FP, tag="erep")
                d2ps = ps_d2.tile([128, BG, 128], FP, tag="d2ps")
                gv = gps[:].rearrange("p i n -> p (i n)")
                ov = ops[:].rearrange("p i n -> p (i n)")
                ev = erep[:].rearrange("p i n -> p (i n)")
                dv = d2ps[:].rearrange("p i n -> p (i n)")
                nc.tensor.matmul(gv, lhsP[:, b, :], rhsG[:, b, :], start=True, stop=True)
                nc.tensor.matmul(ov, lhsP[:, b, :], rhsO[:, b, :], start=True, stop=True)
                nc.tensor.matmul(ev, lhsP[:, b, :], rhsE[:, b, :], start=True, stop=True)
                nc.tensor.matmul(dv, lhsK[:, b, :], rhsQ[:, b, :], start=True, stop=True)

                # -------- item1: t = clamp(round(g'), 0, 9) --------
                s2t = main.tile([128, BG, 128], FP, tag="s2t")
                S.activation(s2t[:], gps[:], Act.Copy, bias=M23)
                s3t = main.tile([128, BG, 128], FP, tag="s3t")
                S.activation(s3t[:], s2t[:], Act.Copy, bias=-M23)
                t = main.tile([128, BG, 128], FP, tag="t")
                V.tensor_scalar(t[:], s3t[:], 0.0, 9.0, Alu.max, Alu.min)
                hq = main.tile([128, BG, 128], FP, tag="hq")
                V.scalar_tensor_tensor(hq[:], gps[:], -2.0, t[:], Alu.mult, Alu.add)
                q = main.tile([128, BG, 128], FP, tag="q")
                Gp.tensor_tensor(q[:], hq[:], t[:], Alu.mult)
                vE = main.tile([128, BG, 128], FP, tag="vE")
                V.tensor_tensor(vE[:], q[:], erep[:], Alu.mult)
                dqA = main.tile([128, BG, 128], FP, tag="dqA")
                V.tensor_tensor(dqA[:], vE[:], ops[:], Alu.add)
                dq = main.tile([128, BG, 128], FP, tag="dq")
                S.activation(dq[:], dqA[:], Act.Copy, bias=-CQ)
                dq2 = main.tile([128, BG, 128], FP, tag="dq2")
                S.activation(dq2[:], d2ps[:], Act.Copy, bias=-CQ)

                pkB = main.tile([128, BG, 2, 128], FP, tag="pkB")
                Gp.tensor_tensor(pkB[:, :, 0, :], dq[:], iotaB, Alu.subtract)
                Gp.tensor_tensor(pkB[:, :, 1, :], dq2[:], iotaB, Alu.subtract)
                mxB = main.tile([128, BG, 2], FP, tag="mxB")
                V.tensor_reduce(mxB[:], pkB[:], Ax.X, Alu.max)
                mxb2 = main.tile([128, BG], FP, tag="mxb2")
                V.tensor_scalar(mxb2[:], mxB[:, :, 1], -1.0, 1.0, Alu.mult, Alu.add)

                oh = main.tile([128, BG, 128], BF, tag="oh")
                oh2 = main.tile([128, BG, 128], BF, tag="oh2")
                for i in range(BG):
                    V.tensor_scalar(oh[:, i, :], pkB[:, i, 0, :], mxB[:, i, 0:1], None, Alu.is_equal)
                    S.activation(oh2[:, i, :], pkB[:, i, 1, :], Act.Relu, bias=mxb2[:, i:i + 1])

                # -------- XBAR-transpose one-hots, gather via matmul --------
                ohT = main.tile([128, BG, 128], BF, tag="ohT")
                oh2T = main.tile([128, BG, 128], BF, tag="oh2T")
                nc.sync.dma_start_transpose(ohT[:], oh[:].rearrange("m i n -> m (i n)"))
                nc.sync.dma_start_transpose(oh2T[:], oh2[:].rearrange("m i n -> m (i n)"))

                exPS = ps_ex.tile([128, BG, 14], FP, tag="exPS")
                for i in range(BG):
                    g = g0 + i
                    nc.tensor.matmul(exPS[:, i, 0:10], ohT[:, i, :], T12[:, g, :], start=True, stop=True)
                    nc.tensor.matmul(exPS[:, i, 10:14], oh2T[:, i, :], T3c[:, g, :], start=True, stop=True)
                V.tensor_copy(ex[:, g0:g0 + BG, :], exPS[:])

            # ---------------- tail (128 x G f32 ops) ----------------
            tl = prep

            def TT(name, a, bb, op, eng=V):
                r = tl.tile([128, G], FP, tag=name)
                eng.tensor_tensor(r[:], a, bb, op)
                return r

            r0 = TT("r0", ex[:, :, 4], ex[:, :, 5], Alu.add)
            r1 = TT("r1", ex[:, :, 6], ex[:, :, 7], Alu.add, Gp)
            r2t = TT("r2t", ex[:, :, 8], ex[:, :, 9], Alu.add)
            v1 = TT("v1", pxP[:], r0[:], Alu.mult, Gp)
            v2 = TT("v2", pyP[:], r1[:], Alu.mult)
            gst = TT("gst", v1[:], v2[:], Alu.add, Gp)
            gst = TT("gst2", gst[:], r2t[:], Alu.add)
            c2t = tl.tile([128, G], FP, tag="c2t")
            S.activation(c2t[:], gst[:], Act.Copy, bias=M23)
            c3t = tl.tile([128, G], FP, tag="c3t")
            S.activation(c3t[:], c2t[:], Act.Copy, bias=-M23)
            tst = tl.tile([128, G], FP, tag="tst")
            V.tensor_scalar(tst[:], c3t[:], 0.0, 9.0, Alu.max, Alu.min)
            m1 = TT("m1", tst[:], ex[:, :, 2], Alu.mult, Gp)
            tgx = tl.tile([128, G], FP, tag="tgx")
            V.scalar_tensor_tensor(tgx[:], m1[:], 0.1, ex[:, :, 0], Alu.mult, Alu.add)
            m2 = TT("m2", tst[:], ex[:, :, 3], Alu.mult, Gp)
            tgy = tl.tile([128, G], FP, tag="tgy")
            V.scalar_tensor_tensor(tgy[:], m2[:], 0.1, ex[:, :, 1], Alu.mult, Alu.add)

            def smooth_l1_sum(pred_x, pred_y, tx, ty, px_, py_, name):
                acc = None
                for ci, (pr, tt_, pp) in enumerate(((pred_x, tx, px_), (pred_y, ty, py_))):
                    sfx = name + str(ci)
                    e1 = TT(sfx + "e1", tt_, pp, Alu.subtract, Gp)
                    dfe = tl.tile([128, G], FP, tag=sfx + "dfe")
                    V.scalar_tensor_tensor(dfe[:], e1[:], -0.25, pr, Alu.mult, Alu.add)
                    ad = tl.tile([128, G], FP, tag=sfx + "ad")
                    S.activation(ad[:], dfe[:], Act.Abs)
                    m = tl.tile([128, G], FP, tag=sfx + "m")
                    V.tensor_scalar(m[:], ad[:], BETA, None, Alu.min)
                    uu = tl.tile([128, G], FP, tag=sfx + "u")
                    V.scalar_tensor_tensor(uu[:], m[:], -0.5, ad[:], Alu.mult, Alu.add)
                    sl = tl.tile([128, G], FP, tag=sfx + "sl")
                    V.scalar_tensor_tensor(sl[:], m[:], 4.0, uu[:], Alu.mult, Alu.mult)
                    if acc is None:
                        acc = sl
                    else:
                        acc = TT(name + "acc", acc[:], sl[:], Alu.add, Gp)
                return acc

            s1 = smooth_l1_sum(oxP[:], oyP[:], tgx[:], tgy[:], pxP[:], pyP[:], "i1")
            s2 = smooth_l1_sum(ex[:, :, 12], ex[:, :, 13], kxP[:], kyP[:],
                               ex[:, :, 10], ex[:, :, 11], "i2")
            s2 = TT("s2m", s2[:], mkP[:], Alu.mult)

            s1r = tl.tile([128, 1], FP, tag="s1r")
            V.tensor_reduce(s1r[:], s1[:], Ax.X, Alu.add)
            s2r = tl.tile([128, 1], FP, tag="s2r")
            V.tensor_reduce(s2r[:], s2[:], Ax.X, Alu.add)
            sboth = tl.tile([128, 2], FP, tag="sboth")
            V.tensor_copy(sboth[:, 0:1], s1r[:])
            V.tensor_copy(sboth[:, 1:2], s2r[:])
            sc_ps = ps_out.tile([2, 1], FP, tag="sc_ps")
            nc.tensor.matmul(sc_ps[:], sboth[:], onesc[:], start=True, stop=True)
            outsb = tl.tile([2, 1], FP, tag="outsb")
            V.tensor_copy(outsb[:], sc_ps[:])
            nc.sync.dma_start(out_d[:].rearrange("(a b) -> a b", b=1), outsb[:])
            ps_out_cm.__exit__(None, None, None)
            ps_ex_cm.__exit__(None, None, None)
            ps_d2_cm.__exit__(None, None, None)
            ps_grid_cm.__exit__(None, None, None)

    return nc


_CACHE = {}


def _get_program():
    if "nc" not in _CACHE:
        nc = bacc.Bacc("TRN2", target_bir_lowering=False, num_devices=N_CORES)
        pc_d = nc.declare_dram_parameter("pc", [G, P, 2], dt.float32, isOutput=False)
        po_d = nc.declare_dram_parameter("po", [G, P, 2], dt.float32, isOutput=False)
        gc_d = nc.declare_dram_parameter("gc", [G, P, 2], dt.float32, isOutput=False)
        gk_d = nc.declare_dram_parameter("gk", [G, P, 2], dt.float32, isOutput=False)
        mk_d = nc.declare_dram_parameter("mk", [G, P], dt.float32, isOutput=False)
        out_d = nc.declare_dram_parameter("out", [2], dt.float32, isOutput=True)
        _build(nc, pc_d[:], po_d[:], gc_d[:], gk_d[:], mk_d[:], out_d[:])
        nc.compile()
        _CACHE["nc"] = nc
    return _CACHE["nc"]


def _in_maps(inputs):
    pc = np.ascontiguousarray(inputs["pred_contours"], dtype=np.float32)
    po = np.ascontiguousarray(inputs["pred_offsets"], dtype=np.float32)
    gc = np.ascontiguousarray(inputs["gt_contours"], dtype=np.float32)
    gk = np.ascontiguousarray(inputs["gt_key_points"], dtype=np.float32)
    mk = np.ascontiguousarray(inputs["gt_key_points_mask"]).astype(np.float32)
    maps = []
    for c in range(N_CORES):
        s = slice(c * G, (c + 1) * G)
        maps.append({
            "pc": pc[s], "po": po[s], "gc": gc[s], "gk": gk[s], "mk": mk[s],
        })
    return maps


def kernel(pred_contours, pred_offsets, gt_contours, gt_key_points, gt_key_points_mask,
           _results_hook=None):
    inputs = {
        "pred_contours": pred_contours,
        "pred_offsets": pred_offsets,
        "gt_contours": gt_contours,
        "gt_key_points": gt_key_points,
        "gt_key_points_mask": gt_key_points_mask,
    }
    nc = _get_program()
    res = run_bass_kernel_spmd(nc, _in_maps(inputs), list(range(N_CORES)))
    if _results_hook is not None:
        _results_hook(res)
    s1 = f32(0.0)
    s2 = f32(0.0)
    for r in res.results:
        s1 = f32(s1 + f32(r["out"][0]))
        s2 = f32(s2 + f32(r["out"][1]))
    cnt1 = f32(N * P * 2)
    cnt2 = f32(max(float(np.sum(gt_key_points_mask != 0)) * 2.0, 1.0))
    loss = f32(f32(s1 / cnt1) * f32(0.5) + f32(s2 / cnt2) * f32(0.5))
    return np.asarray(loss, dtype=np.float32)


# revision 20
# speedup vs baseline: 3.9606x; 1.0702x over previous
"""Trainium2 Bass kernel for nn_DMLoss (contour matching loss), 8-core data parallel.

v4: block-diagonal bf16 split-precision matmuls with the C1 rounding offset
folded in as extra contraction rows, XBAR DMA-transposed one-hots, combined
packed argmin reduce, engine-balanced chain, piecewise prep DMA overlap.

Per instance (P=128 points, TIME=10):
  item1: nearest of 1280 interpolated gt points per pred point.  Segment n
    spans A_n = gt[n-1]..gt[n]; g' = 10*u - 0.5 with u = <p-A,D>/|D|^2; best
    discrete t = clamp(round(g'), 0, 9); dist^2 = |p-A|^2 + (e/100)*t*(t-2g').
    TensorE produces grids g', opsC1 = SC*(2<p,A> - |A|^2) + C1 and
    erep = -SC*e/100 (bf16 hi/lo split rows, C1 = C1H + C1L exactly).
    dq = (q*erep + opsC1) - CQ lands exactly on the 128-grid; pk = dq - n
    packs quantized distance + segment index; reduce-max = argmin;
    exact 0/1 one-hots gather segment data via XBAR transpose + bf16 matmul.
  item2: same machinery without interpolation (nearest pred per key point).

Output per core: [sum_loss1, sum_loss2]; host divides by counts and combines.
"""
import sys

for _p in ("/opt/trn_rl_repo",):
    if _p not in sys.path:
        sys.path.insert(0, _p)

import numpy as np

import concourse.bass as bass
import concourse.tile as tile
from concourse import bacc, mybir
from concourse.bass_utils import run_bass_kernel_spmd

dt = mybir.dt
Alu = mybir.AluOpType
Ax = mybir.AxisListType
Act = mybir.ActivationFunctionType
f32 = np.float32

N_CORES = 8
N, P = 256, 128
G = N // N_CORES          # instances per core = 32
BG = 4                    # instances per block
NB = G // BG              # 8 blocks
SC = 131072.0             # distance scale (quantum = 128/SC = 2^-10)
SHIFT = 48.0
BETA = 0.25               # smooth-l1 beta = 1/STRIDE
CQ = float(2 ** 30 + 2 ** 25)       # 1107296256
C1 = CQ - SC * SHIFT                # 1101004800
C1H = 1098907648.0                  # bf16-exact hi part of C1
C1L = 2097152.0                     # bf16-exact lo part (C1H + C1L == C1)
M23 = 8388608.0

# SPL slab regions, 10 row-types per operand (k = 10*i + t within a block):
#   lhsT: [x_hi, y_hi, x_lo, y_lo, x_hi2, y_hi2, 1, 1, 1, 1]
#   rhs:  [u_hi, v_hi, u_hi2, v_hi2, u_lo, v_lo, c_hi, c_lo, w8, w9]
# pairing t: xh*uh + yh*vh + xl*uh + yl*vh + xh*ul + yh*vl + ch + cl + w8 + w9
P0 = 0
K0 = 10
RG = 20         # w8 = w9 = 0
RO = 30         # w8, w9 = C1H, C1L
RQ = 40         # w8, w9 = C1H, C1L
RE = 50         # [z z z z z z er_hi er_lo z z]
TB = 60         # ax_hi ay_hi dx_hi dy_hi ox_hi oy_hi
NSLAB = 74
KB = 10 * BG    # block contraction rows = 40


def _build(nc, pc_d, po_d, gc_d, gk_d, mk_d, out_d):
    FP = dt.float32
    BF = dt.bfloat16

    with tile.TileContext(nc) as tc:
        with (
            tc.tile_pool(name="const", bufs=1) as cpool,
            tc.tile_pool(name="prep", bufs=1) as prep,
            tc.tile_pool(name="oper", bufs=1) as oper,
            tc.tile_pool(name="main", bufs=3) as main,
            tc.tile_pool(name="keep", bufs=1) as keep,
        ):
            V, Gp, S = nc.vector, nc.gpsimd, nc.scalar

            # ---------------- constants ----------------
            iota_i = cpool.tile([128, 128], dt.int32)
            Gp.iota(iota_i[:], pattern=[[1, 128]], channel_multiplier=0)
            iotaF = cpool.tile([128, 128], FP, tag="iotaF")
            V.tensor_copy(iotaF[:], iota_i[:])
            iotaC_i = cpool.tile([128, 1], dt.int32)
            Gp.iota(iotaC_i[:], pattern=[[0, 1]], channel_multiplier=1)
            iotaC = cpool.tile([128, 1], FP, tag="iotaC")
            V.tensor_copy(iotaC[:], iotaC_i[:])
            ident32F = cpool.tile([32, 32], FP, tag="ident32F")
            V.tensor_scalar(ident32F[:], iotaF[0:32, 0:32], iotaC[0:32], None, Alu.is_equal)
            onesc = cpool.tile([128, 1], FP, tag="onesc")
            Gp.memset(onesc[:], 1.0)

            # ---------------- contiguous input loads ----------------
            pc_i = prep.tile([32, 128, 2], FP, tag="pc_i")
            po_i = prep.tile([32, 128, 2], FP, tag="po_i")
            gc_i = prep.tile([32, 128, 2], FP, tag="gc_i")
            gk_i = prep.tile([32, 128, 2], FP, tag="gk_i")
            mk_i = prep.tile([32, 128], FP, tag="mk_i")
            a_i = prep.tile([32, 128, 2], FP, tag="a_i")
            nc.sync.dma_start(pc_i[:], pc_d[:, :, :])
            nc.sync.dma_start(po_i[:], po_d[:, :, :])
            nc.sync.dma_start(gc_i[:], gc_d[:, :, :])
            nc.scalar.dma_start(gk_i[:], gk_d[:, :, :])
            nc.scalar.dma_start(mk_i[:], mk_d[:, :])
            nc.sync.dma_start(a_i[:, 1:128, :], gc_d[:, 0:127, :])
            nc.scalar.dma_start(a_i[:, 0:1, :], gc_d[:, 127:128, :])

            # ---------------- segment geometry (g-major layout) ----------------
            d_i = prep.tile([32, 128, 2], FP, tag="d_i")
            V.tensor_tensor(d_i[:], gc_i[:], a_i[:], Alu.subtract)
            dsq = prep.tile([32, 128, 2], FP, tag="dsq")
            Gp.tensor_tensor(dsq[:], d_i[:], d_i[:], Alu.mult)
            e = prep.tile([32, 128], FP, tag="e")
            V.tensor_tensor(e[:], dsq[:, :, 0], dsq[:, :, 1], Alu.add)
            einv = prep.tile([32, 128], FP, tag="einv")
            V.reciprocal(einv[:], e[:])
            t_ad = prep.tile([32, 128, 2], FP, tag="t_ad")
            Gp.tensor_tensor(t_ad[:], a_i[:], d_i[:], Alu.mult)
            a2 = prep.tile([32, 128], FP, tag="a2")
            V.tensor_tensor(a2[:], t_ad[:, :, 0], t_ad[:, :, 1], Alu.add)
            asq = prep.tile([32, 128, 2], FP, tag="asq")
            Gp.tensor_tensor(asq[:], a_i[:], a_i[:], Alu.mult)
            zA = prep.tile([32, 128], FP, tag="zA")
            V.tensor_tensor(zA[:], asq[:, :, 0], asq[:, :, 1], Alu.add)
            psq = prep.tile([32, 128, 2], FP, tag="psq")
            Gp.tensor_tensor(psq[:], pc_i[:], pc_i[:], Alu.mult)
            zP = prep.tile([32, 128], FP, tag="zP")
            V.tensor_tensor(zP[:], psq[:, :, 0], psq[:, :, 1], Alu.add)

            # r rows: g' = g - 0.5 = px*r0 + py*r1 + r2'   (-0.5 folded into r2')
            r_01 = prep.tile([32, 128, 2], FP, tag="r_01")
            V.scalar_tensor_tensor(r_01[:, :, 0], d_i[:, :, 0], 10.0, einv[:], Alu.mult, Alu.mult)
            V.scalar_tensor_tensor(r_01[:, :, 1], d_i[:, :, 1], 10.0, einv[:], Alu.mult, Alu.mult)
            r2 = prep.tile([32, 128], FP, tag="r2")
            V.scalar_tensor_tensor(r2[:], a2[:], -10.0, einv[:], Alu.mult, Alu.mult)
            o_01 = prep.tile([32, 128, 2], FP, tag="o_01")
            S.activation(o_01[:], a_i[:], Act.Copy, scale=2.0 * SC)
            o2 = prep.tile([32, 128], FP, tag="o2")
            S.activation(o2[:], zA[:], Act.Copy, scale=-SC)
            er = prep.tile([32, 128], FP, tag="er")
            S.activation(er[:], e[:], Act.Copy, scale=-SC / 100.0)
            q_01 = prep.tile([32, 128, 2], FP, tag="q_01")
            S.activation(q_01[:], pc_i[:], Act.Copy, scale=2.0 * SC)
            q2 = prep.tile([32, 128], FP, tag="q2")
            S.activation(q2[:], zP[:], Act.Copy, scale=-SC)

            # ---------------- bf16 hi/lo splits into SPL slabs ----------------
            SPL = prep.tile([32, NSLAB, 128], BF, tag="SPL")

            def pair_view(s):
                return SPL[:, s:s + 2, :].rearrange("g s q -> g q s")

            def split_pair(src, s_hi, s_lo):
                V.tensor_copy(pair_view(s_hi), src)
                if s_lo is not None:
                    # mixed-dtype subtract: f32 - bf16 -> bf16
                    V.tensor_tensor(pair_view(s_lo), src, pair_view(s_hi), Alu.subtract)

            def split_one(src, s_hi, s_lo):
                V.tensor_copy(SPL[:, s_hi, :], src)
                if s_lo is not None:
                    V.tensor_tensor(SPL[:, s_lo, :], src, SPL[:, s_hi, :], Alu.subtract)

            split_pair(pc_i[:], P0 + 0, P0 + 2)
            split_pair(gk_i[:], K0 + 0, K0 + 2)
            split_pair(r_01[:], RG + 0, RG + 4)
            split_pair(o_01[:], RO + 0, RO + 4)
            split_pair(q_01[:], RQ + 0, RQ + 4)
            split_one(r2[:], RG + 6, RG + 7)
            split_one(o2[:], RO + 6, RO + 7)
            split_one(q2[:], RQ + 6, RQ + 7)
            split_one(er[:], RE + 6, RE + 7)
            split_pair(a_i[:], TB + 0, None)
            split_pair(d_i[:], TB + 2, None)
            split_pair(po_i[:], TB + 12, None)
            # table copies of r rows and pred coords (consecutive TB region)
            V.tensor_copy(SPL[:, TB + 4:TB + 6, :], SPL[:, RG + 0:RG + 5:4, :])
            V.tensor_copy(SPL[:, TB + 6:TB + 8, :], SPL[:, RG + 1:RG + 6:4, :])
            V.tensor_copy(SPL[:, TB + 8:TB + 10, :], SPL[:, RG + 6:RG + 8, :])
            V.tensor_copy(SPL[:, TB + 10:TB + 12, :], SPL[:, P0 + 0:P0 + 2, :])
            # duplicated hi rows
            V.tensor_copy(SPL[:, P0 + 4:P0 + 6, :], SPL[:, P0 + 0:P0 + 2, :])
            V.tensor_copy(SPL[:, K0 + 4:K0 + 6, :], SPL[:, K0 + 0:K0 + 2, :])
            V.tensor_copy(SPL[:, RG + 2:RG + 4, :], SPL[:, RG + 0:RG + 2, :])
            V.tensor_copy(SPL[:, RO + 2:RO + 4, :], SPL[:, RO + 0:RO + 2, :])
            V.tensor_copy(SPL[:, RQ + 2:RQ + 4, :], SPL[:, RQ + 0:RQ + 2, :])
            # ones / zeros / C1 rows
            Gp.memset(SPL[:, P0 + 6:P0 + 10, :], 1.0)
            Gp.memset(SPL[:, K0 + 6:K0 + 10, :], 1.0)
            Gp.memset(SPL[:, RG + 8:RG + 10, :], 0.0)
            Gp.memset(SPL[:, RO + 8, :], C1H)
            Gp.memset(SPL[:, RO + 9, :], C1L)
            Gp.memset(SPL[:, RQ + 8, :], C1H)
            Gp.memset(SPL[:, RQ + 9, :], C1L)
            Gp.memset(SPL[:, RE + 0:RE + 6, :], 0.0)
            Gp.memset(SPL[:, RE + 8:RE + 10, :], 0.0)

            # ---------------- DRAM bounce: merged stores ---------------------
            slab_d = nc.dram_tensor("slabs", [32, NSLAB, 128], BF)
            nc.sync.dma_start(slab_d[:, 0:20, :], SPL[:, 0:20, :])
            nc.scalar.dma_start(slab_d[:, 20:60, :], SPL[:, 20:60, :])

            # zeros region for rhs backfill
            zero_d = nc.dram_tensor("zeros", [KB, 32, 512], BF)
            ztile = prep.tile([KB, 512], BF, tag="ztile")
            V.memset(ztile[:], 0.0)
            nc.scalar.dma_start(zero_d[:, 0, :], ztile[:])
            nc.scalar.dma_start(zero_d[:, 1, :], zero_d[:, 0, :])
            nc.scalar.dma_start(zero_d[:, 2:4, :], zero_d[:, 0:2, :])
            nc.scalar.dma_start(zero_d[:, 4:8, :], zero_d[:, 0:4, :])
            nc.scalar.dma_start(zero_d[:, 8:16, :], zero_d[:, 0:8, :])
            nc.scalar.dma_start(zero_d[:, 16:32, :], zero_d[:, 0:16, :])

            # lhsA[t, s, b, p] = slab (s=0 pred, s=1 key); rhsA[t, r, b, n]
            lhsA = oper.tile([KB, 2, 8, 128], BF, tag="lhsA")
            rhsA = oper.tile([KB, 4, 8, 512], BF, tag="rhsA")
            nc.sync.dma_start(
                rhsA[:], zero_d[:, :, :].rearrange("t (r b) n -> t r b n", r=4))
            for i in range(BG):
                for s in range(2):
                    eng = nc.sync if ((i + s) % 2 == 0) else nc.scalar
                    eng.dma_start(
                        lhsA[10 * i:10 * i + 10, s, :, :],
                        slab_d[i:32:4, 10 * s:10 * s + 10, :].rearrange("b t p -> t b p"),
                    )
                for r in range(4):
                    eng = nc.sync if ((i + r) % 2 == 0) else nc.scalar
                    eng.dma_start(
                        rhsA[10 * i:10 * i + 10, r, :, 128 * i:128 * (i + 1)],
                        slab_d[i:32:4, 20 + 10 * r:30 + 10 * r, :].rearrange("b t p -> t b p"),
                    )

            # ---------------- gather tables (one batched XBAR transpose) -----
            # in (32, 14*128) -> out stgB[n, j, g] = SPL[g, TB+j, n]
            T12 = keep.tile([128, G, 10], BF, tag="T12")
            T3c = keep.tile([128, G, 4], BF, tag="T3c")
            stgB = keep.tile([128, 14, 32], BF, tag="stgB")
            nc.scalar.dma_start_transpose(
                stgB[:], SPL[:, TB:TB + 14, :].rearrange("g j n -> g (j n)"))
            # T12 slots: [ax, ay, dx, dy, r0h, r0l, r1h, r1l, r2h, r2l] = j 0..9
            V.tensor_copy(T12[:], stgB[:, 0:10, :].rearrange("n j g -> n g j"))
            # T3c slots: [px, py, ox, oy] = j 10..13
            V.tensor_copy(T3c[:], stgB[:, 10:14, :].rearrange("n j g -> n g j"))

            # ---------------- f32 transposes for the tail --------------------
            pxP = keep.tile([128, G], FP, tag="pxP")
            pyP = keep.tile([128, G], FP, tag="pyP")
            oxP = keep.tile([128, G], FP, tag="oxP")
            oyP = keep.tile([128, G], FP, tag="oyP")
            kxP = keep.tile([128, G], FP, tag="kxP")
            kyP = keep.tile([128, G], FP, tag="kyP")
            mkP = keep.tile([128, G], FP, tag="mkP")
            with tc.tile_pool(name="ps_prep", bufs=3, space="PSUM") as ps_prep:
                for dst, src in ((pxP, pc_i[:, :, 0]), (pyP, pc_i[:, :, 1]),
                                 (oxP, po_i[:, :, 0]), (oyP, po_i[:, :, 1]),
                                 (kxP, gk_i[:, :, 0]), (kyP, gk_i[:, :, 1]),
                                 (mkP, mk_i[:])):
                    fps = ps_prep.tile([128, 32], FP, tag="tpsF")
                    nc.tensor.transpose(fps[:], src, ident32F[:])
                    S.activation(dst[:], fps[:], Act.Copy)

            ex = keep.tile([128, G, 14], FP, tag="ex")
            iotaB = iotaF[:].rearrange("p (o q) -> p o q", o=1).broadcast_to([128, BG, 128])

            ps_grid_cm = tc.tile_pool(name="ps_grid", bufs=1, space="PSUM")
            ps_d2_cm = tc.tile_pool(name="ps_d2", bufs=2, space="PSUM")
            ps_ex_cm = tc.tile_pool(name="ps_ex", bufs=2, space="PSUM")
            ps_out_cm = tc.tile_pool(name="ps_out", bufs=1, space="PSUM")
            ps_grid = ps_grid_cm.__enter__()
            ps_d2 = ps_d2_cm.__enter__()
            ps_ex = ps_ex_cm.__enter__()
            ps_out = ps_out_cm.__enter__()

            # ---------------- main loop ----------------
            for b in range(NB):
                g0 = b * BG
                gps = ps_grid.tile([128, BG, 128], FP, tag="gps")
                ops = ps_grid.tile([128, BG, 128], FP, tag="ops")
                erep = ps_grid.tile([128, BG, 128], FP, tag="erep")
                d2ps = ps_d2.tile([128, BG, 128], FP, tag="d2ps")
                gv = gps[:].rearrange("p i n -> p (i n)")
                ov = ops[:].rearrange("p i n -> p (i n)")
                ev = erep[:].rearrange("p i n -> p (i n)")
                dv = d2ps[:].rearrange("p i n -> p (i n)")
                nc.tensor.matmul(gv, lhsA[:, 0, b, :], rhsA[:, 0, b, :], start=True, stop=True)
                nc.tensor.matmul(ov, lhsA[:, 0, b, :], rhsA[:, 1, b, :], start=True, stop=True)
                nc.tensor.matmul(ev, lhsA[:, 0, b, :], rhsA[:, 3, b, :], start=True, stop=True)
                nc.tensor.matmul(dv, lhsA[:, 1, b, :], rhsA[:, 2, b, :], start=True, stop=True)

                # -------- item1: t = clamp(round(g'), 0, 9) --------
                s2t = main.tile([128, BG, 128], FP, tag="s2t")
                S.activation(s2t[:], gps[:], Act.Copy, bias=M23)
                s3t = main.tile([128, BG, 128], FP, tag="s3t")
                S.activation(s3t[:], s2t[:], Act.Copy, bias=-M23)
                t = main.tile([128, BG, 128], FP, tag="t")
                V.tensor_scalar(t[:], s3t[:], 0.0, 9.0, Alu.max, Alu.min)
                hq = main.tile([128, BG, 128], FP, tag="hq")
                V.scalar_tensor_tensor(hq[:], gps[:], -2.0, t[:], Alu.mult, Alu.add)
                q = main.tile([128, BG, 128], FP, tag="q")
                Gp.tensor_tensor(q[:], hq[:], t[:], Alu.mult)
                vE = main.tile([128, BG, 128], FP, tag="vE")
                V.tensor_tensor(vE[:], q[:], erep[:], Alu.mult)
                dqA = main.tile([128, BG, 128], FP, tag="dqA")
                V.tensor_tensor(dqA[:], vE[:], ops[:], Alu.add)
                dq = main.tile([128, BG, 128], FP, tag="dq")
                S.activation(dq[:], dqA[:], Act.Copy, bias=-CQ)
                dq2 = main.tile([128, BG, 128], FP, tag="dq2")
                S.activation(dq2[:], d2ps[:], Act.Copy, bias=-CQ)

                pkN = main.tile([128, BG, 128], FP, tag="pkN")
                Gp.tensor_tensor(pkN[:], dq[:], iotaB, Alu.subtract)
                pk2 = main.tile([128, BG, 128], FP, tag="pk2")
                Gp.tensor_tensor(pk2[:], dq2[:], iotaB, Alu.subtract)
                mx = main.tile([128, BG], FP, tag="mx")
                V.tensor_reduce(mx[:], pkN[:], Ax.X, Alu.max)
                mx2 = main.tile([128, BG], FP, tag="mx2")
                V.tensor_reduce(mx2[:], pk2[:], Ax.X, Alu.max)
                mxb2 = main.tile([128, BG], FP, tag="mxb2")
                V.tensor_scalar(mxb2[:], mx2[:], -1.0, 1.0, Alu.mult, Alu.add)

                oh = main.tile([128, BG, 128], BF, tag="oh")
                oh2 = main.tile([128, BG, 128], BF, tag="oh2")
                for i in range(BG):
                    V.tensor_scalar(oh[:, i, :], pkN[:, i, :], mx[:, i:i + 1], None, Alu.is_equal)
                    S.activation(oh2[:, i, :], pk2[:, i, :], Act.Relu, bias=mxb2[:, i:i + 1])

                # -------- XBAR-transpose one-hots, gather via matmul --------
                ohT = main.tile([128, BG, 128], BF, tag="ohT")
                oh2T = main.tile([128, BG, 128], BF, tag="oh2T")
                nc.sync.dma_start_transpose(ohT[:], oh[:].rearrange("m i n -> m (i n)"))
                nc.sync.dma_start_transpose(oh2T[:], oh2[:].rearrange("m i n -> m (i n)"))

                exPS = ps_ex.tile([128, BG, 14], FP, tag="exPS")
                for i in range(BG):
                    g = g0 + i
                    nc.tensor.matmul(exPS[:, i, 0:10], ohT[:, i, :], T12[:, g, :], start=True, stop=True)
                    nc.tensor.matmul(exPS[:, i, 10:14], oh2T[:, i, :], T3c[:, g, :], start=True, stop=True)
                V.tensor_copy(ex[:, g0:g0 + BG, :], exPS[:])

            # ---------------- tail (128 x G f32 ops) ----------------
            tl = prep

            def TT(name, a, bb, op, eng=V):
                r = tl.tile([128, G], FP, tag=name)
                eng.tensor_tensor(r[:], a, bb, op)
                return r

            r0 = TT("r0", ex[:, :, 4], ex[:, :, 5], Alu.add)
            r1 = TT("r1", ex[:, :, 6], ex[:, :, 7], Alu.add, Gp)
            r2t = TT("r2t", ex[:, :, 8], ex[:, :, 9], Alu.add)
            v1 = TT("v1", pxP[:], r0[:], Alu.mult, Gp)
            v2 = TT("v2", pyP[:], r1[:], Alu.mult)
            gst = TT("gst", v1[:], v2[:], Alu.add, Gp)
            gst = TT("gst2", gst[:], r2t[:], Alu.add)
            c2t = tl.tile([128, G], FP, tag="c2t")
            S.activation(c2t[:], gst[:], Act.Copy, bias=M23)
            c3t = tl.tile([128, G], FP, tag="c3t")
            S.activation(c3t[:], c2t[:], Act.Copy, bias=-M23)
            tst = tl.tile([128, G], FP, tag="tst")
            V.tensor_scalar(tst[:], c3t[:], 0.0, 9.0, Alu.max, Alu.min)
            m1 = TT("m1", tst[:], ex[:, :, 2], Alu.mult, Gp)
            tgx = tl.tile([128, G], FP, tag="tgx")
            V.scalar_tensor_tensor(tgx[:], m1[:], 0.1, ex[:, :, 0], Alu.mult, Alu.add)
            m2 = TT("m2", tst[:], ex[:, :, 3], Alu.mult, Gp)
            tgy = tl.tile([128, G], FP, tag="tgy")
            V.scalar_tensor_tensor(tgy[:], m2[:], 0.1, ex[:, :, 1], Alu.mult, Alu.add)

            def smooth_l1_sum(pred_x, pred_y, tx, ty, px_, py_, name):
                acc = None
                for ci, (pr, tt_, pp) in enumerate(((pred_x, tx, px_), (pred_y, ty, py_))):
                    sfx = name + str(ci)
                    e1 = TT(sfx + "e1", tt_, pp, Alu.subtract, Gp)
                    dfe = tl.tile([128, G], FP, tag=sfx + "dfe")
                    V.scalar_tensor_tensor(dfe[:], e1[:], -0.25, pr, Alu.mult, Alu.add)
                    ad = tl.tile([128, G], FP, tag=sfx + "ad")
                    S.activation(ad[:], dfe[:], Act.Abs)
                    m = tl.tile([128, G], FP, tag=sfx + "m")
                    V.tensor_scalar(m[:], ad[:], BETA, None, Alu.min)
                    uu = tl.tile([128, G], FP, tag=sfx + "u")
                    V.scalar_tensor_tensor(uu[:], m[:], -0.5, ad[:], Alu.mult, Alu.add)
                    sl = tl.tile([128, G], FP, tag=sfx + "sl")
                    V.scalar_tensor_tensor(sl[:], m[:], 4.0, uu[:], Alu.mult, Alu.mult)
                    if acc is None:
                        acc = sl
                    else:
                        acc = TT(name + "acc", acc[:], sl[:], Alu.add, Gp)
                return acc

            s1 = smooth_l1_sum(oxP[:], oyP[:], tgx[:], tgy[:], pxP[:], pyP[:], "i1")
            s2 = smooth_l1_sum(ex[:, :, 12], ex[:, :, 13], kxP[:], kyP[:],
                               ex[:, :, 10], ex[:, :, 11], "i2")
            s2 = TT("s2m", s2[:], mkP[:], Alu.mult)

            s1r = tl.tile([128, 1], FP, tag="s1r")
            V.tensor_reduce(s1r[:], s1[:], Ax.X, Alu.add)
            s2r = tl.tile([128, 1], FP, tag="s2r")
            V.tensor_reduce(s2r[:], s2[:], Ax.X, Alu.add)
            sboth = tl.tile([128, 2], FP, tag="sboth")
            V.tensor_copy(sboth[:, 0:1], s1r[:])
            V.tensor_copy(sboth[:, 1:2], s2r[:])
            sc_ps = ps_out.tile([2, 1], FP, tag="sc_ps")
            nc.tensor.matmul(sc_ps[:], sboth[:], onesc[:], start=True, stop=True)
            outsb = tl.tile([2, 1], FP, tag="outsb")
            V.tensor_copy(outsb[:], sc_ps[:])
            nc.sync.dma_start(out_d[:].rearrange("(a b) -> a b", b=1), outsb[:])
            ps_out_cm.__exit__(None, None, None)
            ps_ex_cm.__exit__(None, None, None)
            ps_d2_cm.__exit__(None, None, None)
            ps_grid_cm.__exit__(None, None, None)

    return nc


_CACHE = {}


def _get_program():
    if "nc" not in _CACHE:
        nc = bacc.Bacc("TRN2", target_bir_lowering=False, num_devices=N_CORES)
        pc_d = nc.declare_dram_parameter("pc", [G, P, 2], dt.float32, isOutput=False)
        po_d = nc.declare_dram_parameter("po", [G, P, 2], dt.float32, isOutput=False)
        gc_d = nc.declare_dram_parameter("gc", [G, P, 2], dt.float32, isOutput=False)
        gk_d = nc.declare_dram_parameter("gk", [G, P, 2], dt.float32, isOutput=False)
        mk_d = nc.declare_dram_parameter("mk", [G, P], dt.float32, isOutput=False)
        out_d = nc.declare_dram_parameter("out", [2], dt.float32, isOutput=True)
        _build(nc, pc_d[:], po_d[:], gc_d[:], gk_d[:], mk_d[:], out_d[:])
        nc.compile()
        _CACHE["nc"] = nc
    return _CACHE["nc"]


def _in_maps(inputs):
    pc = np.ascontiguousarray(inputs["pred_contours"], dtype=np.float32)
    po = np.ascontiguousarray(inputs["pred_offsets"], dtype=np.float32)
    gc = np.ascontiguousarray(inputs["gt_contours"], dtype=np.float32)
    gk = np.ascontiguousarray(inputs["gt_key_points"], dtype=np.float32)
    mk = np.ascontiguousarray(inputs["gt_key_points_mask"]).astype(np.float32)
    maps = []
    for c in range(N_CORES):
        s = slice(c * G, (c + 1) * G)
        maps.append({
            "pc": pc[s], "po": po[s], "gc": gc[s], "gk": gk[s], "mk": mk[s],
        })
    return maps


def kernel(pred_contours, pred_offsets, gt_contours, gt_key_points, gt_key_points_mask,
           _results_hook=None):
    inputs = {
        "pred_contours": pred_contours,
        "pred_offsets": pred_offsets,
        "gt_contours": gt_contours,
        "gt_key_points": gt_key_points,
        "gt_key_points_mask": gt_key_points_mask,
    }
    nc = _get_program()
    res = run_bass_kernel_spmd(nc, _in_maps(inputs), list(range(N_CORES)))
    if _results_hook is not None:
        _results_hook(res)
    s1 = f32(0.0)
    s2 = f32(0.0)
    for r in res.results:
        s1 = f32(s1 + f32(r["out"][0]))
        s2 = f32(s2 + f32(r["out"][1]))
    cnt1 = f32(N * P * 2)
    cnt2 = f32(max(float(np.sum(gt_key_points_mask != 0)) * 2.0, 1.0))
    loss = f32(f32(s1 / cnt1) * f32(0.5) + f32(s2 / cnt2) * f32(0.5))
    return np.asarray(loss, dtype=np.float32)


# revision 22
# speedup vs baseline: 4.2465x; 1.0722x over previous
"""Trainium2 Bass kernel for nn_DMLoss (contour matching loss), 8-core data parallel.

v4: block-diagonal bf16 split-precision matmuls with the C1 rounding offset
folded in as extra contraction rows, XBAR DMA-transposed one-hots, combined
packed argmin reduce, engine-balanced chain, piecewise prep DMA overlap.

Per instance (P=128 points, TIME=10):
  item1: nearest of 1280 interpolated gt points per pred point.  Segment n
    spans A_n = gt[n-1]..gt[n]; g' = 10*u - 0.5 with u = <p-A,D>/|D|^2; best
    discrete t = clamp(round(g'), 0, 9); dist^2 = |p-A|^2 + (e/100)*t*(t-2g').
    TensorE produces grids g', opsC1 = SC*(2<p,A> - |A|^2) + C1 and
    erep = -SC*e/100 (bf16 hi/lo split rows, C1 = C1H + C1L exactly).
    dq = (q*erep + opsC1) - CQ lands exactly on the 128-grid; pk = dq - n
    packs quantized distance + segment index; reduce-max = argmin;
    exact 0/1 one-hots gather segment data via XBAR transpose + bf16 matmul.
  item2: same machinery without interpolation (nearest pred per key point).

Output per core: [sum_loss1, sum_loss2]; host divides by counts and combines.
"""
import sys

for _p in ("/opt/trn_rl_repo",):
    if _p not in sys.path:
        sys.path.insert(0, _p)

import numpy as np

import concourse.bass as bass
import concourse.tile as tile
from concourse import bacc, mybir
from concourse.bass_utils import run_bass_kernel_spmd

dt = mybir.dt
Alu = mybir.AluOpType
Ax = mybir.AxisListType
Act = mybir.ActivationFunctionType
f32 = np.float32

N_CORES = 8
N, P = 256, 128
G = N // N_CORES          # instances per core = 32
BG = 4                    # instances per block
NB = G // BG              # 8 blocks
SC = 131072.0             # distance scale (quantum = 128/SC = 2^-10)
SHIFT = 48.0
BETA = 0.25               # smooth-l1 beta = 1/STRIDE
CQ = float(2 ** 30 + 2 ** 25)       # 1107296256
C1 = CQ - SC * SHIFT                # 1101004800
C1H = 1098907648.0                  # bf16-exact hi part of C1
C1L = 2097152.0                     # bf16-exact lo part (C1H + C1L == C1)
M23 = 8388608.0

# SPL slab regions, 10 row-types per operand (k = 10*i + t within a block):
#   lhsT: [x_hi, y_hi, x_lo, y_lo, x_hi2, y_hi2, 1, 1, 1, 1]
#   rhs:  [u_hi, v_hi, u_hi2, v_hi2, u_lo, v_lo, c_hi, c_lo, w8, w9]
# pairing t: xh*uh + yh*vh + xl*uh + yl*vh + xh*ul + yh*vl + ch + cl + w8 + w9
P0 = 0
K0 = 10
RG = 20         # w8 = w9 = 0
RO = 30         # w8, w9 = C1H, C1L
RQ = 40         # w8, w9 = C1H, C1L
RE = 50         # [z z z z z z er_hi er_lo z z]
TB = 60         # ax_hi ay_hi dx_hi dy_hi ox_hi oy_hi
NSLAB = 74
KB = 10 * BG    # block contraction rows = 40


def _build(nc, pc_d, po_d, gc_d, gk_d, mk_d, out_d):
    FP = dt.float32
    BF = dt.bfloat16

    with tile.TileContext(nc) as tc:
        with (
            tc.tile_pool(name="const", bufs=1) as cpool,
            tc.tile_pool(name="prep", bufs=1) as prep,
            tc.tile_pool(name="oper", bufs=1) as oper,
            tc.tile_pool(name="main", bufs=3) as main,
            tc.tile_pool(name="keep", bufs=1) as keep,
        ):
            V, Gp, S = nc.vector, nc.gpsimd, nc.scalar

            # ---------------- constants ----------------
            iota_i = cpool.tile([128, 128], dt.int32)
            Gp.iota(iota_i[:], pattern=[[1, 128]], channel_multiplier=0)
            iotaF = cpool.tile([128, 128], FP, tag="iotaF")
            V.tensor_copy(iotaF[:], iota_i[:])
            iotaC_i = cpool.tile([128, 1], dt.int32)
            Gp.iota(iotaC_i[:], pattern=[[0, 1]], channel_multiplier=1)
            iotaC = cpool.tile([128, 1], FP, tag="iotaC")
            V.tensor_copy(iotaC[:], iotaC_i[:])
            ident32F = cpool.tile([32, 32], FP, tag="ident32F")
            V.tensor_scalar(ident32F[:], iotaF[0:32, 0:32], iotaC[0:32], None, Alu.is_equal)
            onesc = cpool.tile([128, 1], FP, tag="onesc")
            Gp.memset(onesc[:], 1.0)

            # zeros region for rhs backfill
            zero_d = nc.dram_tensor("zeros", [KB, 32, 512], BF)
            ztile = prep.tile([KB, 512], BF, tag="ztile")
            V.memset(ztile[:], 0.0)
            nc.scalar.dma_start(zero_d[:, 0, :], ztile[:])
            nc.scalar.dma_start(zero_d[:, 1, :], zero_d[:, 0, :])
            nc.scalar.dma_start(zero_d[:, 2:4, :], zero_d[:, 0:2, :])
            nc.scalar.dma_start(zero_d[:, 4:8, :], zero_d[:, 0:4, :])
            nc.scalar.dma_start(zero_d[:, 8:16, :], zero_d[:, 0:8, :])
            nc.scalar.dma_start(zero_d[:, 16:32, :], zero_d[:, 0:16, :])

            # ---------------- contiguous input loads ----------------
            pc_i = prep.tile([32, 128, 2], FP, tag="pc_i")
            po_i = prep.tile([32, 128, 2], FP, tag="po_i")
            gc_i = prep.tile([32, 128, 2], FP, tag="gc_i")
            gk_i = prep.tile([32, 128, 2], FP, tag="gk_i")
            mk_i = prep.tile([32, 128], FP, tag="mk_i")
            a_i = prep.tile([32, 128, 2], FP, tag="a_i")
            nc.sync.dma_start(pc_i[:], pc_d[:, :, :])
            nc.sync.dma_start(po_i[:], po_d[:, :, :])
            nc.sync.dma_start(gc_i[:], gc_d[:, :, :])
            nc.scalar.dma_start(gk_i[:], gk_d[:, :, :])
            nc.scalar.dma_start(mk_i[:], mk_d[:, :])
            nc.sync.dma_start(a_i[:, 1:128, :], gc_d[:, 0:127, :])
            nc.scalar.dma_start(a_i[:, 0:1, :], gc_d[:, 127:128, :])

            # ---------------- SPL slabs + fine-grained stores ----------------
            SPL = prep.tile([32, NSLAB, 128], BF, tag="SPL")
            slab_d = nc.dram_tensor("slabs", [32, NSLAB, 128], BF)

            def pair_view(s):
                return SPL[:, s:s + 2, :].rearrange("g s q -> g q s")

            def split_pair(src_, s_hi, s_lo):
                S.activation(pair_view(s_hi), src_, Act.Copy)
                if s_lo is not None:
                    V.tensor_tensor(pair_view(s_lo), src_, pair_view(s_hi), Alu.subtract)

            def split_one(src_, s_hi, s_lo):
                S.activation(SPL[:, s_hi, :], src_, Act.Copy)
                if s_lo is not None:
                    V.tensor_tensor(SPL[:, s_lo, :], src_, SPL[:, s_hi, :], Alu.subtract)

            # ---- phase A: pc/gk-dependent slabs (no gc geometry needed) ----
            Gp.memset(SPL[:, P0 + 6:P0 + 10, :], 1.0)
            Gp.memset(SPL[:, K0 + 6:K0 + 10, :], 1.0)
            Gp.memset(SPL[:, RQ + 8, :], C1H)
            Gp.memset(SPL[:, RQ + 9, :], C1L)
            split_pair(pc_i[:], P0 + 0, P0 + 2)
            split_pair(gk_i[:], K0 + 0, K0 + 2)
            V.tensor_copy(SPL[:, P0 + 4:P0 + 6, :], SPL[:, P0 + 0:P0 + 2, :])
            V.tensor_copy(SPL[:, K0 + 4:K0 + 6, :], SPL[:, K0 + 0:K0 + 2, :])
            q_01 = prep.tile([32, 128, 2], FP, tag="q_01")
            S.activation(q_01[:], pc_i[:], Act.Copy, scale=2.0 * SC)
            psq = prep.tile([32, 128, 2], FP, tag="psq")
            Gp.tensor_tensor(psq[:], pc_i[:], pc_i[:], Alu.mult)
            zP = prep.tile([32, 128], FP, tag="zP")
            Gp.tensor_tensor(zP[:], psq[:, :, 0], psq[:, :, 1], Alu.add)
            q2 = prep.tile([32, 128], FP, tag="q2")
            S.activation(q2[:], zP[:], Act.Copy, scale=-SC)
            split_pair(q_01[:], RQ + 0, RQ + 4)
            split_one(q2[:], RQ + 6, RQ + 7)
            V.tensor_copy(SPL[:, RQ + 2:RQ + 4, :], SPL[:, RQ + 0:RQ + 2, :])
            nc.sync.dma_start(slab_d[:, 0:20, :], SPL[:, 0:20, :])
            nc.scalar.dma_start(slab_d[:, RQ:RQ + 10, :], SPL[:, RQ:RQ + 10, :])

            # ---- phase B: gc geometry ----
            d_i = prep.tile([32, 128, 2], FP, tag="d_i")
            V.tensor_tensor(d_i[:], gc_i[:], a_i[:], Alu.subtract)
            dsq = prep.tile([32, 128, 2], FP, tag="dsq")
            Gp.tensor_tensor(dsq[:], d_i[:], d_i[:], Alu.mult)
            e = prep.tile([32, 128], FP, tag="e")
            Gp.tensor_tensor(e[:], dsq[:, :, 0], dsq[:, :, 1], Alu.add)
            einv = prep.tile([32, 128], FP, tag="einv")
            V.reciprocal(einv[:], e[:])
            t_ad = prep.tile([32, 128, 2], FP, tag="t_ad")
            Gp.tensor_tensor(t_ad[:], a_i[:], d_i[:], Alu.mult)
            a2 = prep.tile([32, 128], FP, tag="a2")
            Gp.tensor_tensor(a2[:], t_ad[:, :, 0], t_ad[:, :, 1], Alu.add)
            asq = prep.tile([32, 128, 2], FP, tag="asq")
            Gp.tensor_tensor(asq[:], a_i[:], a_i[:], Alu.mult)
            zA = prep.tile([32, 128], FP, tag="zA")
            Gp.tensor_tensor(zA[:], asq[:, :, 0], asq[:, :, 1], Alu.add)

            er = prep.tile([32, 128], FP, tag="er")
            S.activation(er[:], e[:], Act.Copy, scale=-SC / 100.0)
            Gp.memset(SPL[:, RE + 0:RE + 6, :], 0.0)
            Gp.memset(SPL[:, RE + 8:RE + 10, :], 0.0)
            split_one(er[:], RE + 6, RE + 7)
            nc.sync.dma_start(slab_d[:, RE:RE + 10, :], SPL[:, RE:RE + 10, :])

            r_01 = prep.tile([32, 128, 2], FP, tag="r_01")
            V.scalar_tensor_tensor(r_01[:, :, 0], d_i[:, :, 0], 10.0, einv[:], Alu.mult, Alu.mult)
            V.scalar_tensor_tensor(r_01[:, :, 1], d_i[:, :, 1], 10.0, einv[:], Alu.mult, Alu.mult)
            r2 = prep.tile([32, 128], FP, tag="r2")
            V.scalar_tensor_tensor(r2[:], a2[:], -10.0, einv[:], Alu.mult, Alu.mult)
            Gp.memset(SPL[:, RG + 8:RG + 10, :], 0.0)
            split_pair(r_01[:], RG + 0, RG + 4)
            split_one(r2[:], RG + 6, RG + 7)
            V.tensor_copy(SPL[:, RG + 2:RG + 4, :], SPL[:, RG + 0:RG + 2, :])
            nc.scalar.dma_start(slab_d[:, RG:RG + 10, :], SPL[:, RG:RG + 10, :])

            o_01 = prep.tile([32, 128, 2], FP, tag="o_01")
            S.activation(o_01[:], a_i[:], Act.Copy, scale=2.0 * SC)
            o2 = prep.tile([32, 128], FP, tag="o2")
            S.activation(o2[:], zA[:], Act.Copy, scale=-SC)
            Gp.memset(SPL[:, RO + 8, :], C1H)
            Gp.memset(SPL[:, RO + 9, :], C1L)
            split_pair(o_01[:], RO + 0, RO + 4)
            split_one(o2[:], RO + 6, RO + 7)
            V.tensor_copy(SPL[:, RO + 2:RO + 4, :], SPL[:, RO + 0:RO + 2, :])
            nc.sync.dma_start(slab_d[:, RO:RO + 10, :], SPL[:, RO:RO + 10, :])

            # ---- table region ----
            split_pair(a_i[:], TB + 0, None)
            split_pair(d_i[:], TB + 2, None)
            split_pair(po_i[:], TB + 12, None)
            V.tensor_copy(SPL[:, TB + 4:TB + 6, :], SPL[:, RG + 0:RG + 5:4, :])
            V.tensor_copy(SPL[:, TB + 6:TB + 8, :], SPL[:, RG + 1:RG + 6:4, :])
            V.tensor_copy(SPL[:, TB + 8:TB + 10, :], SPL[:, RG + 6:RG + 8, :])
            V.tensor_copy(SPL[:, TB + 10:TB + 12, :], SPL[:, P0 + 0:P0 + 2, :])


            # lhsA[t, s, b, p] = slab (s=0 pred, s=1 key); rhsA[t, r, b, n]
            lhsA = oper.tile([KB, 2, 8, 128], BF, tag="lhsA")
            rhsA = oper.tile([KB, 4, 8, 512], BF, tag="rhsA")
            nc.sync.dma_start(
                rhsA[:], zero_d[:, :, :].rearrange("t (r b) n -> t r b n", r=4))

            for i in range(BG):
                for s in range(2):
                    eng = nc.sync if ((i + s) % 2 == 0) else nc.scalar
                    eng.dma_start(
                        lhsA[10 * i:10 * i + 10, s, :, :],
                        slab_d[i:32:4, 10 * s:10 * s + 10, :].rearrange("b t p -> t b p"),
                    )
                for r in range(4):
                    eng = nc.sync if ((i + r) % 2 == 0) else nc.scalar
                    eng.dma_start(
                        rhsA[10 * i:10 * i + 10, r, :, 128 * i:128 * (i + 1)],
                        slab_d[i:32:4, 20 + 10 * r:30 + 10 * r, :].rearrange("b t p -> t b p"),
                    )

            # ---------------- gather tables (one batched XBAR transpose) -----
            # in (32, 14*128) -> out stgB[n, j, g] = SPL[g, TB+j, n]
            T12 = keep.tile([128, G, 10], BF, tag="T12")
            T3c = keep.tile([128, G, 4], BF, tag="T3c")
            stgB = keep.tile([128, 14, 32], BF, tag="stgB")
            nc.scalar.dma_start_transpose(
                stgB[:], SPL[:, TB:TB + 14, :].rearrange("g j n -> g (j n)"))
            # T12 slots: [ax, ay, dx, dy, r0h, r0l, r1h, r1l, r2h, r2l] = j 0..9
            V.tensor_copy(T12[:], stgB[:, 0:10, :].rearrange("n j g -> n g j"))
            # T3c slots: [px, py, ox, oy] = j 10..13
            V.tensor_copy(T3c[:], stgB[:, 10:14, :].rearrange("n j g -> n g j"))

            # ---------------- f32 transposes for the tail --------------------
            pxP = keep.tile([128, G], FP, tag="pxP")
            pyP = keep.tile([128, G], FP, tag="pyP")
            oxP = keep.tile([128, G], FP, tag="oxP")
            oyP = keep.tile([128, G], FP, tag="oyP")
            kxP = keep.tile([128, G], FP, tag="kxP")
            kyP = keep.tile([128, G], FP, tag="kyP")
            mkP = keep.tile([128, G], FP, tag="mkP")
            with tc.tile_pool(name="ps_prep", bufs=3, space="PSUM") as ps_prep:
                for dst, src in ((pxP, pc_i[:, :, 0]), (pyP, pc_i[:, :, 1]),
                                 (oxP, po_i[:, :, 0]), (oyP, po_i[:, :, 1]),
                                 (kxP, gk_i[:, :, 0]), (kyP, gk_i[:, :, 1]),
                                 (mkP, mk_i[:])):
                    fps = ps_prep.tile([128, 32], FP, tag="tpsF")
                    nc.tensor.transpose(fps[:], src, ident32F[:])
                    S.activation(dst[:], fps[:], Act.Copy)

            ex = keep.tile([128, G, 14], FP, tag="ex")
            iotaB = iotaF[:].rearrange("p (o q) -> p o q", o=1).broadcast_to([128, BG, 128])

            ps_grid_cm = tc.tile_pool(name="ps_grid", bufs=1, space="PSUM")
            ps_d2_cm = tc.tile_pool(name="ps_d2", bufs=2, space="PSUM")
            ps_ex_cm = tc.tile_pool(name="ps_ex", bufs=2, space="PSUM")
            ps_out_cm = tc.tile_pool(name="ps_out", bufs=1, space="PSUM")
            ps_grid = ps_grid_cm.__enter__()
            ps_d2 = ps_d2_cm.__enter__()
            ps_ex = ps_ex_cm.__enter__()
            ps_out = ps_out_cm.__enter__()

            # ---------------- main loop ----------------
            for b in range(NB):
                g0 = b * BG
                gps = ps_grid.tile([128, BG, 128], FP, tag="gps")
                ops = ps_grid.tile([128, BG, 128], FP, tag="ops")
                erep = ps_grid.tile([128, BG, 128], FP, tag="erep")
                d2ps = ps_d2.tile([128, BG, 128], FP, tag="d2ps")
                gv = gps[:].rearrange("p i n -> p (i n)")
                ov = ops[:].rearrange("p i n -> p (i n)")
                ev = erep[:].rearrange("p i n -> p (i n)")
                dv = d2ps[:].rearrange("p i n -> p (i n)")
                nc.tensor.matmul(gv, lhsA[:, 0, b, :], rhsA[:, 0, b, :], start=True, stop=True)
                nc.tensor.matmul(ov, lhsA[:, 0, b, :], rhsA[:, 1, b, :], start=True, stop=True)
                nc.tensor.matmul(ev, lhsA[:, 0, b, :], rhsA[:, 3, b, :], start=True, stop=True)
                nc.tensor.matmul(dv, lhsA[:, 1, b, :], rhsA[:, 2, b, :], start=True, stop=True)

                # -------- item1: t = clamp(round(g'), 0, 9) --------
                s2t = main.tile([128, BG, 128], FP, tag="s2t")
                S.activation(s2t[:], gps[:], Act.Copy, bias=M23)
                s3t = main.tile([128, BG, 128], FP, tag="s3t")
                S.activation(s3t[:], s2t[:], Act.Copy, bias=-M23)
                t = main.tile([128, BG, 128], FP, tag="t")
                V.tensor_scalar(t[:], s3t[:], 0.0, 9.0, Alu.max, Alu.min)
                hq = main.tile([128, BG, 128], FP, tag="hq")
                V.scalar_tensor_tensor(hq[:], gps[:], -2.0, t[:], Alu.mult, Alu.add)
                q = main.tile([128, BG, 128], FP, tag="q")
                Gp.tensor_tensor(q[:], hq[:], t[:], Alu.mult)
                vE = main.tile([128, BG, 128], FP, tag="vE")
                V.tensor_tensor(vE[:], q[:], erep[:], Alu.mult)
                dqA = main.tile([128, BG, 128], FP, tag="dqA")
                V.tensor_tensor(dqA[:], vE[:], ops[:], Alu.add)
                dq = main.tile([128, BG, 128], FP, tag="dq")
                S.activation(dq[:], dqA[:], Act.Copy, bias=-CQ)
                dq2 = main.tile([128, BG, 128], FP, tag="dq2")
                S.activation(dq2[:], d2ps[:], Act.Copy, bias=-CQ)

                pkN = main.tile([128, BG, 128], FP, tag="pkN")
                Gp.tensor_tensor(pkN[:], dq[:], iotaB, Alu.subtract)
                pk2 = main.tile([128, BG, 128], FP, tag="pk2")
                Gp.tensor_tensor(pk2[:], dq2[:], iotaB, Alu.subtract)
                mx = main.tile([128, BG], FP, tag="mx")
                V.tensor_reduce(mx[:], pkN[:], Ax.X, Alu.max)
                mx2 = main.tile([128, BG], FP, tag="mx2")
                V.tensor_reduce(mx2[:], pk2[:], Ax.X, Alu.max)
                mxb2 = main.tile([128, BG], FP, tag="mxb2")
                V.tensor_scalar(mxb2[:], mx2[:], -1.0, 1.0, Alu.mult, Alu.add)

                oh = main.tile([128, BG, 128], BF, tag="oh")
                oh2 = main.tile([128, BG, 128], BF, tag="oh2")
                for i in range(BG):
                    V.tensor_scalar(oh[:, i, :], pkN[:, i, :], mx[:, i:i + 1], None, Alu.is_equal)
                    S.activation(oh2[:, i, :], pk2[:, i, :], Act.Relu, bias=mxb2[:, i:i + 1])

                # -------- XBAR-transpose one-hots, gather via matmul --------
                ohT = main.tile([128, BG, 128], BF, tag="ohT")
                oh2T = main.tile([128, BG, 128], BF, tag="oh2T")
                nc.sync.dma_start_transpose(ohT[:], oh[:].rearrange("m i n -> m (i n)"))
                nc.sync.dma_start_transpose(oh2T[:], oh2[:].rearrange("m i n -> m (i n)"))

                exPS = ps_ex.tile([128, BG, 14], FP, tag="exPS")
                for i in range(BG):
                    g = g0 + i
                    nc.tensor.matmul(exPS[:, i, 0:10], ohT[:, i, :], T12[:, g, :], start=True, stop=True)
                    nc.tensor.matmul(exPS[:, i, 10:14], oh2T[:, i, :], T3c[:, g, :], start=True, stop=True)
                V.tensor_copy(ex[:, g0:g0 + BG, :], exPS[:])

            # ---------------- tail (128 x G f32 ops) ----------------
            tl = prep

            def TT(name, a, bb, op, eng=V):
                r = tl.tile([128, G], FP, tag=name)
                eng.tensor_tensor(r[:], a, bb, op)
                return r

            r0 = TT("r0", ex[:, :, 4], ex[:, :, 5], Alu.add)
            r1 = TT("r1", ex[:, :, 6], ex[:, :, 7], Alu.add, Gp)
            r2t = TT("r2t", ex[:, :, 8], ex[:, :, 9], Alu.add)
            v1 = TT("v1", pxP[:], r0[:], Alu.mult, Gp)
            v2 = TT("v2", pyP[:], r1[:], Alu.mult)
            gst = TT("gst", v1[:], v2[:], Alu.add, Gp)
            gst = TT("gst2", gst[:], r2t[:], Alu.add)
            c2t = tl.tile([128, G], FP, tag="c2t")
            S.activation(c2t[:], gst[:], Act.Copy, bias=M23)
            c3t = tl.tile([128, G], FP, tag="c3t")
            S.activation(c3t[:], c2t[:], Act.Copy, bias=-M23)
            tst = tl.tile([128, G], FP, tag="tst")
            V.tensor_scalar(tst[:], c3t[:], 0.0, 9.0, Alu.max, Alu.min)
            m1 = TT("m1", tst[:], ex[:, :, 2], Alu.mult, Gp)
            tgx = tl.tile([128, G], FP, tag="tgx")
            V.scalar_tensor_tensor(tgx[:], m1[:], 0.1, ex[:, :, 0], Alu.mult, Alu.add)
            m2 = TT("m2", tst[:], ex[:, :, 3], Alu.mult, Gp)
            tgy = tl.tile([128, G], FP, tag="tgy")
            V.scalar_tensor_tensor(tgy[:], m2[:], 0.1, ex[:, :, 1], Alu.mult, Alu.add)

            def smooth_l1_sum(pred_x, pred_y, tx, ty, px_, py_, name):
                acc = None
                for ci, (pr, tt_, pp) in enumerate(((pred_x, tx, px_), (pred_y, ty, py_))):
                    sfx = name + str(ci)
                    e1 = TT(sfx + "e1", tt_, pp, Alu.subtract, Gp)
                    dfe = tl.tile([128, G], FP, tag=sfx + "dfe")
                    V.scalar_tensor_tensor(dfe[:], e1[:], -0.25, pr, Alu.mult, Alu.add)
                    ad = tl.tile([128, G], FP, tag=sfx + "ad")
                    S.activation(ad[:], dfe[:], Act.Abs)
                    m = tl.tile([128, G], FP, tag=sfx + "m")
                    V.tensor_scalar(m[:], ad[:], BETA, None, Alu.min)
                    uu = tl.tile([128, G], FP, tag=sfx + "u")
                    V.scalar_tensor_tensor(uu[:], m[:], -0.5, ad[:], Alu.mult, Alu.add)
                    sl = tl.tile([128, G], FP, tag=sfx + "sl")
                    V.scalar_tensor_tensor(sl[:], m[:], 4.0, uu[:], Alu.mult, Alu.mult)
                    if acc is None:
                        acc = sl
                    else:
                        acc = TT(name + "acc", acc[:], sl[:], Alu.add, Gp)
                return acc

            s1 = smooth_l1_sum(oxP[:], oyP[:], tgx[:], tgy[:], pxP[:], pyP[:], "i1")
            s2 = smooth_l1_sum(ex[:, :, 12], ex[:, :, 13], kxP[:], kyP[:],
                               ex[:, :, 10], ex[:, :, 11], "i2")
            s2 = TT("s2m", s2[:], mkP[:], Alu.mult)

            s1r = tl.tile([128, 1], FP, tag="s1r")
            V.tensor_reduce(s1r[:], s1[:], Ax.X, Alu.add)
            s2r = tl.tile([128, 1], FP, tag="s2r")
            V.tensor_reduce(s2r[:], s2[:], Ax.X, Alu.add)
            sboth = tl.tile([128, 2], FP, tag="sboth")
            V.tensor_copy(sboth[:, 0:1], s1r[:])
            V.tensor_copy(sboth[:, 1:2], s2r[:])
            sc_ps = ps_out.tile([2, 1], FP, tag="sc_ps")
            nc.tensor.matmul(sc_ps[:], sboth[:], onesc[:], start=True, stop=True)
            outsb = tl.tile([2, 1], FP, tag="outsb")
            V.tensor_copy(outsb[:], sc_ps[:])
            nc.sync.dma_start(out_d[:].rearrange("(a b) -> a b", b=1), outsb[:])
            ps_out_cm.__exit__(None, None, None)
            ps_ex_cm.__exit__(None, None, None)
            ps_d2_cm.__exit__(None, None, None)
            ps_grid_cm.__exit__(None, None, None)

    return nc


_CACHE = {}


def _get_program():
    if "nc" not in _CACHE:
        nc = bacc.Bacc("TRN2", target_bir_lowering=False, num_devices=N_CORES)
        pc_d = nc.declare_dram_parameter("pc", [G, P, 2], dt.float32, isOutput=False)
        po_d = nc.declare_dram_parameter("po", [G, P, 2], dt.float32, isOutput=False)
        gc_d = nc.declare_dram_parameter("gc", [G, P, 2], dt.float32, isOutput=False)
        gk_d = nc.declare_dram_parameter("gk", [G, P, 2], dt.float32, isOutput=False)
        mk_d = nc.declare_dram_parameter("mk", [G, P], dt.float32, isOutput=False)
        out_d = nc.declare_dram_parameter("out", [2], dt.float32, isOutput=True)
        _build(nc, pc_d[:], po_d[:], gc_d[:], gk_d[:], mk_d[:], out_d[:])
        nc.compile()
        _CACHE["nc"] = nc
    return _CACHE["nc"]


def _in_maps(inputs):
    pc = np.ascontiguousarray(inputs["pred_contours"], dtype=np.float32)
    po = np.ascontiguousarray(inputs["pred_offsets"], dtype=np.float32)
    gc = np.ascontiguousarray(inputs["gt_contours"], dtype=np.float32)
    gk = np.ascontiguousarray(inputs["gt_key_points"], dtype=np.float32)
    mk = np.ascontiguousarray(inputs["gt_key_points_mask"]).astype(np.float32)
    maps = []
    for c in range(N_CORES):
        s = slice(c * G, (c + 1) * G)
        maps.append({
            "pc": pc[s], "po": po[s], "gc": gc[s], "gk": gk[s], "mk": mk[s],
        })
    return maps


def kernel(pred_contours, pred_offsets, gt_contours, gt_key_points, gt_key_points_mask,
           _results_hook=None):
    inputs = {
        "pred_contours": pred_contours,
        "pred_offsets": pred_offsets,
        "gt_contours": gt_contours,
        "gt_key_points": gt_key_points,
        "gt_key_points_mask": gt_key_points_mask,
    }
    nc = _get_program()
    res = run_bass_kernel_spmd(nc, _in_maps(inputs), list(range(N_CORES)))
    if _results_hook is not None:
        _results_hook(res)
    s1 = f32(0.0)
    s2 = f32(0.0)
    for r in res.results:
        s1 = f32(s1 + f32(r["out"][0]))
        s2 = f32(s2 + f32(r["out"][1]))
    cnt1 = f32(N * P * 2)
    cnt2 = f32(max(float(np.sum(gt_key_points_mask != 0)) * 2.0, 1.0))
    loss = f32(f32(s1 / cnt1) * f32(0.5) + f32(s2 / cnt2) * f32(0.5))
    return np.asarray(loss, dtype=np.float32)


# revision 24
# speedup vs baseline: 4.3745x; 1.0301x over previous
"""Trainium2 Bass kernel for nn_DMLoss (contour matching loss), 8-core data parallel.

v4: block-diagonal bf16 split-precision matmuls with the C1 rounding offset
folded in as extra contraction rows, XBAR DMA-transposed one-hots, combined
packed argmin reduce, engine-balanced chain, piecewise prep DMA overlap.

Per instance (P=128 points, TIME=10):
  item1: nearest of 1280 interpolated gt points per pred point.  Segment n
    spans A_n = gt[n-1]..gt[n]; g' = 10*u - 0.5 with u = <p-A,D>/|D|^2; best
    discrete t = clamp(round(g'), 0, 9); dist^2 = |p-A|^2 + (e/100)*t*(t-2g').
    TensorE produces grids g', opsC1 = SC*(2<p,A> - |A|^2) + C1 and
    erep = -SC*e/100 (bf16 hi/lo split rows, C1 = C1H + C1L exactly).
    dq = (q*erep + opsC1) - CQ lands exactly on the 128-grid; pk = dq - n
    packs quantized distance + segment index; reduce-max = argmin;
    exact 0/1 one-hots gather segment data via XBAR transpose + bf16 matmul.
  item2: same machinery without interpolation (nearest pred per key point).

Output per core: [sum_loss1, sum_loss2]; host divides by counts and combines.
"""
import sys

for _p in ("/opt/trn_rl_repo",):
    if _p not in sys.path:
        sys.path.insert(0, _p)

import numpy as np

import concourse.bass as bass
import concourse.tile as tile
from concourse import bacc, mybir
from concourse.bass_utils import run_bass_kernel_spmd

dt = mybir.dt
Alu = mybir.AluOpType
Ax = mybir.AxisListType
Act = mybir.ActivationFunctionType
f32 = np.float32

N_CORES = 8
N, P = 256, 128
G = N // N_CORES          # instances per core = 32
BG = 4                    # instances per block
NB = G // BG              # 8 blocks
SC = 131072.0             # distance scale (quantum = 128/SC = 2^-10)
SHIFT = 48.0
BETA = 0.25               # smooth-l1 beta = 1/STRIDE
CQ = float(2 ** 30 + 2 ** 25)       # 1107296256
C1 = CQ - SC * SHIFT                # 1101004800
C1H = 1098907648.0                  # bf16-exact hi part of C1
C1L = 2097152.0                     # bf16-exact lo part (C1H + C1L == C1)
M23 = 8388608.0

# SPL slab regions, 10 row-types per operand (k = 10*i + t within a block):
#   lhsT: [x_hi, y_hi, x_lo, y_lo, x_hi2, y_hi2, 1, 1, 1, 1]
#   rhs:  [u_hi, v_hi, u_hi2, v_hi2, u_lo, v_lo, c_hi, c_lo, w8, w9]
# pairing t: xh*uh + yh*vh + xl*uh + yl*vh + xh*ul + yh*vl + ch + cl + w8 + w9
P0 = 0
K0 = 10
RG = 20         # w8 = w9 = 0
RO = 30         # w8, w9 = C1H, C1L
RQ = 40         # w8, w9 = C1H, C1L
RE = 50         # [z z z z z z er_hi er_lo z z]
TB = 60         # ax_hi ay_hi dx_hi dy_hi ox_hi oy_hi
NSLAB = 74
KB = 10 * BG    # block contraction rows = 40


def _build(nc, pc_d, po_d, gc_d, gk_d, mk_d, out_d):
    FP = dt.float32
    BF = dt.bfloat16

    with tile.TileContext(nc) as tc:
        with (
            tc.tile_pool(name="const", bufs=1) as cpool,
            tc.tile_pool(name="prep", bufs=1) as prep,
            tc.tile_pool(name="oper", bufs=1) as oper,
            tc.tile_pool(name="main", bufs=3) as main,
            tc.tile_pool(name="keep", bufs=1) as keep,
        ):
            V, Gp, S = nc.vector, nc.gpsimd, nc.scalar

            # ---------------- constants ----------------
            iota_i = cpool.tile([128, 128], dt.int32)
            Gp.iota(iota_i[:], pattern=[[1, 128]], channel_multiplier=0)
            iotaF = cpool.tile([128, 128], FP, tag="iotaF")
            V.tensor_copy(iotaF[:], iota_i[:])
            iotaC_i = cpool.tile([128, 1], dt.int32)
            Gp.iota(iotaC_i[:], pattern=[[0, 1]], channel_multiplier=1)
            iotaC = cpool.tile([128, 1], FP, tag="iotaC")
            V.tensor_copy(iotaC[:], iotaC_i[:])
            ident32F = cpool.tile([32, 32], FP, tag="ident32F")
            V.tensor_scalar(ident32F[:], iotaF[0:32, 0:32], iotaC[0:32], None, Alu.is_equal)
            onesc = cpool.tile([128, 1], FP, tag="onesc")
            Gp.memset(onesc[:], 1.0)

            # zeros region for rhs backfill
            zero_d = nc.dram_tensor("zeros", [KB, 32, 512], BF)
            ztile = prep.tile([KB, 512], BF, tag="ztile")
            V.memset(ztile[:], 0.0)
            nc.scalar.dma_start(zero_d[:, 0, :], ztile[:])
            nc.scalar.dma_start(zero_d[:, 1, :], zero_d[:, 0, :])
            nc.scalar.dma_start(zero_d[:, 2:4, :], zero_d[:, 0:2, :])
            nc.scalar.dma_start(zero_d[:, 4:8, :], zero_d[:, 0:4, :])
            nc.scalar.dma_start(zero_d[:, 8:16, :], zero_d[:, 0:8, :])
            nc.scalar.dma_start(zero_d[:, 16:32, :], zero_d[:, 0:16, :])

            # ---------------- contiguous input loads ----------------
            pc_i = prep.tile([32, 128, 2], FP, tag="pc_i")
            po_i = prep.tile([32, 128, 2], FP, tag="po_i")
            gc_i = prep.tile([32, 128, 2], FP, tag="gc_i")
            gk_i = prep.tile([32, 128, 2], FP, tag="gk_i")
            mk_i = prep.tile([32, 128], FP, tag="mk_i")
            a_i = prep.tile([32, 128, 2], FP, tag="a_i")
            nc.sync.dma_start(pc_i[:], pc_d[:, :, :])
            nc.sync.dma_start(po_i[:], po_d[:, :, :])
            nc.sync.dma_start(gc_i[:], gc_d[:, :, :])
            nc.scalar.dma_start(gk_i[:], gk_d[:, :, :])
            nc.scalar.dma_start(mk_i[:], mk_d[:, :])
            nc.sync.dma_start(a_i[:, 1:128, :], gc_d[:, 0:127, :])
            nc.scalar.dma_start(a_i[:, 0:1, :], gc_d[:, 127:128, :])

            # ---------------- SPL slabs + fine-grained stores ----------------
            SPL = prep.tile([32, NSLAB, 128], BF, tag="SPL")
            slab_d = nc.dram_tensor("slabs", [32, NSLAB, 128], BF)

            def split_pair(src_, s_hi, s_lo):
                # contiguous bf16 writes; channel-major strided reads
                srcT = src_.rearrange("g q c -> g c q")
                S.activation(SPL[:, s_hi:s_hi + 2, :], srcT, Act.Copy)
                if s_lo is not None:
                    V.tensor_tensor(SPL[:, s_lo:s_lo + 2, :], srcT,
                                    SPL[:, s_hi:s_hi + 2, :], Alu.subtract)

            def split_one(src_, s_hi, s_lo):
                S.activation(SPL[:, s_hi, :], src_, Act.Copy)
                if s_lo is not None:
                    V.tensor_tensor(SPL[:, s_lo, :], src_, SPL[:, s_hi, :], Alu.subtract)

            # ---- phase A: pc/gk-dependent slabs (no gc geometry needed) ----
            Gp.memset(SPL[:, P0 + 6:P0 + 10, :], 1.0)
            Gp.memset(SPL[:, K0 + 6:K0 + 10, :], 1.0)
            Gp.memset(SPL[:, RQ + 8, :], C1H)
            Gp.memset(SPL[:, RQ + 9, :], C1L)
            split_pair(pc_i[:], P0 + 0, P0 + 2)
            split_pair(gk_i[:], K0 + 0, K0 + 2)
            V.tensor_copy(SPL[:, P0 + 4:P0 + 6, :], SPL[:, P0 + 0:P0 + 2, :])
            V.tensor_copy(SPL[:, K0 + 4:K0 + 6, :], SPL[:, K0 + 0:K0 + 2, :])
            q_01 = prep.tile([32, 128, 2], FP, tag="q_01")
            S.activation(q_01[:], pc_i[:], Act.Copy, scale=2.0 * SC)
            psq = prep.tile([32, 128, 2], FP, tag="psq")
            Gp.tensor_tensor(psq[:], pc_i[:], pc_i[:], Alu.mult)
            zP = prep.tile([32, 128], FP, tag="zP")
            Gp.tensor_tensor(zP[:], psq[:, :, 0], psq[:, :, 1], Alu.add)
            q2 = prep.tile([32, 128], FP, tag="q2")
            S.activation(q2[:], zP[:], Act.Copy, scale=-SC)
            split_pair(q_01[:], RQ + 0, RQ + 4)
            split_one(q2[:], RQ + 6, RQ + 7)
            V.tensor_copy(SPL[:, RQ + 2:RQ + 4, :], SPL[:, RQ + 0:RQ + 2, :])
            nc.sync.dma_start(slab_d[:, 0:20, :], SPL[:, 0:20, :])
            nc.scalar.dma_start(slab_d[:, RQ:RQ + 10, :], SPL[:, RQ:RQ + 10, :])

            # ---- phase B: gc geometry ----
            d_i = prep.tile([32, 128, 2], FP, tag="d_i")
            V.tensor_tensor(d_i[:], gc_i[:], a_i[:], Alu.subtract)
            dsq = prep.tile([32, 128, 2], FP, tag="dsq")
            Gp.tensor_tensor(dsq[:], d_i[:], d_i[:], Alu.mult)
            e = prep.tile([32, 128], FP, tag="e")
            Gp.tensor_tensor(e[:], dsq[:, :, 0], dsq[:, :, 1], Alu.add)
            einv = prep.tile([32, 128], FP, tag="einv")
            V.reciprocal(einv[:], e[:])
            t_ad = prep.tile([32, 128, 2], FP, tag="t_ad")
            Gp.tensor_tensor(t_ad[:], a_i[:], d_i[:], Alu.mult)
            a2 = prep.tile([32, 128], FP, tag="a2")
            Gp.tensor_tensor(a2[:], t_ad[:, :, 0], t_ad[:, :, 1], Alu.add)
            asq = prep.tile([32, 128, 2], FP, tag="asq")
            Gp.tensor_tensor(asq[:], a_i[:], a_i[:], Alu.mult)
            zA = prep.tile([32, 128], FP, tag="zA")
            Gp.tensor_tensor(zA[:], asq[:, :, 0], asq[:, :, 1], Alu.add)

            er = prep.tile([32, 128], FP, tag="er")
            S.activation(er[:], e[:], Act.Copy, scale=-SC / 100.0)
            Gp.memset(SPL[:, RE + 0:RE + 6, :], 0.0)
            Gp.memset(SPL[:, RE + 8:RE + 10, :], 0.0)
            split_one(er[:], RE + 6, RE + 7)
            nc.sync.dma_start(slab_d[:, RE:RE + 10, :], SPL[:, RE:RE + 10, :])

            r_01 = prep.tile([32, 128, 2], FP, tag="r_01")
            V.scalar_tensor_tensor(r_01[:, :, 0], d_i[:, :, 0], 10.0, einv[:], Alu.mult, Alu.mult)
            V.scalar_tensor_tensor(r_01[:, :, 1], d_i[:, :, 1], 10.0, einv[:], Alu.mult, Alu.mult)
            r2 = prep.tile([32, 128], FP, tag="r2")
            V.scalar_tensor_tensor(r2[:], a2[:], -10.0, einv[:], Alu.mult, Alu.mult)
            Gp.memset(SPL[:, RG + 8:RG + 10, :], 0.0)
            split_pair(r_01[:], RG + 0, RG + 4)
            split_one(r2[:], RG + 6, RG + 7)
            V.tensor_copy(SPL[:, RG + 2:RG + 4, :], SPL[:, RG + 0:RG + 2, :])
            nc.scalar.dma_start(slab_d[:, RG:RG + 10, :], SPL[:, RG:RG + 10, :])

            o_01 = prep.tile([32, 128, 2], FP, tag="o_01")
            S.activation(o_01[:], a_i[:], Act.Copy, scale=2.0 * SC)
            o2 = prep.tile([32, 128], FP, tag="o2")
            S.activation(o2[:], zA[:], Act.Copy, scale=-SC)
            Gp.memset(SPL[:, RO + 8, :], C1H)
            Gp.memset(SPL[:, RO + 9, :], C1L)
            split_pair(o_01[:], RO + 0, RO + 4)
            split_one(o2[:], RO + 6, RO + 7)
            V.tensor_copy(SPL[:, RO + 2:RO + 4, :], SPL[:, RO + 0:RO + 2, :])
            nc.sync.dma_start(slab_d[:, RO:RO + 10, :], SPL[:, RO:RO + 10, :])

            # ---- table region ----
            split_pair(a_i[:], TB + 0, None)
            split_pair(d_i[:], TB + 2, None)
            split_pair(po_i[:], TB + 12, None)
            V.tensor_copy(SPL[:, TB + 4:TB + 6, :], SPL[:, RG + 0:RG + 5:4, :])
            V.tensor_copy(SPL[:, TB + 6:TB + 8, :], SPL[:, RG + 1:RG + 6:4, :])
            V.tensor_copy(SPL[:, TB + 8:TB + 10, :], SPL[:, RG + 6:RG + 8, :])
            V.tensor_copy(SPL[:, TB + 10:TB + 12, :], SPL[:, P0 + 0:P0 + 2, :])


            # lhsA[t, s, b, p] = slab (s=0 pred, s=1 key); rhsA[t, r, b, n]
            lhsA = oper.tile([KB, 2, 8, 128], BF, tag="lhsA")
            rhsA = oper.tile([KB, 4, 8, 512], BF, tag="rhsA")
            nc.sync.dma_start(
                rhsA[:], zero_d[:, :, :].rearrange("t (r b) n -> t r b n", r=4))

            for i in range(BG):
                for s in range(2):
                    eng = nc.sync if ((i + s) % 2 == 0) else nc.scalar
                    eng.dma_start(
                        lhsA[10 * i:10 * i + 10, s, :, :],
                        slab_d[i:32:4, 10 * s:10 * s + 10, :].rearrange("b t p -> t b p"),
                    )
                for r in range(4):
                    eng = nc.sync if ((i + r) % 2 == 0) else nc.scalar
                    eng.dma_start(
                        rhsA[10 * i:10 * i + 10, r, :, 128 * i:128 * (i + 1)],
                        slab_d[i:32:4, 20 + 10 * r:30 + 10 * r, :].rearrange("b t p -> t b p"),
                    )

            # ---------------- gather tables (one batched XBAR transpose) -----
            # in (32, 14*128) -> out stgB[n, j, g] = SPL[g, TB+j, n]
            T12 = keep.tile([128, G, 10], BF, tag="T12")
            T3c = keep.tile([128, G, 4], BF, tag="T3c")
            stgB = keep.tile([128, 14, 32], BF, tag="stgB")
            nc.scalar.dma_start_transpose(
                stgB[:], SPL[:, TB:TB + 14, :].rearrange("g j n -> g (j n)"))
            # T12 slots: [ax, ay, dx, dy, r0h, r0l, r1h, r1l, r2h, r2l] = j 0..9
            V.tensor_copy(T12[:], stgB[:, 0:10, :].rearrange("n j g -> n g j"))
            # T3c slots: [px, py, ox, oy] = j 10..13
            V.tensor_copy(T3c[:], stgB[:, 10:14, :].rearrange("n j g -> n g j"))

            # ---------------- f32 transposes for the tail --------------------
            pxP = keep.tile([128, G], FP, tag="pxP")
            pyP = keep.tile([128, G], FP, tag="pyP")
            oxP = keep.tile([128, G], FP, tag="oxP")
            oyP = keep.tile([128, G], FP, tag="oyP")
            kxP = keep.tile([128, G], FP, tag="kxP")
            kyP = keep.tile([128, G], FP, tag="kyP")
            mkP = keep.tile([128, G], FP, tag="mkP")
            with tc.tile_pool(name="ps_prep", bufs=3, space="PSUM") as ps_prep:
                for dst, src in ((pxP, pc_i[:, :, 0]), (pyP, pc_i[:, :, 1]),
                                 (oxP, po_i[:, :, 0]), (oyP, po_i[:, :, 1]),
                                 (kxP, gk_i[:, :, 0]), (kyP, gk_i[:, :, 1]),
                                 (mkP, mk_i[:])):
                    fps = ps_prep.tile([128, 32], FP, tag="tpsF")
                    nc.tensor.transpose(fps[:], src, ident32F[:])
                    S.activation(dst[:], fps[:], Act.Copy)

            exA = keep.tile([128, 16, 14], FP, tag="exA")
            exB = keep.tile([128, 16, 14], FP, tag="exB")
            tl = prep

            def tail_chunk(ex, gl, sfx):
                gs = slice(gl, gl + 16)

                def TT(name, a, bb, op, eng=V):
                    r = tl.tile([128, 16], FP, tag=sfx + name)
                    eng.tensor_tensor(r[:], a, bb, op)
                    return r

                r0 = TT("r0", ex[:, :, 4], ex[:, :, 5], Alu.add)
                r1 = TT("r1", ex[:, :, 6], ex[:, :, 7], Alu.add, Gp)
                r2t = TT("r2t", ex[:, :, 8], ex[:, :, 9], Alu.add)
                v1 = TT("v1", pxP[:, gs], r0[:], Alu.mult, Gp)
                v2 = TT("v2", pyP[:, gs], r1[:], Alu.mult)
                gst = TT("gst", v1[:], v2[:], Alu.add, Gp)
                gst = TT("gst2", gst[:], r2t[:], Alu.add)
                c2t = tl.tile([128, 16], FP, tag=sfx + "c2t")
                S.activation(c2t[:], gst[:], Act.Copy, bias=M23)
                c3t = tl.tile([128, 16], FP, tag=sfx + "c3t")
                S.activation(c3t[:], c2t[:], Act.Copy, bias=-M23)
                tst = tl.tile([128, 16], FP, tag=sfx + "tst")
                V.tensor_scalar(tst[:], c3t[:], 0.0, 9.0, Alu.max, Alu.min)
                m1 = TT("m1", tst[:], ex[:, :, 2], Alu.mult, Gp)
                tgx = tl.tile([128, 16], FP, tag=sfx + "tgx")
                V.scalar_tensor_tensor(tgx[:], m1[:], 0.1, ex[:, :, 0], Alu.mult, Alu.add)
                m2 = TT("m2", tst[:], ex[:, :, 3], Alu.mult, Gp)
                tgy = tl.tile([128, 16], FP, tag=sfx + "tgy")
                V.scalar_tensor_tensor(tgy[:], m2[:], 0.1, ex[:, :, 1], Alu.mult, Alu.add)

                def smooth_l1_sum(pred_x, pred_y, tx, ty, px_, py_, name):
                    acc = None
                    for ci, (pr, tt_, pp) in enumerate(((pred_x, tx, px_), (pred_y, ty, py_))):
                        s2fx = sfx + name + str(ci)
                        e1 = TT(name + str(ci) + "e1", tt_, pp, Alu.subtract, Gp)
                        dfe = tl.tile([128, 16], FP, tag=s2fx + "dfe")
                        V.scalar_tensor_tensor(dfe[:], e1[:], -0.25, pr, Alu.mult, Alu.add)
                        ad = tl.tile([128, 16], FP, tag=s2fx + "ad")
                        S.activation(ad[:], dfe[:], Act.Abs)
                        m = tl.tile([128, 16], FP, tag=s2fx + "m")
                        V.tensor_scalar(m[:], ad[:], BETA, None, Alu.min)
                        uu = tl.tile([128, 16], FP, tag=s2fx + "u")
                        V.scalar_tensor_tensor(uu[:], m[:], -0.5, ad[:], Alu.mult, Alu.add)
                        sl = tl.tile([128, 16], FP, tag=s2fx + "sl")
                        V.scalar_tensor_tensor(sl[:], m[:], 4.0, uu[:], Alu.mult, Alu.mult)
                        if acc is None:
                            acc = sl
                        else:
                            acc = TT(name + "acc", acc[:], sl[:], Alu.add, Gp)
                    return acc

                s1 = smooth_l1_sum(oxP[:, gs], oyP[:, gs], tgx[:], tgy[:],
                                   pxP[:, gs], pyP[:, gs], "i1")
                s2 = smooth_l1_sum(ex[:, :, 12], ex[:, :, 13], kxP[:, gs], kyP[:, gs],
                                   ex[:, :, 10], ex[:, :, 11], "i2")
                s2 = TT("s2m", s2[:], mkP[:, gs], Alu.mult)
                s1r = tl.tile([128, 1], FP, tag=sfx + "s1r")
                V.tensor_reduce(s1r[:], s1[:], Ax.X, Alu.add)
                s2r = tl.tile([128, 1], FP, tag=sfx + "s2r")
                V.tensor_reduce(s2r[:], s2[:], Ax.X, Alu.add)
                return s1r, s2r


            iotaB = iotaF[:].rearrange("p (o q) -> p o q", o=1).broadcast_to([128, BG, 128])

            ps_grid_cm = tc.tile_pool(name="ps_grid", bufs=1, space="PSUM")
            ps_d2_cm = tc.tile_pool(name="ps_d2", bufs=2, space="PSUM")
            ps_ex_cm = tc.tile_pool(name="ps_ex", bufs=2, space="PSUM")
            ps_out_cm = tc.tile_pool(name="ps_out", bufs=1, space="PSUM")
            ps_grid = ps_grid_cm.__enter__()
            ps_d2 = ps_d2_cm.__enter__()
            ps_ex = ps_ex_cm.__enter__()
            ps_out = ps_out_cm.__enter__()

            # ---------------- main loop ----------------
            for b in range(NB):
                g0 = b * BG
                gps = ps_grid.tile([128, BG, 128], FP, tag="gps")
                ops = ps_grid.tile([128, BG, 128], FP, tag="ops")
                erep = ps_grid.tile([128, BG, 128], FP, tag="erep")
                d2ps = ps_d2.tile([128, BG, 128], FP, tag="d2ps")
                gv = gps[:].rearrange("p i n -> p (i n)")
                ov = ops[:].rearrange("p i n -> p (i n)")
                ev = erep[:].rearrange("p i n -> p (i n)")
                dv = d2ps[:].rearrange("p i n -> p (i n)")
                nc.tensor.matmul(gv, lhsA[:, 0, b, :], rhsA[:, 0, b, :], start=True, stop=True)
                nc.tensor.matmul(ov, lhsA[:, 0, b, :], rhsA[:, 1, b, :], start=True, stop=True)
                nc.tensor.matmul(ev, lhsA[:, 0, b, :], rhsA[:, 3, b, :], start=True, stop=True)
                nc.tensor.matmul(dv, lhsA[:, 1, b, :], rhsA[:, 2, b, :], start=True, stop=True)

                # -------- item1: t = clamp(round(g'), 0, 9) --------
                s2t = main.tile([128, BG, 128], FP, tag="s2t")
                S.activation(s2t[:], gps[:], Act.Copy, bias=M23)
                s3t = main.tile([128, BG, 128], FP, tag="s3t")
                S.activation(s3t[:], s2t[:], Act.Copy, bias=-M23)
                t = main.tile([128, BG, 128], FP, tag="t")
                V.tensor_scalar(t[:], s3t[:], 0.0, 9.0, Alu.max, Alu.min)
                hq = main.tile([128, BG, 128], FP, tag="hq")
                V.scalar_tensor_tensor(hq[:], gps[:], -2.0, t[:], Alu.mult, Alu.add)
                q = main.tile([128, BG, 128], FP, tag="q")
                Gp.tensor_tensor(q[:], hq[:], t[:], Alu.mult)
                vE = main.tile([128, BG, 128], FP, tag="vE")
                V.tensor_tensor(vE[:], q[:], erep[:], Alu.mult)
                dqA = main.tile([128, BG, 128], FP, tag="dqA")
                V.tensor_tensor(dqA[:], vE[:], ops[:], Alu.add)
                dq = main.tile([128, BG, 128], FP, tag="dq")
                S.activation(dq[:], dqA[:], Act.Copy, bias=-CQ)
                dq2 = main.tile([128, BG, 128], FP, tag="dq2")
                S.activation(dq2[:], d2ps[:], Act.Copy, bias=-CQ)

                pkN = main.tile([128, BG, 128], FP, tag="pkN")
                Gp.tensor_tensor(pkN[:], dq[:], iotaB, Alu.subtract)
                pk2 = main.tile([128, BG, 128], FP, tag="pk2")
                Gp.tensor_tensor(pk2[:], dq2[:], iotaB, Alu.subtract)
                mx = main.tile([128, BG], FP, tag="mx")
                V.tensor_reduce(mx[:], pkN[:], Ax.X, Alu.max)
                mx2 = main.tile([128, BG], FP, tag="mx2")
                V.tensor_reduce(mx2[:], pk2[:], Ax.X, Alu.max)
                mxb1 = main.tile([128, BG], FP, tag="mxb1")
                V.tensor_scalar(mxb1[:], mx[:], -1.0, 1.0, Alu.mult, Alu.add)
                mxb2 = main.tile([128, BG], FP, tag="mxb2")
                V.tensor_scalar(mxb2[:], mx2[:], -1.0, 1.0, Alu.mult, Alu.add)

                oh = main.tile([128, BG, 128], BF, tag="oh")
                oh2 = main.tile([128, BG, 128], BF, tag="oh2")
                for i in range(BG):
                    if i % 2 == 0:
                        V.tensor_scalar(oh[:, i, :], pkN[:, i, :], mx[:, i:i + 1], None, Alu.is_equal)
                    else:
                        S.activation(oh[:, i, :], pkN[:, i, :], Act.Relu, bias=mxb1[:, i:i + 1])
                    S.activation(oh2[:, i, :], pk2[:, i, :], Act.Relu, bias=mxb2[:, i:i + 1])

                # -------- XBAR-transpose one-hots, gather via matmul --------
                ohT = main.tile([128, BG, 128], BF, tag="ohT")
                oh2T = main.tile([128, BG, 128], BF, tag="oh2T")
                nc.sync.dma_start_transpose(ohT[:], oh[:].rearrange("m i n -> m (i n)"))
                nc.sync.dma_start_transpose(oh2T[:], oh2[:].rearrange("m i n -> m (i n)"))

                exPS = ps_ex.tile([128, BG, 14], FP, tag="exPS")
                for i in range(BG):
                    g = g0 + i
                    nc.tensor.matmul(exPS[:, i, 0:10], ohT[:, i, :], T12[:, g, :], start=True, stop=True)
                    nc.tensor.matmul(exPS[:, i, 10:14], oh2T[:, i, :], T3c[:, g, :], start=True, stop=True)
                ext = exA if b < 4 else exB
                S.activation(ext[:, (g0 % 16):(g0 % 16) + BG, :], exPS[:], Act.Copy)
                if b == 3:
                    _TAILA = tail_chunk(exA, 0, "A")

            # ---------------- tail (chunked, overlaps main loop) -------------
            s1a, s2a = _TAILA
            s1b, s2b = tail_chunk(exB, 16, "B")
            sboth = tl.tile([128, 2], FP, tag="sboth")
            V.tensor_tensor(sboth[:, 0:1], s1a[:], s1b[:], Alu.add)
            V.tensor_tensor(sboth[:, 1:2], s2a[:], s2b[:], Alu.add)
            sc_ps = ps_out.tile([2, 1], FP, tag="sc_ps")
            nc.tensor.matmul(sc_ps[:], sboth[:], onesc[:], start=True, stop=True)
            outsb = tl.tile([2, 1], FP, tag="outsb")
            V.tensor_copy(outsb[:], sc_ps[:])
            nc.sync.dma_start(out_d[:].rearrange("(a b) -> a b", b=1), outsb[:])
            ps_out_cm.__exit__(None, None, None)
            ps_ex_cm.__exit__(None, None, None)
            ps_d2_cm.__exit__(None, None, None)
            ps_grid_cm.__exit__(None, None, None)

    return nc


_CACHE = {}


def _get_program():
    if "nc" not in _CACHE:
        nc = bacc.Bacc("TRN2", target_bir_lowering=False, num_devices=N_CORES)
        pc_d = nc.declare_dram_parameter("pc", [G, P, 2], dt.float32, isOutput=False)
        po_d = nc.declare_dram_parameter("po", [G, P, 2], dt.float32, isOutput=False)
        gc_d = nc.declare_dram_parameter("gc", [G, P, 2], dt.float32, isOutput=False)
        gk_d = nc.declare_dram_parameter("gk", [G, P, 2], dt.float32, isOutput=False)
        mk_d = nc.declare_dram_parameter("mk", [G, P], dt.float32, isOutput=False)
        out_d = nc.declare_dram_parameter("out", [2], dt.float32, isOutput=True)
        _build(nc, pc_d[:], po_d[:], gc_d[:], gk_d[:], mk_d[:], out_d[:])
        nc.compile()
        _CACHE["nc"] = nc
    return _CACHE["nc"]


def _in_maps(inputs):
    pc = np.ascontiguousarray(inputs["pred_contours"], dtype=np.float32)
    po = np.ascontiguousarray(inputs["pred_offsets"], dtype=np.float32)
    gc = np.ascontiguousarray(inputs["gt_contours"], dtype=np.float32)
    gk = np.ascontiguousarray(inputs["gt_key_points"], dtype=np.float32)
    mk = np.ascontiguousarray(inputs["gt_key_points_mask"]).astype(np.float32)
    maps = []
    for c in range(N_CORES):
        s = slice(c * G, (c + 1) * G)
        maps.append({
            "pc": pc[s], "po": po[s], "gc": gc[s], "gk": gk[s], "mk": mk[s],
        })
    return maps


def kernel(pred_contours, pred_offsets, gt_contours, gt_key_points, gt_key_points_mask,
           _results_hook=None):
    inputs = {
        "pred_contours": pred_contours,
        "pred_offsets": pred_offsets,
        "gt_contours": gt_contours,
        "gt_key_points": gt_key_points,
        "gt_key_points_mask": gt_key_points_mask,
    }
    nc = _get_program()
    res = run_bass_kernel_spmd(nc, _in_maps(inputs), list(range(N_CORES)))
    if _results_hook is not None:
        _results_hook(res)
    s1 = f32(0.0)
    s2 = f32(0.0)
    for r in res.results:
        s1 = f32(s1 + f32(r["out"][0]))
        s2 = f32(s2 + f32(r["out"][1]))
    cnt1 = f32(N * P * 2)
    cnt2 = f32(max(float(np.sum(gt_key_points_mask != 0)) * 2.0, 1.0))
    loss = f32(f32(s1 / cnt1) * f32(0.5) + f32(s2 / cnt2) * f32(0.5))
    return np.asarray(loss, dtype=np.float32)
